# revision 1
# baseline (speedup 1.0000x reference)
# Trainium2 Bass kernel for nn_CKDLoss: KD loss + virtual-outer-product L1/L2
# + Gram-matrix sub-losses, computed entirely on device.
#
# Strategy notes (sharding): total FLOPs after algorithmic reduction are tiny
# (the O(N^2)=1e9-pair L1 term collapses to a K=1024-bucket weighted-histogram
# contraction, O(N*(K1+K2)) work for N=32000), so the kernel is latency-bound,
# not throughput-bound. Cross-core collectives on trn2 have a ~20us latency
# floor, which exceeds the whole computation. Therefore every core runs the
# identical full computation on the full (replicated) inputs -- a degenerate
# but optimal "sharding" for this regime -- and the host takes core 0's
# output. No inter-core communication.
#
# L1 math: with u_n = log s_n - log t_n (all t,s > 0 softmax probs),
#   sum_{a,b} |t_a t_b - s_a s_b| = sum sign(-u_a-u_b) (t_a t_b - s_a s_b)
# Bucketize u on a symmetric grid of K=K1*K2 buckets, c = floor((u+UMAX)/w).
# A pair is positive iff c_a + c_b <= K-2 (the ambiguous diagonal c_a+c_b=K-1
# contributes O(w) error; empirically 1.4e-4 relative on the loss).
# With weighted bucket histograms W[c] = sum_{n: c_n=c} t_n and c = K2*hi+lo:
#   r[jh]    = sum_jl W2[jl,jh]
#   S1       = r^T M1 r                    (M1[a,b] = 1[a+b<=K1-2])
#   P[kl,kh] = sum_jl M1[jl,kl] W2[jl,K1-1-kh]
#   S2       = sum_{kl,kh} W2[kl,kh] P[kl,kh]
#   S_tt     = S1 + S2,   S_l1 = 2*S_tt - Ttot^2 - (2*S_ss - Stot^2)
# W2[lo,hi] is built as a sum of rank-1 outer products onehot_lo (x) onehot_hi
# on the PE (PSUM-accumulated over 250 chunks of 128 elements).
#
# Written in raw Bass (engine blocks + manual semaphores): this toolchain's
# codegen rejects the Tile layer's multi-wait instructions, and raw blocks
# allow standalone wait instructions plus explicitly choreographed overlap.
# PSUM discipline: never PE-write and DVE-read the same bank concurrently
# (ping-pong serialized through the vsem/tsem milestones below).

import numpy as np
from contextlib import ExitStack

B, C, NT = 64, 100, 5           # batch, classes, temps 1..5
N = B * C * NT                   # 32000 flattened cube elements
K1, K2 = 32, 8                   # two-level bucket split, K = 256
K = K1 * K2
UMAX = 16.0                      # u-range clamp; observed |u| < 6
INVW = K / (2.0 * UMAX)
ALPHA = 0.7
NCHUNK = N // 128                # 250 PE chunks
NG = 5                           # build groups (DVE -> PE pipelining)
GW = NCHUNK // NG                # 25 chunks per group
HC = NT * C // 2                 # 250 columns after the [64,500]->[128,250] fold


def _mkap(tensor_ap, dims, extra_off=0):
    import concourse.bass as bass
    return bass.AP(tensor=tensor_ap.tensor, offset=tensor_ap.offset + extra_off,
                   ap=[list(d) for d in dims])


def _ap3(ap, bcast_inner=None, bcast_mid=None):
    """Append/insert stride-0 dims on an AP: [P,F] -> [P,F,bi] or [P,bm,F]."""
    dims = [list(d) for d in ap.ap]
    if bcast_inner is not None:
        dims = dims + [[0, bcast_inner]]
    if bcast_mid is not None:
        dims = [dims[0], [0, bcast_mid]] + dims[1:]
    return _mkap(ap, dims)


def _fold_ap(ap):
    """View a [64, 500] AP as a [64, 2, 250] iteration for the fold DMA."""
    dims = [list(d) for d in ap.ap]
    p, f = dims
    assert f == [1, 2 * HC], f"unexpected ap {dims}"
    return _mkap(ap, [p, [HC, 2], [1, HC]])


def _rev_free(ap, n):
    """Reverse the (single) free dim of a [P, n] AP."""
    dims = [list(d) for d in ap.ap]
    assert dims[-1][0] == 1 and dims[-1][1] == n
    return _mkap(ap, dims[:-1] + [[-1, n]], extra_off=n - 1)


def build(sub_half=True):
    """sub_half: subtract 0.5 before the f32->i32 convert (round-to-nearest
    conversion then implements floor)."""
    import concourse.bass as bass
    from concourse import mybir

    dt = mybir.dt
    AL = mybir.AluOpType
    AF = mybir.ActivationFunctionType
    AX = mybir.AxisListType

    nc = bass.Bass()
    ls_d = nc.declare_dram_parameter("logits_student", [B, C], dt.float32, isOutput=False)
    lt_d = nc.declare_dram_parameter("logits_teacher", [B, C], dt.float32, isOutput=False)
    tg_d = nc.declare_dram_parameter("target", [B, 1], dt.int32, isOutput=False)
    out_d = nc.declare_dram_parameter("out", [1, 1], dt.float32, isOutput=True)

    ctx = ExitStack()
    _n = [0]

    def sb(shape, d=dt.float32):
        _n[0] += 1
        return ctx.enter_context(nc.sbuf_tensor(f"sb{_n[0]}", shape, d))

    def ps(shape):
        _n[0] += 1
        return ctx.enter_context(nc.psum_tensor(f"ps{_n[0]}", shape, dt.float32))

    with ctx:
        # constants
        ones = sb([128, 1])
        iota32p = sb([128, K1])
        iota100p = sb([64, C])
        ones32sq = sb([32, 32])
        m1 = sb([32, 32])
        m1lo = sb([K2, K2])
        ident64 = sb([64, 64])
        wrow = sb([64, NT])
        # inputs
        sl_ = sb([64, C])
        tl_ = sb([64, C])
        tg = sb([64, 1], dt.int32)
        # softmax stage
        m_s, m_t = sb([64, 1]), sb([64, 1])
        mb_s, mb_t = sb([64, NT]), sb([64, NT])
        se_s, se_t = sb([64, NT]), sb([64, NT])
        rs_s, rs_t = sb([64, NT]), sb([64, NT])
        lse_s, lse_t = sb([64, NT]), sb([64, NT])
        scube, tcube = sb([64, NT * C]), sb([64, NT * C])
        zd = sb([64, NT])
        d64 = sb([64, C])
        u64 = sb([64, NT * C])
        cf = sb([64, NT * C])
        ci64 = sb([64, NT * C], dt.int32)
        # folded
        ci128 = sb([128, HC], dt.int32)
        t128 = sb([128, HC])
        s128 = sb([128, HC])
        hi_i, lo_i = sb([128, HC], dt.int32), sb([128, HC], dt.int32)
        hi_f, lo_f = sb([128, HC]), sb([128, HC])
        # histogram build (full tensors; group-sliced for pipelining)
        eg = sb([128, NCHUNK, K1])
        dd = sb([128, NCHUNK, K2])
        tsef = sb([128, NCHUNK, 2 * K2])
        # contraction
        w2 = sb([2 * K2, K1])
        ws = sb([K2, K1])
        rs2 = sb([32, 2])
        t1 = sb([32, 2])
        scr = sb([K2, 2 * K1])
        accp = sb([K2, 2])
        # KD/CE
        ttu = sb([64, NT * C])
        ttuT = sb([64, NT])
        scr5 = sb([64, NT])
        kdb = sb([64, 1])
        tgf = sb([64, 1])
        oh = sb([64, C])
        ohs = sb([64, C])
        cep = sb([64, 1])
        ceb = sb([64, 1])
        kdceb = sb([64, 1])
        # L2
        accs, acct = sb([64, 1]), sb([64, 1])
        acc2 = sb([128, 1])
        scrb = sb([128, NT * C])
        scrb2 = sb([128, HC])
        # grams
        trT = sb([100, NT, 64])
        trS = sb([100, NT, 64])
        gs_sb = sb([64, NT * 64])
        gd = sb([64, NT * 64])
        gds = sb([64, NT * 64])
        accg = sb([64, 1])
        hs_sb = sb([100, NT * C])
        hd = sb([100, NT * C])
        hds = sb([100, NT * C])
        acch = sb([100, 1])
        # final
        sbs = sb([1, 16])
        fs = sb([1, 12])
        # PSUM: 8 tensors = 8 banks
        psumW = ps([2 * K2, K1])
        psmall = ps([32, 128])
        psum_gt = ps([64, NT * 64])
        psum_gs = ps([64, NT * 64])
        psum_ht = ps([100, NT * C])
        psum_hs = ps([100, NT * C])
        ptrT = ps([100, NT, 64])
        ptrS = ps([100, NT, 64])

        psum_r = psmall[:, 64:66]
        psum_t1 = psmall[:, 66:68]
        psum_p = psmall[:, 0:2 * K1]
        psum_s = psmall[0:1, 68:75]    # S1t S1s Ttot Stot S2t S2s kdce
        psum_l2 = psmall[0:1, 75:78]   # ss tt ts
        psum_sub = psmall[0:1, 78:80]  # G H

        off = K / 2 - (0.5 if sub_half else 0.0)

        with (
            nc.semaphore("d_in") as d_in,
            nc.semaphore("d_tl") as d_tl,
            nc.semaphore("d_tg") as d_tg,
            nc.semaphore("d_fold") as d_fold,
            nc.semaphore("d_ws") as d_ws,
            nc.semaphore("d_out") as d_out,
            nc.semaphore("vsem") as vsem,
            nc.semaphore("asem") as asem,
            nc.semaphore("psem") as psem,
            nc.semaphore("tsem") as tsem,
            nc.Block() as block,
        ):
            # ---------------- Pool: constants only ----------------
            @block.gpsimd
            def _(g):
                g.memset(ones[:], 1.0)
                for T in range(1, NT + 1):
                    g.memset(wrow[:, T - 1:T], -ALPHA * T * T / (B * C))
                g.iota(iota32p[:], [[1, K1]], channel_multiplier=0,
                       allow_small_or_imprecise_dtypes=True)
                g.iota(iota100p[:], [[1, C]], channel_multiplier=0,
                       allow_small_or_imprecise_dtypes=True)
                g.memset(ones32sq[:], 1.0)
                g.memset(ident64[:], 0.0)
                g.drain()
                g.affine_select(m1[:], ones32sq[:], [[-1, 32]], AL.is_ge, 0.0,
                                base=K1 - 2, channel_multiplier=-1)
                g.affine_select(m1lo[:], ones32sq[0:K2, 0:K2], [[-1, K2]], AL.is_ge,
                                0.0, base=K2 - 2, channel_multiplier=-1)
                g.affine_select(ident64[:], ident64[:], [[-1, 64]], AL.not_equal,
                                1.0, base=0, channel_multiplier=1).then_inc(psem, 1)

            # ---------------- SP: DMA choreography ----------------
            @block.sync
            def _(s):
                s.dma_start(out=sl_[:], in_=ls_d[:, :]).then_inc(d_in, 16)
                s.dma_start(out=tl_[:], in_=lt_d[:, :]).then_inc(d_tl, 16)
                s.dma_start(out=tg[:], in_=tg_d[:, :]).then_inc(d_tg, 16)
                s.wait_ge(vsem, 3)    # cubes normalized
                s.dma_start(out=t128[:], in_=_fold_ap(tcube[:])).then_inc(d_fold, 16)
                s.dma_start(out=s128[:], in_=_fold_ap(scube[:])).then_inc(d_fold, 16)
                s.wait_ge(vsem, 5)    # ci64 ready
                s.dma_start(out=ci128[:], in_=_fold_ap(ci64[:])).then_inc(d_fold, 16)
                s.wait_ge(vsem, 14)   # w2 copied to SBUF
                s.dma_start(out=ws[:], in_=w2[K2:2 * K2, :]).then_inc(d_ws, 16)
                s.wait_ge(vsem, 19)   # final scalar ready
                s.dma_start(out=out_d[:, :], in_=fs[:, 0:1]).then_inc(d_out, 16)
                s.wait_ge(d_out, 16)

            # ---------------- ACT ----------------
            @block.scalar
            def _(a):
                for (se, lse, cube, lg, dsem) in (
                    (se_s, lse_s, scube, sl_, d_in),
                    (se_t, lse_t, tcube, tl_, d_tl),
                ):
                    a.wait_ge(dsem, 16)
                    ins = None
                    for T in range(1, NT + 1):
                        i = T - 1
                        slc = slice(i * C, (i + 1) * C)
                        ins = nc.scalar.activation(out=cube[:, slc], in_=lg[:],
                                                   func=AF.Exp,
                                                   scale=1.0 / T,
                                                   accum_out=se[:, i:i + 1])
                    _ = ins
                    a.drain()
                    ins = None
                    for T in range(1, NT + 1):
                        i = T - 1
                        ins = nc.scalar.activation(out=lse[:, i:i + 1],
                                                   in_=se[:, i:i + 1], func=AF.Ln)
                    ins.then_inc(asem, 1)   # asem 1 = student, 2 = teacher
                a.wait_ge(vsem, 4)    # zd, d64 ready
                ins = None
                for T in range(1, NT + 1):
                    i = T - 1
                    ins = nc.scalar.activation(out=u64[:, i * C:(i + 1) * C],
                                               in_=d64[:], func=AF.Identity,
                                               scale=1.0 / T, bias=zd[:, i:i + 1])
                ins.then_inc(asem, 1)   # asem 3 = u64 done
                a.wait_ge(vsem, 13)   # ceb, kdb ready
                nc.scalar.activation(out=kdceb[:], in_=ceb[:], func=AF.Identity,
                                     scale=NT * (1.0 - ALPHA) / B,
                                     bias=kdb[:]).then_inc(asem, 1)  # asem 4

            # ---------------- DVE ----------------
            # NB: consecutive DVE ops with a distance-1 RAW race on hardware
            # (pipeline); dependent pairs are spaced by >= 1 independent op
            # or an explicit fsem self-sync.
            @block.vector
            def _(v):
                v.wait_ge(d_in, 16)
                v.wait_ge(d_tl, 16)
                nc.vector.tensor_sub(out=d64[:], in0=sl_[:], in1=tl_[:]).then_inc(vsem, 2)  # V1+V2
                for (se, rsum, cube, wv) in (
                    (se_s, rs_s, scube, 1),
                    (se_t, rs_t, tcube, 2),
                ):
                    v.wait_ge(asem, wv)
                    nc.vector.reciprocal(out=rsum[:], in_=se[:])
                    v.drain()
                    ins = None
                    for T in range(1, NT + 1):
                        i = T - 1
                        slc = slice(i * C, (i + 1) * C)
                        ins = nc.vector.tensor_scalar_mul(cube[:, slc], cube[:, slc],
                                                          rsum[:, i:i + 1])
                ins.then_inc(vsem, 1)   # V3: both cubes normalized
                nc.vector.tensor_sub(out=zd[:], in0=lse_t[:], in1=lse_s[:]).then_inc(vsem, 1)  # V4
                v.wait_ge(asem, 3)    # u64 done
                v.wait_ge(psem, 1)    # Pool constants (iota100p/iota32p)
                # cf chain; drains order the in-place updates, with independent
                # KD/CE/L2 ops filling the pipeline between them
                nc.vector.tensor_scalar(cf[:], u64[:], INVW, float(off), AL.mult, AL.add)
                nc.vector.tensor_mul(out=ttu[:], in0=tcube[:], in1=u64[:])
                v.wait_ge(d_tg, 16)
                nc.vector.tensor_copy(out=tgf[:], in_=tg[:])
                v.drain()
                nc.vector.tensor_scalar(cf[:], cf[:], 0.0, float(K - 1) - 0.6,
                                        AL.max, AL.min)
                nc.vector.tensor_tensor(out=scrb[0:64, :], in0=scube[:], in1=scube[:],
                                        op=AL.mult)
                nc.vector.tensor_tensor(out=hds[0:64, :], in0=tcube[:], in1=tcube[:],
                                        op=AL.mult)
                v.drain()
                nc.vector.tensor_tensor(out=oh[:],
                                        in0=_ap3(tgf[:], bcast_inner=C)[:, 0, :],
                                        in1=iota100p[:], op=AL.is_equal)
                nc.vector.tensor_reduce(out=ttuT[:],
                                        in_=ttu[:].rearrange("p (t c) -> p t c", t=NT),
                                        axis=AX.X, op=AL.add)
                v.drain()
                nc.vector.tensor_copy(out=ci64[:], in_=cf[:]).then_inc(vsem, 1)  # V5
                v.wait_ge(d_fold, 48)
                nc.vector.tensor_scalar(hi_i[:], ci128[:], 3, None, AL.arith_shift_right)
                nc.vector.tensor_scalar(lo_i[:], ci128[:], 7, None, AL.bitwise_and)
                v.drain()
                nc.vector.tensor_copy(out=lo_f[:], in_=lo_i[:])
                nc.vector.tensor_copy(out=hi_f[:], in_=hi_i[:])
                v.drain()

                def group(gi):
                    cs = slice(gi * GW, (gi + 1) * GW)
                    nc.vector.tensor_tensor(
                        out=dd[:, cs, :], in0=_ap3(lo_f[:, cs], bcast_inner=K2),
                        in1=_ap3(iota32p[:, 0:K2], bcast_mid=GW), op=AL.subtract)
                    nc.vector.tensor_tensor(
                        out=eg[:, cs, :], in0=_ap3(hi_f[:, cs], bcast_inner=K1),
                        in1=_ap3(iota32p[:, 0:K1], bcast_mid=GW), op=AL.is_equal)
                    v.drain()
                    nc.vector.scalar_tensor_tensor(
                        out=tsef[:, cs, 0:K2], in0=dd[:, cs, :], scalar=0.0,
                        in1=_ap3(t128[:, cs], bcast_inner=K2),
                        op0=AL.is_equal, op1=AL.mult)
                    nc.vector.scalar_tensor_tensor(
                        out=tsef[:, cs, K2:2 * K2], in0=dd[:, cs, :], scalar=0.0,
                        in1=_ap3(s128[:, cs], bcast_inner=K2),
                        op0=AL.is_equal, op1=AL.mult).then_inc(vsem, 1)

                group(0)              # V6: group 0 built
                v.wait_ge(tsem, 1)    # transposes done
                ins = None
                for k in range(NT):
                    nc.vector.tensor_copy(out=trT[:, k, :], in_=ptrT[:, k, :])
                    ins = nc.vector.tensor_copy(out=trS[:, k, :], in_=ptrS[:, k, :])
                ins.then_inc(vsem, 1)  # V7: tr copies done
                for gi in range(1, NG):
                    group(gi)         # V8..V16
                # KD / CE / L2 tail (ttu/ttuT, tgf, oh, squares computed above)
                nc.vector.tensor_tensor(out=ohs[:], in0=oh[:], in1=sl_[:], op=AL.mult)
                nc.vector.tensor_tensor(out=scr5[:], in0=ttuT[:], in1=wrow[:], op=AL.mult)
                nc.vector.tensor_tensor(out=scrb2[:], in0=t128[:], in1=s128[:],
                                        op=AL.mult)
                v.drain()
                nc.vector.tensor_reduce(out=cep[:], in_=ohs[:], axis=AX.X, op=AL.add)
                nc.vector.tensor_reduce(out=kdb[:], in_=scr5[:], axis=AX.X, op=AL.add)
                nc.vector.tensor_reduce(out=accs[:], in_=scrb[0:64, :], axis=AX.X,
                                        op=AL.add)
                nc.vector.tensor_reduce(out=acct[:], in_=hds[0:64, :], axis=AX.X,
                                        op=AL.add)
                nc.vector.tensor_reduce(out=acc2[:], in_=scrb2[:], axis=AX.X,
                                        op=AL.add)
                v.drain()
                nc.vector.tensor_sub(out=ceb[:], in0=lse_s[:, 0:1],
                                     in1=cep[:]).then_inc(vsem, 2)  # V17+V18
                v.wait_ge(tsem, 2)    # histogram matmuls done
                nc.vector.tensor_copy(out=w2[:], in_=psumW[:]).then_inc(vsem, 1)  # V19
                v.wait_ge(tsem, 3)    # gram matmuls done
                nc.vector.tensor_copy(out=gs_sb[:], in_=psum_gs[:])
                nc.vector.tensor_copy(out=hs_sb[:], in_=psum_hs[:])
                v.drain()
                nc.vector.tensor_sub(out=gd[:], in0=psum_gt[:], in1=gs_sb[:])
                nc.vector.tensor_sub(out=hd[:], in0=psum_ht[:], in1=hs_sb[:])
                v.drain()
                nc.vector.tensor_tensor(out=gds[:], in0=gd[:], in1=gd[:], op=AL.mult)
                nc.vector.tensor_tensor(out=hds[:], in0=hd[:], in1=hd[:], op=AL.mult)
                v.drain()
                nc.vector.tensor_reduce(out=accg[:], in_=gds[:], axis=AX.X, op=AL.add)
                nc.vector.tensor_reduce(out=acch[:], in_=hds[:], axis=AX.X,
                                        op=AL.add).then_inc(vsem, 1)  # V20
                v.wait_ge(tsem, 4)    # r matmuls done
                nc.vector.tensor_copy(out=rs2[:], in_=psum_r[:, :]).then_inc(vsem, 1)  # V21
                v.wait_ge(tsem, 5)    # P matmuls done
                nc.vector.tensor_tensor(out=scr[:, 0:K1], in0=w2[0:K2, :],
                                        in1=psum_p[0:K2, 0:K1], op=AL.mult)
                nc.vector.tensor_tensor(out=scr[:, K1:2 * K1], in0=ws[:],
                                        in1=psum_p[0:K2, K1:2 * K1], op=AL.mult)
                v.drain()
                nc.vector.tensor_reduce(out=accp[:, 0:1], in_=scr[:, 0:K1],
                                        axis=AX.X, op=AL.add)
                nc.vector.tensor_reduce(out=accp[:, 1:2], in_=scr[:, K1:2 * K1],
                                        axis=AX.X, op=AL.add).then_inc(vsem, 1)  # V22
                v.wait_ge(tsem, 6)    # t1 matmul done
                nc.vector.tensor_copy(out=t1[:], in_=psum_t1[:, :]).then_inc(vsem, 1)  # V23
                v.wait_ge(tsem, 7)    # all scalar matmuls done
                nc.vector.tensor_copy(out=sbs[:, 0:12], in_=psmall[0:1, 68:80])
                S1t, S1s, Ttot, Stot, S2t, S2s, kdce = (sbs[:, i:i + 1] for i in range(7))
                ss_, tt_, ts_ = sbs[:, 7:8], sbs[:, 8:9], sbs[:, 9:10]
                subg, subh = sbs[:, 10:11], sbs[:, 11:12]
                v.drain()
                # level 1
                nc.vector.tensor_add(out=fs[:, 0:1], in0=S1t, in1=S2t)
                nc.vector.tensor_add(out=fs[:, 1:2], in0=S1s, in1=S2s)
                nc.vector.tensor_mul(out=fs[:, 3:4], in0=Ttot, in1=Ttot)
                nc.vector.tensor_mul(out=fs[:, 4:5], in0=Stot, in1=Stot)
                nc.vector.tensor_mul(out=fs[:, 7:8], in0=tt_, in1=tt_)
                nc.vector.tensor_mul(out=fs[:, 8:9], in0=ss_, in1=ss_)
                nc.vector.tensor_mul(out=sbs[:, 13:14], in0=ts_, in1=ts_)
                nc.vector.tensor_add(out=fs[:, 11:12], in0=subg, in1=subh)
                v.drain()
                # level 2
                nc.vector.tensor_sub(out=fs[:, 2:3], in0=fs[:, 0:1], in1=fs[:, 1:2])
                nc.vector.tensor_sub(out=fs[:, 5:6], in0=fs[:, 4:5], in1=fs[:, 3:4])
                nc.vector.tensor_add(out=fs[:, 7:8], in0=fs[:, 7:8], in1=fs[:, 8:9])
                nc.vector.tensor_add(out=fs[:, 11:12], in0=fs[:, 11:12], in1=kdce)
                v.drain()
                # level 3
                nc.vector.scalar_tensor_tensor(out=fs[:, 6:7], in0=fs[:, 2:3],
                                               scalar=2.0, in1=fs[:, 5:6],
                                               op0=AL.mult, op1=AL.add)  # S_l1
                nc.vector.scalar_tensor_tensor(out=fs[:, 9:10], in0=sbs[:, 13:14],
                                               scalar=-2.0, in1=fs[:, 7:8],
                                               op0=AL.mult, op1=AL.add)  # l2raw
                v.drain()
                # level 4
                nc.vector.tensor_add(out=fs[:, 10:11], in0=fs[:, 6:7], in1=fs[:, 9:10])
                v.drain()
                nc.vector.scalar_tensor_tensor(out=fs[:, 0:1], in0=fs[:, 10:11],
                                               scalar=0.00025, in1=fs[:, 11:12],
                                               op0=AL.mult, op1=AL.add).then_inc(vsem, 1)  # V24

            # ---------------- PE ----------------
            @block.tensor
            def _(t):
                t.wait_ge(psem, 1)    # ident64 / m1
                t.wait_ge(vsem, 3)    # cubes
                ins = None
                for k in range(NT):
                    nc.tensor.transpose(out=ptrT[:, k, :],
                                        in_=tcube[:, k * C:(k + 1) * C],
                                        identity=ident64[:])
                    ins = nc.tensor.transpose(out=ptrS[:, k, :],
                                              in_=scube[:, k * C:(k + 1) * C],
                                              identity=ident64[:])
                ins.then_inc(tsem, 1)   # T1
                ins = None
                for gi in range(NG):
                    t.wait_ge(vsem, 6 if gi == 0 else 7 + gi)
                    for i in range(GW):
                        ch = gi * GW + i
                        ins = nc.tensor.matmul(psumW[:], lhsT=tsef[:, ch, :],
                                               rhs=eg[:, ch, :],
                                               start=(ch == 0),
                                               stop=(ch == NCHUNK - 1))
                ins.then_inc(tsem, 1)   # T2: histogram done
                t.wait_ge(vsem, 7)    # trT/trS in SBUF
                ins = None
                for k in range(NT):
                    nc.tensor.matmul(psum_gt[:, k * 64:(k + 1) * 64],
                                     lhsT=trT[:, k, :], rhs=trT[:, k, :],
                                     start=True, stop=True,
                                     skip_group_check=(k > 0))
                    nc.tensor.matmul(psum_gs[:, k * 64:(k + 1) * 64],
                                     lhsT=trS[:, k, :], rhs=trS[:, k, :],
                                     start=True, stop=True,
                                     skip_group_check=(k > 0))
                    nc.tensor.matmul(psum_ht[:, k * C:(k + 1) * C],
                                     lhsT=tcube[:, k * C:(k + 1) * C],
                                     rhs=tcube[:, k * C:(k + 1) * C],
                                     start=True, stop=True,
                                     skip_group_check=(k > 0))
                    ins = nc.tensor.matmul(psum_hs[:, k * C:(k + 1) * C],
                                           lhsT=scube[:, k * C:(k + 1) * C],
                                           rhs=scube[:, k * C:(k + 1) * C],
                                           start=True, stop=True,
                                           skip_group_check=(k > 0))
                ins.then_inc(tsem, 1)   # T3: gram matmuls done
                # scalar matmuls into psmall (bank ping-pong with DVE reads)
                t.wait_ge(vsem, 13)   # accs/acct/acc2
                nc.tensor.matmul(psum_l2[:, 0:1], lhsT=accs[:], rhs=ones[0:64, :],
                                 start=True, stop=True, skip_group_check=True)
                nc.tensor.matmul(psum_l2[:, 1:2], lhsT=acct[:], rhs=ones[0:64, :],
                                 start=True, stop=True, skip_group_check=True)
                nc.tensor.matmul(psum_l2[:, 2:3], lhsT=acc2[:], rhs=ones[:],
                                 start=True, stop=True, skip_group_check=True)
                t.wait_ge(vsem, 15)   # accg/acch
                nc.tensor.matmul(psum_sub[:, 0:1], lhsT=accg[:], rhs=ones[0:64, :],
                                 start=True, stop=True, skip_group_check=True)
                nc.tensor.matmul(psum_sub[:, 1:2], lhsT=acch[:], rhs=ones[0:100, :],
                                 start=True, stop=True, skip_group_check=True)
                t.wait_ge(asem, 4)    # kdceb
                nc.tensor.matmul(psum_s[:, 6:7], lhsT=kdceb[:], rhs=ones[0:64, :],
                                 start=True, stop=True, skip_group_check=True)
                t.wait_ge(vsem, 14)   # w2
                t.wait_ge(d_ws, 16)   # ws
                nc.tensor.matmul(psum_r[:, 0:1], lhsT=w2[0:K2, :], rhs=ones[0:K2, :],
                                 start=True, stop=True, skip_group_check=True)
                nc.tensor.matmul(psum_r[:, 1:2], lhsT=ws[:], rhs=ones[0:K2, :],
                                 start=True, stop=True,
                                 skip_group_check=True).then_inc(tsem, 1)  # T4
                t.wait_ge(vsem, 16)   # rs2 copied (frees psmall bank)
                nc.tensor.matmul(psum_p[0:K2, 0:K1], lhsT=m1lo[:],
                                 rhs=_rev_free(w2[0:K2, :], K1),
                                 start=True, stop=True, skip_group_check=True)
                nc.tensor.matmul(psum_p[0:K2, K1:2 * K1], lhsT=m1lo[:],
                                 rhs=_rev_free(ws[:], K1),
                                 start=True, stop=True,
                                 skip_group_check=True).then_inc(tsem, 1)  # T5
                t.wait_ge(vsem, 17)   # accp done (DVE finished reading psum_p)
                nc.tensor.matmul(psum_t1[:, :], lhsT=m1[:], rhs=rs2[:],
                                 start=True, stop=True,
                                 skip_group_check=True).then_inc(tsem, 1)  # T6
                t.wait_ge(vsem, 18)   # t1 copied
                nc.tensor.matmul(psum_s[:, 0:1], lhsT=t1[:, 0:1], rhs=rs2[:, 0:1],
                                 start=True, stop=True, skip_group_check=True)
                nc.tensor.matmul(psum_s[:, 1:2], lhsT=t1[:, 1:2], rhs=rs2[:, 1:2],
                                 start=True, stop=True, skip_group_check=True)
                nc.tensor.matmul(psum_s[:, 2:3], lhsT=rs2[:, 0:1], rhs=ones[0:32, :],
                                 start=True, stop=True, skip_group_check=True)
                nc.tensor.matmul(psum_s[:, 3:4], lhsT=rs2[:, 1:2], rhs=ones[0:32, :],
                                 start=True, stop=True, skip_group_check=True)
                nc.tensor.matmul(psum_s[:, 4:5], lhsT=accp[:, 0:1], rhs=ones[0:K2, :],
                                 start=True, stop=True, skip_group_check=True)
                nc.tensor.matmul(psum_s[:, 5:6], lhsT=accp[:, 1:2], rhs=ones[0:K2, :],
                                 start=True, stop=True,
                                 skip_group_check=True).then_inc(tsem, 1)  # T7

    return nc


_cache = {}


def _get_nc():
    if "nc" not in _cache:
        _cache["nc"] = build()
    return _cache["nc"]


def kernel(logits_student, logits_teacher, target):
    from concourse.bass_utils import run_bass_kernel_spmd

    nc = _get_nc()
    in_map = {
        "logits_student": np.ascontiguousarray(logits_student, dtype=np.float32),
        "logits_teacher": np.ascontiguousarray(logits_teacher, dtype=np.float32),
        "target": np.ascontiguousarray(np.asarray(target).reshape(B, 1).astype(np.int32)),
    }
    core_ids = list(range(8))
    res = run_bass_kernel_spmd(nc, [in_map] * 8, core_ids)
    out = res.results[0]["out"]
    return np.float32(out.reshape(())).reshape(())



# revision 33
# speedup vs baseline: 2.4591x; 2.4591x over previous
# Trainium2 Bass kernel for nn_CKDLoss: KD loss + virtual-outer-product L1/L2
# + Gram-matrix sub-losses, computed entirely on device.
#
# Sharding: total work after algorithmic reduction is tiny and latency-bound;
# cross-core collectives cost more than the whole computation. Every core runs
# the identical full computation on replicated inputs; host takes core 0.
#
# L1 math: with u = log s - log t (normalized softmax cubes flattened to N),
#   sum_{a,b} |t_a t_b - s_a s_b| = 2*(S_tt - S_ss)   (T = S = 320 cancel),
#   S_tt = sum_{pairs: u_a+u_b<0} t_a t_b.
# Key identity: u = (sl - tl)/T + (lse_t - lse_s) per element — no exp/log of
# cube values on the u path; five tensor_scalar ops on the raw logits plus
# the row-lse bias build u64 exactly.
# Bucketize c = floor((u+UMAX)*K/(2 UMAX)) in [0,K), c = 8*hi + lo.
# The positive-pair test c_a+c_b <= K-2 splits exactly into
#   (hi_a+hi_b <= K1-2)  OR  (hi_a+hi_b = K1-1 AND lo_a+lo_b <= K2-2).
# Build W[hi, lo] = sum_n t_n 1[hi_n=hi] 1[lo_n=lo] (PSUM-accumulated one-hot
# matmuls, bf16). Reversed rows Wrev = P_antidiag @ W via one permutation
# matmul. Then with r[hi] = sum_lo W[hi, lo]:
#   S1 = r^T M1 r                 (M1[a,b] = 1[a+b<=K1-2])
#   U  = W^T Wrev  (8x8),  S2 = sum m1lo * U   (m1lo[a,b] = 1[a+b<=K2-2])
#   S_tt = S1 + S2.
# bf16 in the heavy path is safe: measured end-to-end shift vs f32 is ~1e-5
# of the loss (the bucketization itself is 2.3e-3).
#
# Layout: [64, 512] flat tensors (500 data + 12 zero pad) fold to [128, 256]
# via two permutation matmuls (split at flat col 256). f32r at 256 output
# columns runs 1 cycle/row, so the u-fold costs ~0.5us. Pad elements carry
# zero weight everywhere, so they never contribute. One-hots are built in
# [128, K, chunk] layout (bucket index as MIDDLE dim) so all build operands
# have packed 2-byte last dims -> DVE 2x mode. Pool (gpsimd) cannot run
# is_equal/shift through the walrus codegen, so DVE builds the one-hots and
# Pool applies the t/s weights. A PSUM bank is read by at most one engine
# per phase (HW forbids concurrent multi-engine reads of a bank).

import numpy as np
from contextlib import ExitStack

B, C, NT = 64, 100, 5            # batch, classes, temps 1..5
N = B * C * NT                   # 32000 flattened cube elements
K1, K2 = 32, 8                   # two-level bucket split, K = 256
K = K1 * K2
UMAX = 16.0
INVW = K / (2.0 * UMAX)          # 8.0
ALPHA = 0.7
FLAT = 512                       # padded flat width (500 data + 12 pad)
HC = FLAT // 2                   # 256 folded columns
NCHUNK = HC                      # 256 PE chunks of 128 elements
NG = 5
GS = [64, 64, 64, 44, 20]        # descending group sizes (small tail group)
GO = [0, 64, 128, 192, 236]


def _mkap(tensor_ap, dims, extra_off=0):
    import concourse.bass as bass
    return bass.AP(tensor=tensor_ap.tensor, offset=tensor_ap.offset + extra_off,
                   ap=[list(d) for d in dims])


def _ap3(ap, bcast_inner=None, bcast_mid=None):
    """Append/insert stride-0 dims on an AP: [P,F] -> [P,F,bi] or [P,bm,F]."""
    dims = [list(d) for d in ap.ap]
    if bcast_inner is not None:
        dims = dims + [[0, bcast_inner]]
    if bcast_mid is not None:
        dims = [dims[0], [0, bcast_mid]] + dims[1:]
    return _mkap(ap, dims)


def _gslice(t_ap, ncols, gi, colhalf=0):
    """[128, ncols, NCHUNK] tensor -> [128, ncols, GS[gi]] AP for group gi."""
    dims = [list(t_ap.ap[0]), [NCHUNK, ncols], [1, GS[gi]]]
    return _mkap(t_ap, dims, extra_off=colhalf * K2 * NCHUNK + GO[gi])


def _chunkap(t_ap, ncols, ch):
    """[128, ncols, NCHUNK] tensor -> [128, ncols] AP for chunk ch."""
    dims = [list(t_ap.ap[0]), [NCHUNK, ncols]]
    return _mkap(t_ap, dims, extra_off=ch)


def build():
    import concourse.bass as bass
    from concourse import mybir

    dt = mybir.dt
    AL = mybir.AluOpType
    AF = mybir.ActivationFunctionType
    AX = mybir.AxisListType
    bf = dt.bfloat16
    fr = dt.float32r

    nc = bass.Bass()
    ls_d = nc.declare_dram_parameter("logits_student", [B, C], dt.float32, isOutput=False)
    lt_d = nc.declare_dram_parameter("logits_teacher", [B, C], dt.float32, isOutput=False)
    tg_d = nc.declare_dram_parameter("target", [B, 1], dt.int32, isOutput=False)
    out_d = nc.declare_dram_parameter("out", [1, 1], dt.float32, isOutput=True)

    ctx = ExitStack()
    _n = [0]

    def sb(shape, d=dt.float32):
        _n[0] += 1
        return ctx.enter_context(nc.sbuf_tensor(f"sb{_n[0]}", shape, d))

    def ps(shape, d=dt.float32):
        _n[0] += 1
        return ctx.enter_context(nc.psum_tensor(f"ps{_n[0]}", shape, d))

    with ctx:
        # ---- constants ----
        ones = sb([128, 1])
        ones_b = sb([128, 1], bf)
        iota100 = sb([64, C])
        onesq = sb([32, 32])
        m1 = sb([32, 32])                 # 1[a+b<=K1-2]
        m1lo = sb([K2, K2])               # 1[a+b<=K2-2]
        prev32 = sb([32, 32])             # antidiagonal permutation 1[a+b=31]
        id64b = sb([64, 64], bf)
        e1b = sb([64, 128], bf)           # fold identity half 1: 1[f==p]
        e2b = sb([64, 128], bf)           # fold identity half 2: 1[f==p+64]
        wrow64 = sb([64, NT])             # KD weights -a*T^2/(B*C)
        irep32 = sb([128, K1, 64], bf)    # irep32[p, j, m] = j
        irep8 = sb([128, K2, 64], bf)
        # ---- inputs ----
        sl_ = sb([64, C])
        tl_ = sb([64, C])
        tg = sb([64, 1], dt.int32)
        # ---- softmax / u stage ----
        sls = sb([64, NT * C])            # student logits / T
        tls = sb([64, NT * C])            # teacher logits / T
        d0 = sb([64, C])                  # sl - tl
        u64 = sb([64, FLAT], bf)          # (sl-tl)/T + zd, padded
        es = sb([64, NT * C], bf)         # exp(student)
        et = sb([64, NT * C], bf)
        se_s = sb([64, NT], bf)
        se_t = sb([64, NT], bf)
        rs_s = sb([64, NT])
        rs_t = sb([64, NT])
        rs_sn = sb([64, NT])
        lse_s = sb([64, NT])
        lse_t = sb([64, NT])
        zd = sb([64, NT])
        cube_tb = sb([64, FLAT], bf)      # normalized teacher cube (padded)
        cube_sb = sb([64, FLAT], bf)
        cube_snb = sb([64, FLAT], bf)
        # ---- folded [128, 256] ----
        t128b = sb([128, HC], bf)
        s128b = sb([128, HC], bf)
        cf = sb([128, HC])
        ci = sb([128, HC], dt.int32)
        hi_i = sb([128, HC], dt.int32)
        lo_i = sb([128, HC], dt.int32)
        hi_b = sb([128, HC], bf)
        lo_b = sb([128, HC], bf)
        # ---- histogram build ----
        eg2 = sb([128, K1, NCHUNK], bf)
        lm2 = sb([128, K2, NCHUNK], bf)
        tsef2 = sb([128, 2 * K2, NCHUNK], bf)
        # ---- grams ----
        trT = sb([100, NT, 64], bf)
        trS = sb([100, NT, 64], bf)
        trSn = sb([100, NT, 64], bf)
        sqg = sb([64, NT * 64], bf)
        sqh = sb([100, NT * C], bf)
        accg = sb([64, 1])
        acch = sb([100, 1])
        # ---- L2 / KD / CE ----
        sq_t = sb([128, HC], bf)
        sq_s = sb([128, HC], bf)
        tt128 = sb([128, 1])
        ss128 = sb([128, 1])
        tsprod = sb([128, HC], bf)
        ts128 = sb([128, 1])
        d0b = sb([64, C], bf)
        cw = sb([64, NT * C], bf)         # cube_tb * (w_T/T)
        cwp = sb([64, NT * C], bf)        # cw * d0
        kscr = sb([64, NT * C], bf)       # ACT accum scratch
        cscr = sb([64, C], bf)            # CE accum scratch
        zscr = sb([64, NT], bf)
        tscr = sb([128, HC], bf)
        k1act = sb([64, 1])
        kz = sb([64, NT])
        kdzd = sb([64, 1])
        kdv = sb([64, 1])
        tgf = sb([64, 1])
        oh = sb([64, C])
        ohs = sb([64, C])
        cep = sb([64, 1])
        ceb = sb([64, 1])
        # ---- contraction ----
        w2sb = sb([32, 2 * K2])
        wrevsb = sb([32, 2 * K2])
        r2 = sb([32, 2])
        t1sb = sb([32, 2])
        p8 = sb([K2, 2 * K2])
        s2v = sb([K2, 2])
        # ---- final ----
        sbs = sb([1, 16])
        fs = sb([1, 12])
        warm = sb([1, 1])
        # ---- PSUM: 8 tensors = 8 banks ----
        psumF = ps([128, HC])             # u-fold, then teacher fold
        psumFs = ps([128, HC])            # student fold
        ptrT = ps([100, NT, 64], bf)
        ptrS = ps([100, NT, 64], bf)
        psum_gd = ps([64, NT * 64])
        psum_hd = ps([100, NT * C])
        psumW = ps([K1, 2 * K2])
        psmall = ps([32, 64])

        SC_TT, SC_SS, SC_TS, SC_KD, SC_CE, SC_SG, SC_SH = 0, 1, 2, 3, 4, 5, 6
        SC_S1T, SC_S1S, SC_S2T, SC_S2S = 7, 8, 9, 10
        SC_UT, SC_US = 16, 24
        SC_T1 = 32
        SC_WR = 40

        wv = [-ALPHA * T * T / (B * C) for T in range(1, NT + 1)]

        with (
            nc.semaphore("d_in") as d_in,
            nc.semaphore("d_tl") as d_tl,
            nc.semaphore("d_tg") as d_tg,
            nc.semaphore("d_out") as d_out,
            nc.semaphore("vsem") as vsem,
            nc.semaphore("psem") as psem,
            nc.semaphore("asem") as asem,
            nc.semaphore("tsem") as tsem,
            nc.Block() as block,
        ):
            # ---------------- SP: DMA only ----------------
            @block.sync
            def _(s):
                s.dma_start(out=sl_[:], in_=ls_d[:, :]).then_inc(d_in, 16)
                s.dma_start(out=tl_[:], in_=lt_d[:, :]).then_inc(d_tl, 16)
                s.dma_start(out=tg[:], in_=tg_d[:, :]).then_inc(d_tg, 16)
                s.wait_ge(vsem, 24)       # final scalar ready
                s.dma_start(out=out_d[:, :], in_=fs[:, 0:1]).then_inc(d_out, 16)
                s.wait_ge(d_out, 16)

            # ---------------- Pool ----------------
            # psem: 1=ones 2=quick constants 3=tls 4=ireps 5=cube_snb
            #       6=cube_tb 7=tsprod 8=hi_b 9..13=tsef2 groups
            @block.gpsimd
            def _(g):
                g.memset(ones[:], 1.0).then_inc(psem, 1)   # 1: ACT warmup gate
                g.memset(ones_b[:], 1.0)
                g.memset(onesq[:], 1.0)
                g.memset(id64b[:], 0.0)
                g.memset(prev32[:], 0.0)
                g.memset(e1b[:], 0.0)
                g.memset(e2b[:], 0.0)
                g.iota(iota100[:], [[1, C]], channel_multiplier=0,
                       allow_small_or_imprecise_dtypes=True)
                for T in range(1, NT + 1):
                    g.memset(wrow64[:, T - 1:T], wv[T - 1])
                # zero pads (data cols are written later; ranges are disjoint)
                g.memset(u64[:, NT * C:FLAT], 0.0)
                g.memset(cube_tb[:, NT * C:FLAT], 0.0)
                g.memset(cube_sb[:, NT * C:FLAT], 0.0)
                g.memset(cube_snb[:, NT * C:FLAT], 0.0)
                g.drain()
                g.affine_select(m1[:], onesq[:], [[-1, 32]], AL.is_ge, 0.0,
                                base=K1 - 2, channel_multiplier=-1)
                g.affine_select(m1lo[:], onesq[0:K2, 0:K2], [[-1, K2]], AL.is_ge,
                                0.0, base=K2 - 2, channel_multiplier=-1)
                g.affine_select(id64b[:], id64b[:], [[-1, 64]], AL.not_equal,
                                1.0, base=0, channel_multiplier=1)
                g.affine_select(prev32[:], prev32[:], [[-1, 32]], AL.not_equal,
                                1.0, base=K1 - 1, channel_multiplier=-1)
                g.affine_select(e1b[:], e1b[:], [[-1, 128]], AL.not_equal,
                                1.0, base=0, channel_multiplier=1)
                g.affine_select(e2b[:], e2b[:], [[-1, 128]], AL.not_equal,
                                1.0, base=64, channel_multiplier=1).then_inc(psem, 1)   # 2
                # teacher prescale (ahead of the slow irep iotas)
                g.wait_ge(d_tl, 16)
                ins = None
                for T in range(1, NT + 1):
                    i = T - 1
                    ins = nc.gpsimd.tensor_scalar_mul(
                        tls[:, i * C:(i + 1) * C], tl_[:], 1.0 / T)
                ins.then_inc(psem, 1)     # 3: tls
                g.iota(irep32[:], [[1, K1], [0, 64]], channel_multiplier=0,
                       allow_small_or_imprecise_dtypes=True)
                g.iota(irep8[:], [[1, K2], [0, 64]], channel_multiplier=0,
                       allow_small_or_imprecise_dtypes=True).then_inc(psem, 1)  # 4
                g.wait_ge(vsem, 3)        # rs_sn
                ins = None
                for T in range(1, NT + 1):
                    i = T - 1
                    slc = slice(i * C, (i + 1) * C)
                    ins = nc.gpsimd.tensor_scalar_mul(cube_snb[:, slc], es[:, slc],
                                                      rs_sn[:, i:i + 1])
                ins.then_inc(psem, 1)     # 5: cube_snb
                g.wait_ge(vsem, 5)        # rs_t
                ins = None
                for T in range(1, NT + 1):
                    i = T - 1
                    slc = slice(i * C, (i + 1) * C)
                    ins = nc.gpsimd.tensor_scalar_mul(cube_tb[:, slc], et[:, slc],
                                                      rs_t[:, i:i + 1])
                ins.then_inc(psem, 1)     # 6: cube_tb
                g.wait_ge(vsem, 9)        # hi_i (DVE shift)
                nc.gpsimd.tensor_copy(out=hi_b[:], in_=hi_i[:]).then_inc(psem, 1)  # 7
                g.wait_ge(asem, 6)        # t128b + s128b (ACT copies)
                nc.gpsimd.tensor_tensor(out=tsprod[:], in0=t128b[:], in1=s128b[:],
                                        op=AL.mult).then_inc(psem, 1)  # 8: tsprod
                # KD product pieces (consumed by one ACT accum op)
                nc.gpsimd.tensor_copy(out=d0b[:], in_=d0[:])
                ins = None
                for T in range(1, NT + 1):
                    i = T - 1
                    ins = nc.gpsimd.tensor_scalar_mul(
                        cw[:, i * C:(i + 1) * C], cube_tb[:, i * C:(i + 1) * C],
                        wv[i] / T)
                g.drain()
                nc.gpsimd.tensor_tensor(out=cwp[:], in0=cw[:],
                                        in1=_ap3(d0b[:], bcast_mid=NT),
                                        op=AL.mult).then_inc(psem, 1)  # 9: cwp
                for gi in range(NG):
                    cs = slice(GO[gi], GO[gi] + GS[gi])
                    g.wait_ge(vsem, 10 + 2 * gi)   # lm2 group built
                    nc.gpsimd.tensor_tensor(
                        out=_gslice(tsef2[:], K2, gi),
                        in0=_gslice(lm2[:], K2, gi),
                        in1=_ap3(t128b[:, cs], bcast_mid=K2), op=AL.mult)
                    g.drain()
                    nc.gpsimd.tensor_tensor(
                        out=_gslice(tsef2[:], K2, gi, colhalf=1),
                        in0=_gslice(lm2[:], K2, gi),
                        in1=_ap3(s128b[:, cs], bcast_mid=K2),
                        op=AL.mult).then_inc(psem, 1)   # 10..14


            # ---------------- ACT ----------------
            # asem: 1=exp_s 2=exp_t 3=lse_s 4=lse_t 5=s128b 6=t128b
            #       7=L2 squares 8=tr copies 9=gram squares
            @block.scalar
            def _(a):
                a.wait_ge(psem, 1)
                nc.scalar.activation(out=warm[:], in_=ones[0:1, :], func=AF.Exp)
                a.wait_ge(vsem, 1)        # sls
                nc.scalar.activation(out=es[:], in_=sls[:], func=AF.Exp).then_inc(asem, 1)
                a.wait_ge(psem, 3)        # tls
                nc.scalar.activation(out=et[:], in_=tls[:], func=AF.Exp).then_inc(asem, 1)
                a.wait_ge(vsem, 2)        # se_s
                nc.scalar.activation(out=lse_s[:], in_=se_s[:],
                                     func=AF.Ln).then_inc(asem, 1)
                a.wait_ge(vsem, 4)        # se_t (before recip)
                nc.scalar.activation(out=lse_t[:], in_=se_t[:],
                                     func=AF.Ln).then_inc(asem, 1)
                a.wait_ge(tsem, 2)        # fold S in PSUM
                nc.scalar.activation(out=s128b[:], in_=psumFs[:],
                                     func=AF.Identity).then_inc(asem, 1)  # 5
                a.wait_ge(tsem, 3)        # fold T in PSUM
                nc.scalar.activation(out=t128b[:], in_=psumF[:],
                                     func=AF.Identity).then_inc(asem, 1)  # 6
                nc.scalar.activation(out=sq_t[:], in_=psumF[:], func=AF.Square,
                                     accum_out=tt128[:])
                nc.scalar.activation(out=sq_s[:], in_=psumFs[:], func=AF.Square,
                                     accum_out=ss128[:]).then_inc(asem, 1)  # 7
                a.wait_ge(tsem, 4)        # transposes done
                nc.scalar.activation(out=trT[:], in_=ptrT[:], func=AF.Identity)
                nc.scalar.activation(out=trS[:], in_=ptrS[:], func=AF.Identity)
                nc.scalar.activation(out=trSn[:], in_=ptrS[:], func=AF.Identity,
                                     scale=-1.0).then_inc(asem, 1)   # 8
                a.wait_ge(psem, 9)        # cwp
                nc.scalar.activation(out=kscr[:], in_=cwp[:], func=AF.Identity,
                                     accum_out=k1act[:]).then_inc(asem, 1)  # 9
                a.wait_ge(vsem, 8)        # ohs + kz written (cf implies both)
                a.wait_ge(psem, 8)        # tsprod
                nc.scalar.activation(out=cscr[:], in_=ohs[:], func=AF.Identity,
                                     accum_out=cep[:])
                nc.scalar.activation(out=zscr[:], in_=kz[:], func=AF.Identity,
                                     accum_out=kdzd[:])
                nc.scalar.activation(out=tscr[:], in_=tsprod[:], func=AF.Identity,
                                     accum_out=ts128[:]).then_inc(asem, 1)  # 10
                a.wait_ge(tsem, 5)        # gram mms done
                nc.scalar.activation(out=sqg[:], in_=psum_gd[:], func=AF.Square,
                                     accum_out=accg[:])
                nc.scalar.activation(out=sqh[:], in_=psum_hd[:], func=AF.Square,
                                     accum_out=acch[:]).then_inc(asem, 1)  # 11

            # ---------------- DVE ----------------
            # vsem: 1=sls 2=se_s 3=rs_sn 4=se_t+recip_t 5=u64 6=cube_sb 7=ceb
            #       8=cf 9=hi_i  10,12,14,16,18=lm2  11,13,15,17,19=eg2
            #       20=kdv+ts128 21=w2sb+r2 22=wrevsb+t1sb 23=s2v 24=final
            @block.vector
            def _(v):
                v.wait_ge(d_in, 16)
                ins = None
                for T in range(1, NT + 1):
                    i = T - 1
                    ins = nc.vector.tensor_scalar_mul(
                        sls[:, i * C:(i + 1) * C], sl_[:], 1.0 / T)
                ins.then_inc(vsem, 1)
                v.wait_ge(d_tl, 16)
                nc.vector.tensor_sub(out=d0[:], in0=sl_[:], in1=tl_[:])
                v.wait_ge(asem, 1)        # exp_s
                v.drain()
                with nc.allow_low_precision(reason="se sums tolerate bf16"):
                    nc.vector.tensor_reduce(out=se_s[:], in_=es[:].rearrange(
                        "p (t c) -> p t c", t=NT), axis=AX.X, op=AL.add).then_inc(vsem, 1)
                v.drain()
                nc.vector.reciprocal(out=rs_s[:], in_=se_s[:])
                v.drain()
                nc.vector.tensor_scalar_mul(rs_sn[:], rs_s[:], -1.0).then_inc(vsem, 1)  # 3
                v.wait_ge(asem, 2)        # exp_t
                with nc.allow_low_precision(reason="se sums tolerate bf16"):
                    nc.vector.tensor_reduce(out=se_t[:], in_=et[:].rearrange(
                        "p (t c) -> p t c", t=NT), axis=AX.X, op=AL.add).then_inc(vsem, 1)  # 4
                v.drain()
                nc.vector.reciprocal(out=rs_t[:], in_=se_t[:]).then_inc(vsem, 1)  # 5
                v.wait_ge(asem, 4)        # lse_t (and lse_s)
                nc.vector.tensor_sub(out=zd[:], in0=lse_t[:], in1=lse_s[:])
                v.drain()
                ins = None
                for T in range(1, NT + 1):
                    i = T - 1
                    ins = nc.vector.tensor_scalar(
                        u64[:, i * C:(i + 1) * C], d0[:], 1.0 / T, zd[:, i:i + 1],
                        AL.mult, AL.add)
                ins.then_inc(vsem, 1)     # 6: u64
                nc.vector.tensor_tensor(out=kz[:], in0=zd[:], in1=wrow64[:],
                                        op=AL.mult)
                ins = None
                for T in range(1, NT + 1):
                    i = T - 1
                    slc = slice(i * C, (i + 1) * C)
                    ins = nc.vector.tensor_scalar_mul(cube_sb[:, slc], es[:, slc],
                                                      rs_s[:, i:i + 1])
                ins.then_inc(vsem, 1)     # 7: cube_sb
                # CE one-hot products fill the wait for the u-fold
                v.wait_ge(d_tg, 16)
                nc.vector.tensor_copy(out=tgf[:], in_=tg[:])
                v.drain()
                nc.vector.tensor_tensor(out=oh[:],
                                        in0=_ap3(tgf[:], bcast_inner=C)[:, 0, :],
                                        in1=iota100[:], op=AL.is_equal)
                v.drain()
                nc.vector.tensor_tensor(out=ohs[:], in0=oh[:], in1=sl_[:], op=AL.mult)
                # bucket chain straight from the u-fold PSUM (sole reader here)
                v.wait_ge(tsem, 1)        # u-fold done
                nc.vector.tensor_scalar(cf[:], psumF[:], INVW, K / 2.0 - 0.5,
                                        AL.mult, AL.add).then_inc(vsem, 1)  # 8
                v.drain()
                nc.vector.tensor_copy(out=ci[:], in_=cf[:])
                v.drain()
                nc.vector.tensor_scalar(hi_i[:], ci[:], 3, None,
                                        AL.arith_shift_right).then_inc(vsem, 1)  # 9
                nc.vector.tensor_scalar(lo_i[:], ci[:], 7, None, AL.bitwise_and)
                v.drain()
                nc.vector.tensor_copy(out=lo_b[:], in_=lo_i[:])
                v.drain()
                v.wait_ge(psem, 4)        # ireps
                for gi in range(NG):
                    cs = slice(GO[gi], GO[gi] + GS[gi])
                    nc.vector.tensor_tensor(
                        out=_gslice(lm2[:], K2, gi),
                        in0=_ap3(lo_b[:, cs], bcast_mid=K2),
                        in1=_mkap(irep8[:], [list(irep8[:].ap[0]),
                                             [64, K2], [1, GS[gi]]]),
                        op=AL.is_equal).then_inc(vsem, 1)   # 9,11,... (lm2)
                    if gi == 0:
                        v.wait_ge(psem, 7)    # hi_b (Pool, also implies ireps)
                    v.drain()
                    nc.vector.tensor_tensor(
                        out=_gslice(eg2[:], K1, gi),
                        in0=_ap3(hi_b[:, cs], bcast_mid=K1),
                        in1=_mkap(irep32[:], [list(irep32[:].ap[0]),
                                              [64, K1], [1, GS[gi]]]),
                        op=AL.is_equal).then_inc(vsem, 1)   # 10,12,... (eg2)
                # CE + KD finals (reduces went to ACT accum)
                v.wait_ge(asem, 10)       # cep/kdzd/ts accums
                nc.vector.tensor_sub(out=ceb[:], in0=lse_s[:, 0:1], in1=cep[:])
                v.drain()
                nc.vector.tensor_add(out=kdv[:], in0=k1act[:],
                                     in1=kdzd[:]).then_inc(vsem, 1)  # 20 (ceb+kdv)
                v.wait_ge(tsem, 6)        # histogram done
                nc.vector.tensor_reduce(out=r2[:, 0:1], in_=psumW[:, 0:K2],
                                        axis=AX.X, op=AL.add)
                nc.vector.tensor_reduce(out=r2[:, 1:2], in_=psumW[:, K2:2 * K2],
                                        axis=AX.X, op=AL.add)
                nc.vector.tensor_copy(out=w2sb[:], in_=psumW[:]).then_inc(vsem, 1)  # 21
                v.wait_ge(tsem, 7)        # wrev/t1 mms done
                nc.vector.tensor_copy(out=wrevsb[:], in_=psmall[:, SC_WR:SC_WR + 2 * K2])
                nc.vector.tensor_copy(out=t1sb[:], in_=psmall[:, SC_T1:SC_T1 + 2]).then_inc(vsem, 1)  # 22
                v.wait_ge(tsem, 8)        # U/S1 mms done
                nc.vector.tensor_tensor(out=p8[:, 0:K2], in0=m1lo[:],
                                        in1=psmall[0:K2, SC_UT:SC_UT + K2], op=AL.mult)
                nc.vector.tensor_tensor(out=p8[:, K2:2 * K2], in0=m1lo[:],
                                        in1=psmall[0:K2, SC_US:SC_US + K2], op=AL.mult)
                v.drain()
                nc.vector.tensor_reduce(out=s2v[:, 0:1], in_=p8[:, 0:K2],
                                        axis=AX.X, op=AL.add)
                nc.vector.tensor_reduce(out=s2v[:, 1:2], in_=p8[:, K2:2 * K2],
                                        axis=AX.X, op=AL.add).then_inc(vsem, 1)  # 23
                v.wait_ge(tsem, 9)        # all scalar mms done
                nc.vector.tensor_copy(out=sbs[:, 0:11], in_=psmall[0:1, 0:11])
                tt_, ss_, ts_ = sbs[:, 0:1], sbs[:, 1:2], sbs[:, 2:3]
                kd_, ce_ = sbs[:, 3:4], sbs[:, 4:5]
                sg_, sh_ = sbs[:, 5:6], sbs[:, 6:7]
                s1t, s1s, s2t, s2s = (sbs[:, i:i + 1] for i in range(7, 11))
                v.drain()
                nc.vector.tensor_add(out=fs[:, 0:1], in0=s1t, in1=s2t)
                nc.vector.tensor_add(out=fs[:, 1:2], in0=s1s, in1=s2s)
                nc.vector.tensor_mul(out=fs[:, 2:3], in0=tt_, in1=tt_)
                nc.vector.tensor_mul(out=fs[:, 3:4], in0=ss_, in1=ss_)
                nc.vector.tensor_mul(out=fs[:, 4:5], in0=ts_, in1=ts_)
                nc.vector.tensor_add(out=fs[:, 5:6], in0=sg_, in1=sh_)
                v.drain()
                nc.vector.tensor_sub(out=fs[:, 6:7], in0=fs[:, 0:1], in1=fs[:, 1:2])
                nc.vector.tensor_add(out=fs[:, 7:8], in0=fs[:, 2:3], in1=fs[:, 3:4])
                nc.vector.tensor_add(out=fs[:, 8:9], in0=fs[:, 5:6], in1=kd_)
                v.drain()
                nc.vector.scalar_tensor_tensor(out=fs[:, 9:10], in0=fs[:, 4:5],
                                               scalar=-2.0, in1=fs[:, 7:8],
                                               op0=AL.mult, op1=AL.add)
                nc.vector.scalar_tensor_tensor(out=fs[:, 10:11], in0=ce_,
                                               scalar=NT * (1.0 - ALPHA) / B,
                                               in1=fs[:, 8:9],
                                               op0=AL.mult, op1=AL.add)
                v.drain()
                nc.vector.scalar_tensor_tensor(out=fs[:, 11:12], in0=fs[:, 6:7],
                                               scalar=2.0, in1=fs[:, 9:10],
                                               op0=AL.mult, op1=AL.add)
                v.drain()
                nc.vector.scalar_tensor_tensor(out=fs[:, 0:1], in0=fs[:, 11:12],
                                               scalar=0.00025, in1=fs[:, 10:11],
                                               op0=AL.mult, op1=AL.add).then_inc(vsem, 1)  # 24

            # ---------------- PE ----------------
            # tsem: 1=u-fold 2=foldS 3=foldT 4=transposes 5=grams 6=hist
            #       7=wrev+t1 8=U+S1 9=S2+scalars
            @block.tensor
            def _(t):
                t.wait_ge(psem, 2)        # identities
                t.wait_ge(vsem, 6)        # u64
                nc.tensor.matmul(psumF[:], lhsT=e1b[:], rhs=u64[:, 0:HC],
                                 start=True, stop=False)
                nc.tensor.matmul(psumF[:], lhsT=e2b[:], rhs=u64[:, HC:FLAT],
                                 start=False, stop=True).then_inc(tsem, 1)
                t.wait_ge(vsem, 7)        # cube_sb
                nc.tensor.matmul(psumFs[:], lhsT=e1b[:], rhs=cube_sb[:, 0:HC],
                                 start=True, stop=False, skip_group_check=True)
                nc.tensor.matmul(psumFs[:], lhsT=e2b[:], rhs=cube_sb[:, HC:FLAT],
                                 start=False, stop=True,
                                 skip_group_check=True).then_inc(tsem, 1)  # 2
                t.wait_ge(psem, 6)        # cube_tb
                t.wait_ge(vsem, 8)        # cf has finished reading psumF
                nc.tensor.matmul(psumF[:], lhsT=e1b[:], rhs=cube_tb[:, 0:HC],
                                 start=True, stop=False, skip_group_check=True)
                nc.tensor.matmul(psumF[:], lhsT=e2b[:], rhs=cube_tb[:, HC:FLAT],
                                 start=False, stop=True,
                                 skip_group_check=True).then_inc(tsem, 1)  # 3
                ins = None
                for k in range(NT):
                    slc = slice(k * C, (k + 1) * C)
                    nc.tensor.transpose(out=ptrT[:, k, :], in_=cube_tb[:, slc],
                                        identity=id64b[:])
                    ins = nc.tensor.transpose(out=ptrS[:, k, :], in_=cube_sb[:, slc],
                                              identity=id64b[:])
                ins.then_inc(tsem, 1)     # 4
                t.wait_ge(asem, 8)        # trT/trS/trSn
                t.wait_ge(psem, 5)        # cube_snb
                ins = None
                for k in range(NT):
                    slc = slice(k * C, (k + 1) * C)
                    nc.tensor.matmul(psum_hd[:, slc], lhsT=cube_tb[:, slc],
                                     rhs=cube_tb[:, slc], start=True, stop=False,
                                     skip_group_check=True)
                    nc.tensor.matmul(psum_hd[:, slc], lhsT=cube_snb[:, slc],
                                     rhs=cube_sb[:, slc], start=False, stop=True,
                                     skip_group_check=True)
                    nc.tensor.matmul(psum_gd[:, k * 64:(k + 1) * 64],
                                     lhsT=trT[:, k, :], rhs=trT[:, k, :],
                                     start=True, stop=False, skip_group_check=True)
                    ins = nc.tensor.matmul(psum_gd[:, k * 64:(k + 1) * 64],
                                           lhsT=trSn[:, k, :], rhs=trS[:, k, :],
                                           start=False, stop=True,
                                           skip_group_check=True)
                ins.then_inc(tsem, 1)     # 5
                ins = None
                for gi in range(NG):
                    t.wait_ge(vsem, 11 + 2 * gi)
                    t.wait_ge(psem, 10 + gi)
                    for i in range(GS[gi]):
                        ch = GO[gi] + i
                        ins = nc.tensor.matmul(psumW[:],
                                               lhsT=_chunkap(eg2[:], K1, ch),
                                               rhs=_chunkap(tsef2[:], 2 * K2, ch),
                                               start=(ch == 0), stop=(ch == NCHUNK - 1),
                                               skip_group_check=True)
                ins.then_inc(tsem, 1)     # 6: histogram done
                t.wait_ge(vsem, 21)       # w2sb + r2
                nc.tensor.matmul(psmall[:, SC_WR:SC_WR + 2 * K2], lhsT=prev32[:],
                                 rhs=w2sb[:], start=True, stop=True,
                                 skip_group_check=True)
                nc.tensor.matmul(psmall[:, SC_T1:SC_T1 + 2], lhsT=m1[:], rhs=r2[:],
                                 start=True, stop=True,
                                 skip_group_check=True).then_inc(tsem, 1)  # 7
                t.wait_ge(vsem, 22)       # wrevsb + t1sb
                nc.tensor.matmul(psmall[0:K2, SC_UT:SC_UT + K2],
                                 lhsT=w2sb[:, 0:K2], rhs=wrevsb[:, 0:K2],
                                 start=True, stop=True, skip_group_check=True)
                nc.tensor.matmul(psmall[0:K2, SC_US:SC_US + K2],
                                 lhsT=w2sb[:, K2:2 * K2], rhs=wrevsb[:, K2:2 * K2],
                                 start=True, stop=True, skip_group_check=True)
                nc.tensor.matmul(psmall[0:1, SC_S1T:SC_S1T + 1], lhsT=t1sb[:, 0:1],
                                 rhs=r2[:, 0:1], start=True, stop=True,
                                 skip_group_check=True)
                nc.tensor.matmul(psmall[0:1, SC_S1S:SC_S1S + 1], lhsT=t1sb[:, 1:2],
                                 rhs=r2[:, 1:2], start=True, stop=True,
                                 skip_group_check=True).then_inc(tsem, 1)  # 8
                t.wait_ge(asem, 7)        # tt128/ss128
                nc.tensor.matmul(psmall[0:1, SC_TT:SC_TT + 1], lhsT=ones[:, 0:1],
                                 rhs=tt128[:], start=True, stop=True,
                                 skip_group_check=True)
                nc.tensor.matmul(psmall[0:1, SC_SS:SC_SS + 1], lhsT=ones[:, 0:1],
                                 rhs=ss128[:], start=True, stop=True,
                                 skip_group_check=True)
                t.wait_ge(asem, 10)       # ts128
                nc.tensor.matmul(psmall[0:1, SC_TS:SC_TS + 1], lhsT=ones[:, 0:1],
                                 rhs=ts128[:], start=True, stop=True,
                                 skip_group_check=True)
                nc.tensor.matmul(psmall[0:1, SC_KD:SC_KD + 1], lhsT=ones[0:64, 0:1],
                                 rhs=kdv[:], start=True, stop=True,
                                 skip_group_check=True)
                t.wait_ge(vsem, 20)       # ceb+kdv
                nc.tensor.matmul(psmall[0:1, SC_CE:SC_CE + 1], lhsT=ones[0:64, 0:1],
                                 rhs=ceb[:], start=True, stop=True,
                                 skip_group_check=True)
                t.wait_ge(asem, 11)       # accg/acch
                nc.tensor.matmul(psmall[0:1, SC_SG:SC_SG + 1], lhsT=ones[0:64, 0:1],
                                 rhs=accg[:], start=True, stop=True,
                                 skip_group_check=True)
                nc.tensor.matmul(psmall[0:1, SC_SH:SC_SH + 1], lhsT=ones[0:100, 0:1],
                                 rhs=acch[:], start=True, stop=True,
                                 skip_group_check=True)
                t.wait_ge(vsem, 23)       # s2v
                nc.tensor.matmul(psmall[0:1, SC_S2T:SC_S2T + 1], lhsT=ones[0:K2, 0:1],
                                 rhs=s2v[:, 0:1], start=True, stop=True,
                                 skip_group_check=True)
                nc.tensor.matmul(psmall[0:1, SC_S2S:SC_S2S + 1], lhsT=ones[0:K2, 0:1],
                                 rhs=s2v[:, 1:2], start=True, stop=True,
                                 skip_group_check=True).then_inc(tsem, 1)  # 9

    return nc


_cache = {}


def _get_nc():
    if "nc" not in _cache:
        _cache["nc"] = build()
    return _cache["nc"]


def kernel(logits_student, logits_teacher, target):
    from concourse.bass_utils import run_bass_kernel_spmd

    nc = _get_nc()
    in_map = {
        "logits_student": np.ascontiguousarray(logits_student, dtype=np.float32),
        "logits_teacher": np.ascontiguousarray(logits_teacher, dtype=np.float32),
        "target": np.ascontiguousarray(np.asarray(target).reshape(B, 1).astype(np.int32)),
    }
    core_ids = list(range(8))
    res = run_bass_kernel_spmd(nc, [in_map] * 8, core_ids)
    out = res.results[0]["out"]
    return np.float32(out.reshape(())).reshape(())


# revision 34
# speedup vs baseline: 2.4737x; 1.0059x over previous
# Trainium2 Bass kernel for nn_CKDLoss: KD loss + virtual-outer-product L1/L2
# + Gram-matrix sub-losses, computed entirely on device.
#
# Sharding: total work after algorithmic reduction is tiny and latency-bound;
# cross-core collectives cost more than the whole computation. Every core runs
# the identical full computation on replicated inputs; host takes core 0.
#
# L1 math: with u = log s - log t (normalized softmax cubes flattened to N),
#   sum_{a,b} |t_a t_b - s_a s_b| = 2*(S_tt - S_ss)   (T = S = 320 cancel),
#   S_tt = sum_{pairs: u_a+u_b<0} t_a t_b.
# Key identity: u = (sl - tl)/T + (lse_t - lse_s) per element — no exp/log of
# cube values on the u path; five tensor_scalar ops on the raw logits plus
# the row-lse bias build u64 exactly.
# Bucketize c = floor((u+UMAX)*K/(2 UMAX)) in [0,K), c = 8*hi + lo.
# The positive-pair test c_a+c_b <= K-2 splits exactly into
#   (hi_a+hi_b <= K1-2)  OR  (hi_a+hi_b = K1-1 AND lo_a+lo_b <= K2-2).
# Build W[hi, lo] = sum_n t_n 1[hi_n=hi] 1[lo_n=lo] (PSUM-accumulated one-hot
# matmuls, bf16). Reversed rows Wrev = P_antidiag @ W via one permutation
# matmul. Then with r[hi] = sum_lo W[hi, lo]:
#   S1 = r^T M1 r                 (M1[a,b] = 1[a+b<=K1-2])
#   U  = W^T Wrev  (8x8),  S2 = sum m1lo * U   (m1lo[a,b] = 1[a+b<=K2-2])
#   S_tt = S1 + S2.
# bf16 in the heavy path is safe: measured end-to-end shift vs f32 is ~1e-5
# of the loss (the bucketization itself is 2.3e-3).
#
# Layout: [64, 512] flat tensors (500 data + 12 zero pad) fold to [128, 256]
# via two permutation matmuls (split at flat col 256). f32r at 256 output
# columns runs 1 cycle/row, so the u-fold costs ~0.5us. Pad elements carry
# zero weight everywhere, so they never contribute. One-hots are built in
# [128, K, chunk] layout (bucket index as MIDDLE dim) so all build operands
# have packed 2-byte last dims -> DVE 2x mode. Pool (gpsimd) cannot run
# is_equal/shift through the walrus codegen, so DVE builds the one-hots and
# Pool applies the t/s weights. A PSUM bank is read by at most one engine
# per phase (HW forbids concurrent multi-engine reads of a bank).

import numpy as np
from contextlib import ExitStack

B, C, NT = 64, 100, 5            # batch, classes, temps 1..5
N = B * C * NT                   # 32000 flattened cube elements
K1, K2 = 32, 8                   # two-level bucket split, K = 256
K = K1 * K2
UMAX = 16.0
INVW = K / (2.0 * UMAX)          # 8.0
ALPHA = 0.7
FLAT = 512                       # padded flat width (500 data + 12 pad)
HC = FLAT // 2                   # 256 folded columns
NCHUNK = HC                      # 256 PE chunks of 128 elements
NG = 5
GS = [64, 64, 64, 44, 20]        # descending group sizes (small tail group)
GO = [0, 64, 128, 192, 236]


def _mkap(tensor_ap, dims, extra_off=0):
    import concourse.bass as bass
    return bass.AP(tensor=tensor_ap.tensor, offset=tensor_ap.offset + extra_off,
                   ap=[list(d) for d in dims])


def _ap3(ap, bcast_inner=None, bcast_mid=None):
    """Append/insert stride-0 dims on an AP: [P,F] -> [P,F,bi] or [P,bm,F]."""
    dims = [list(d) for d in ap.ap]
    if bcast_inner is not None:
        dims = dims + [[0, bcast_inner]]
    if bcast_mid is not None:
        dims = [dims[0], [0, bcast_mid]] + dims[1:]
    return _mkap(ap, dims)


def _gslice(t_ap, ncols, gi, colhalf=0):
    """[128, ncols, NCHUNK] tensor -> [128, ncols, GS[gi]] AP for group gi."""
    dims = [list(t_ap.ap[0]), [NCHUNK, ncols], [1, GS[gi]]]
    return _mkap(t_ap, dims, extra_off=colhalf * K2 * NCHUNK + GO[gi])


def _chunkap(t_ap, ncols, ch):
    """[128, ncols, NCHUNK] tensor -> [128, ncols] AP for chunk ch."""
    dims = [list(t_ap.ap[0]), [NCHUNK, ncols]]
    return _mkap(t_ap, dims, extra_off=ch)


def build():
    import concourse.bass as bass
    from concourse import mybir

    dt = mybir.dt
    AL = mybir.AluOpType
    AF = mybir.ActivationFunctionType
    AX = mybir.AxisListType
    bf = dt.bfloat16
    fr = dt.float32r

    nc = bass.Bass()
    ls_d = nc.declare_dram_parameter("logits_student", [B, C], dt.float32, isOutput=False)
    lt_d = nc.declare_dram_parameter("logits_teacher", [B, C], dt.float32, isOutput=False)
    tg_d = nc.declare_dram_parameter("target", [B, 1], dt.int32, isOutput=False)
    out_d = nc.declare_dram_parameter("out", [1, 1], dt.float32, isOutput=True)

    ctx = ExitStack()
    _n = [0]

    def sb(shape, d=dt.float32):
        _n[0] += 1
        return ctx.enter_context(nc.sbuf_tensor(f"sb{_n[0]}", shape, d))

    def ps(shape, d=dt.float32):
        _n[0] += 1
        return ctx.enter_context(nc.psum_tensor(f"ps{_n[0]}", shape, d))

    with ctx:
        # ---- constants ----
        ones = sb([128, 1])
        ones_b = sb([128, 1], bf)
        iota100 = sb([64, C])
        onesq = sb([32, 32])
        m1 = sb([32, 32])                 # 1[a+b<=K1-2]
        m1lo = sb([K2, K2])               # 1[a+b<=K2-2]
        prev32 = sb([32, 32])             # antidiagonal permutation 1[a+b=31]
        id64b = sb([64, 64], bf)
        e1b = sb([64, 128], bf)           # fold identity half 1: 1[f==p]
        e2b = sb([64, 128], bf)           # fold identity half 2: 1[f==p+64]
        wrow64 = sb([64, NT])             # KD weights -a*T^2/(B*C)
        irep32 = sb([128, K1, 64], bf)    # irep32[p, j, m] = j
        irep8 = sb([128, K2, 64], bf)
        # ---- inputs ----
        sl_ = sb([64, C])
        tl_ = sb([64, C])
        tg = sb([64, 1], dt.int32)
        # ---- softmax / u stage ----
        sls = sb([64, NT * C])            # student logits / T
        tls = sb([64, NT * C])            # teacher logits / T
        d0 = sb([64, C])                  # sl - tl
        u64 = sb([64, FLAT], bf)          # (sl-tl)/T + zd, padded
        es = sb([64, NT * C], bf)         # exp(student)
        et = sb([64, NT * C], bf)
        se_s = sb([64, NT], bf)
        se_t = sb([64, NT], bf)
        rs_s = sb([64, NT])
        rs_t = sb([64, NT])
        rs_sn = sb([64, NT])
        lse_s = sb([64, NT])
        lse_t = sb([64, NT])
        zd = sb([64, NT])
        cube_tb = sb([64, FLAT], bf)      # normalized teacher cube (padded)
        cube_sb = sb([64, FLAT], bf)
        cube_snb = sb([64, FLAT], bf)
        # ---- folded [128, 256] ----
        t128b = sb([128, HC], bf)
        s128b = sb([128, HC], bf)
        cf = sb([128, HC])
        ci = sb([128, HC], dt.int32)
        hi_i = sb([128, HC], dt.int32)
        lo_i = sb([128, HC], dt.int32)
        hi_b = sb([128, HC], bf)
        lo_b = sb([128, HC], bf)
        # ---- histogram build ----
        eg2 = sb([128, K1, NCHUNK], bf)
        lm2 = sb([128, K2, NCHUNK], bf)
        tsef2 = sb([128, 2 * K2, NCHUNK], bf)
        # ---- grams ----
        trT = sb([100, NT, 64], bf)
        trS = sb([100, NT, 64], bf)
        trSn = sb([100, NT, 64], bf)
        sqg = sb([64, NT * 64], bf)
        sqh = sb([100, NT * C], bf)
        accg = sb([64, 1])
        acch = sb([100, 1])
        # ---- L2 / KD / CE ----
        sq_t = sb([128, HC], bf)
        sq_s = sb([128, HC], bf)
        tt128 = sb([128, 1])
        ss128 = sb([128, 1])
        tsprod = sb([128, HC], bf)
        ts128 = sb([128, 1])
        d0b = sb([64, C], bf)
        cw = sb([64, NT * C], bf)         # cube_tb * (w_T/T)
        cwp = sb([64, NT * C], bf)        # cw * d0
        kscr = sb([64, NT * C], bf)       # ACT accum scratch
        cscr = sb([64, C], bf)            # CE accum scratch
        zscr = sb([64, NT], bf)
        tscr = sb([128, HC], bf)
        k1act = sb([64, 1])
        kz = sb([64, NT])
        kdzd = sb([64, 1])
        kdv = sb([64, 1])
        tgf = sb([64, 1])
        oh = sb([64, C])
        ohs = sb([64, C])
        cep = sb([64, 1])
        ceb = sb([64, 1])
        # ---- contraction ----
        w2sb = sb([32, 2 * K2])
        wrevsb = sb([32, 2 * K2])
        r2 = sb([32, 2])
        t1sb = sb([32, 2])
        p8 = sb([K2, 2 * K2])
        s2v = sb([K2, 2])
        # ---- final ----
        sbs = sb([1, 16])
        fs = sb([1, 12])
        warm = sb([1, 1])
        # ---- PSUM: 8 tensors = 8 banks ----
        psumF = ps([128, HC])             # u-fold, then teacher fold
        psumFs = ps([128, HC])            # student fold
        ptrT = ps([100, NT, 64], bf)
        ptrS = ps([100, NT, 64], bf)
        psum_gd = ps([64, NT * 64])
        psum_hd = ps([100, NT * C])
        psumW = ps([K1, 2 * K2])
        psmall = ps([32, 64])

        SC_TT, SC_SS, SC_TS, SC_KD, SC_CE, SC_SG, SC_SH = 0, 1, 2, 3, 4, 5, 6
        SC_S1T, SC_S1S, SC_S2T, SC_S2S = 7, 8, 9, 10
        SC_UT, SC_US = 16, 24
        SC_T1 = 32
        SC_WR = 40

        wv = [-ALPHA * T * T / (B * C) for T in range(1, NT + 1)]

        with (
            nc.semaphore("d_in") as d_in,
            nc.semaphore("d_tl") as d_tl,
            nc.semaphore("d_tg") as d_tg,
            nc.semaphore("d_out") as d_out,
            nc.semaphore("vsem") as vsem,
            nc.semaphore("psem") as psem,
            nc.semaphore("asem") as asem,
            nc.semaphore("tsem") as tsem,
            nc.Block() as block,
        ):
            # ---------------- SP: DMA only ----------------
            @block.sync
            def _(s):
                s.dma_start(out=sl_[:], in_=ls_d[:, :]).then_inc(d_in, 16)
                s.dma_start(out=tl_[:], in_=lt_d[:, :]).then_inc(d_tl, 16)
                s.dma_start(out=tg[:], in_=tg_d[:, :]).then_inc(d_tg, 16)
                s.wait_ge(vsem, 24)       # final scalar ready
                s.dma_start(out=out_d[:, :], in_=fs[:, 0:1]).then_inc(d_out, 16)
                s.wait_ge(d_out, 16)

            # ---------------- Pool ----------------
            # psem: 1=ones 2=quick constants 3=tls 4=ireps 5=cube_snb
            #       6=cube_tb 7=tsprod 8=hi_b 9..13=tsef2 groups
            @block.gpsimd
            def _(g):
                g.memset(ones[:], 1.0).then_inc(psem, 1)   # 1: ACT warmup gate
                g.memset(ones_b[:], 1.0)
                g.memset(onesq[:], 1.0)
                g.memset(id64b[:], 0.0)
                g.memset(prev32[:], 0.0)
                g.memset(e1b[:], 0.0)
                g.memset(e2b[:], 0.0)
                g.iota(iota100[:], [[1, C]], channel_multiplier=0,
                       allow_small_or_imprecise_dtypes=True)
                for T in range(1, NT + 1):
                    g.memset(wrow64[:, T - 1:T], wv[T - 1])
                # zero pads (data cols are written later; ranges are disjoint)
                g.memset(u64[:, NT * C:FLAT], 0.0)
                g.memset(cube_tb[:, NT * C:FLAT], 0.0)
                g.memset(cube_sb[:, NT * C:FLAT], 0.0)
                g.memset(cube_snb[:, NT * C:FLAT], 0.0)
                g.drain()
                g.affine_select(m1[:], onesq[:], [[-1, 32]], AL.is_ge, 0.0,
                                base=K1 - 2, channel_multiplier=-1)
                g.affine_select(m1lo[:], onesq[0:K2, 0:K2], [[-1, K2]], AL.is_ge,
                                0.0, base=K2 - 2, channel_multiplier=-1)
                g.affine_select(id64b[:], id64b[:], [[-1, 64]], AL.not_equal,
                                1.0, base=0, channel_multiplier=1)
                g.affine_select(prev32[:], prev32[:], [[-1, 32]], AL.not_equal,
                                1.0, base=K1 - 1, channel_multiplier=-1)
                g.affine_select(e1b[:], e1b[:], [[-1, 128]], AL.not_equal,
                                1.0, base=0, channel_multiplier=1)
                g.affine_select(e2b[:], e2b[:], [[-1, 128]], AL.not_equal,
                                1.0, base=64, channel_multiplier=1).then_inc(psem, 1)   # 2
                # teacher prescale (ahead of the slow irep iotas)
                g.wait_ge(d_tl, 16)
                ins = None
                for T in range(1, NT + 1):
                    i = T - 1
                    ins = nc.gpsimd.tensor_scalar_mul(
                        tls[:, i * C:(i + 1) * C], tl_[:], 1.0 / T)
                ins.then_inc(psem, 1)     # 3: tls
                g.iota(irep32[:], [[1, K1], [0, 64]], channel_multiplier=0,
                       allow_small_or_imprecise_dtypes=True)
                g.iota(irep8[:], [[1, K2], [0, 64]], channel_multiplier=0,
                       allow_small_or_imprecise_dtypes=True).then_inc(psem, 1)  # 4
                g.wait_ge(vsem, 4)        # rs_sn
                ins = None
                for T in range(1, NT + 1):
                    i = T - 1
                    slc = slice(i * C, (i + 1) * C)
                    ins = nc.gpsimd.tensor_scalar_mul(cube_snb[:, slc], es[:, slc],
                                                      rs_sn[:, i:i + 1])
                ins.then_inc(psem, 1)     # 5: cube_snb
                g.wait_ge(vsem, 5)        # rs_t
                ins = None
                for T in range(1, NT + 1):
                    i = T - 1
                    slc = slice(i * C, (i + 1) * C)
                    ins = nc.gpsimd.tensor_scalar_mul(cube_tb[:, slc], et[:, slc],
                                                      rs_t[:, i:i + 1])
                ins.then_inc(psem, 1)     # 6: cube_tb
                g.wait_ge(vsem, 9)        # hi_i (DVE shift)
                nc.gpsimd.tensor_copy(out=hi_b[:], in_=hi_i[:]).then_inc(psem, 1)  # 7
                g.wait_ge(asem, 6)        # t128b + s128b (ACT copies)
                nc.gpsimd.tensor_tensor(out=tsprod[:], in0=t128b[:], in1=s128b[:],
                                        op=AL.mult).then_inc(psem, 1)  # 8: tsprod
                # KD product pieces (consumed by one ACT accum op)
                nc.gpsimd.tensor_copy(out=d0b[:], in_=d0[:])
                ins = None
                for T in range(1, NT + 1):
                    i = T - 1
                    ins = nc.gpsimd.tensor_scalar_mul(
                        cw[:, i * C:(i + 1) * C], cube_tb[:, i * C:(i + 1) * C],
                        wv[i] / T)
                g.drain()
                nc.gpsimd.tensor_tensor(out=cwp[:], in0=cw[:],
                                        in1=_ap3(d0b[:], bcast_mid=NT),
                                        op=AL.mult).then_inc(psem, 1)  # 9: cwp
                for gi in range(NG):
                    cs = slice(GO[gi], GO[gi] + GS[gi])
                    g.wait_ge(vsem, 10 + 2 * gi)   # lm2 group built
                    nc.gpsimd.tensor_tensor(
                        out=_gslice(tsef2[:], K2, gi),
                        in0=_gslice(lm2[:], K2, gi),
                        in1=_ap3(t128b[:, cs], bcast_mid=K2), op=AL.mult)
                    g.drain()
                    nc.gpsimd.tensor_tensor(
                        out=_gslice(tsef2[:], K2, gi, colhalf=1),
                        in0=_gslice(lm2[:], K2, gi),
                        in1=_ap3(s128b[:, cs], bcast_mid=K2),
                        op=AL.mult).then_inc(psem, 1)   # 10..14


            # ---------------- ACT ----------------
            # asem: 1=exp_s 2=exp_t 3=lse_s 4=lse_t 5=s128b 6=t128b
            #       7=L2 squares 8=tr copies 9=gram squares
            @block.scalar
            def _(a):
                a.wait_ge(psem, 1)
                nc.scalar.activation(out=warm[:], in_=ones[0:1, :], func=AF.Exp)
                a.wait_ge(vsem, 1)        # sls
                nc.scalar.activation(out=es[:], in_=sls[:], func=AF.Exp).then_inc(asem, 1)
                a.wait_ge(psem, 3)        # tls
                nc.scalar.activation(out=et[:], in_=tls[:], func=AF.Exp).then_inc(asem, 1)
                a.wait_ge(vsem, 2)        # se_s
                nc.scalar.activation(out=lse_s[:], in_=se_s[:],
                                     func=AF.Ln).then_inc(asem, 1)
                a.wait_ge(vsem, 3)        # se_t (before recips)
                nc.scalar.activation(out=lse_t[:], in_=se_t[:],
                                     func=AF.Ln).then_inc(asem, 1)
                a.wait_ge(tsem, 2)        # fold S in PSUM
                nc.scalar.activation(out=s128b[:], in_=psumFs[:],
                                     func=AF.Identity).then_inc(asem, 1)  # 5
                a.wait_ge(tsem, 3)        # fold T in PSUM
                nc.scalar.activation(out=t128b[:], in_=psumF[:],
                                     func=AF.Identity).then_inc(asem, 1)  # 6
                nc.scalar.activation(out=sq_t[:], in_=psumF[:], func=AF.Square,
                                     accum_out=tt128[:])
                nc.scalar.activation(out=sq_s[:], in_=psumFs[:], func=AF.Square,
                                     accum_out=ss128[:]).then_inc(asem, 1)  # 7
                a.wait_ge(tsem, 4)        # transposes done
                nc.scalar.activation(out=trT[:], in_=ptrT[:], func=AF.Identity)
                nc.scalar.activation(out=trS[:], in_=ptrS[:], func=AF.Identity)
                nc.scalar.activation(out=trSn[:], in_=ptrS[:], func=AF.Identity,
                                     scale=-1.0).then_inc(asem, 1)   # 8
                a.wait_ge(psem, 9)        # cwp
                nc.scalar.activation(out=kscr[:], in_=cwp[:], func=AF.Identity,
                                     accum_out=k1act[:]).then_inc(asem, 1)  # 9
                a.wait_ge(vsem, 8)        # ohs + kz written (cf implies both)
                a.wait_ge(psem, 8)        # tsprod
                nc.scalar.activation(out=cscr[:], in_=ohs[:], func=AF.Identity,
                                     accum_out=cep[:])
                nc.scalar.activation(out=zscr[:], in_=kz[:], func=AF.Identity,
                                     accum_out=kdzd[:])
                nc.scalar.activation(out=tscr[:], in_=tsprod[:], func=AF.Identity,
                                     accum_out=ts128[:]).then_inc(asem, 1)  # 10
                a.wait_ge(tsem, 5)        # gram mms done
                nc.scalar.activation(out=sqg[:], in_=psum_gd[:], func=AF.Square,
                                     accum_out=accg[:])
                nc.scalar.activation(out=sqh[:], in_=psum_hd[:], func=AF.Square,
                                     accum_out=acch[:]).then_inc(asem, 1)  # 11

            # ---------------- DVE ----------------
            # vsem: 1=sls 2=se_s 3=rs_sn 4=se_t+recip_t 5=u64 6=cube_sb 7=ceb
            #       8=cf 9=hi_i  10,12,14,16,18=lm2  11,13,15,17,19=eg2
            #       20=kdv+ts128 21=w2sb+r2 22=wrevsb+t1sb 23=s2v 24=final
            @block.vector
            def _(v):
                v.wait_ge(d_in, 16)
                ins = None
                for T in range(1, NT + 1):
                    i = T - 1
                    ins = nc.vector.tensor_scalar_mul(
                        sls[:, i * C:(i + 1) * C], sl_[:], 1.0 / T)
                ins.then_inc(vsem, 1)
                v.wait_ge(d_tl, 16)
                nc.vector.tensor_sub(out=d0[:], in0=sl_[:], in1=tl_[:])
                v.wait_ge(asem, 1)        # exp_s
                v.drain()
                with nc.allow_low_precision(reason="se sums tolerate bf16"):
                    nc.vector.tensor_reduce(out=se_s[:], in_=es[:].rearrange(
                        "p (t c) -> p t c", t=NT), axis=AX.X, op=AL.add).then_inc(vsem, 1)
                v.wait_ge(asem, 2)        # exp_t
                v.drain()
                with nc.allow_low_precision(reason="se sums tolerate bf16"):
                    nc.vector.tensor_reduce(out=se_t[:], in_=et[:].rearrange(
                        "p (t c) -> p t c", t=NT), axis=AX.X, op=AL.add).then_inc(vsem, 1)  # 3
                v.drain()
                nc.vector.reciprocal(out=rs_s[:], in_=se_s[:])
                v.drain()
                nc.vector.tensor_scalar_mul(rs_sn[:], rs_s[:], -1.0).then_inc(vsem, 1)  # 4
                v.drain()
                nc.vector.reciprocal(out=rs_t[:], in_=se_t[:]).then_inc(vsem, 1)  # 5
                v.wait_ge(asem, 4)        # lse_t (and lse_s)
                nc.vector.tensor_sub(out=zd[:], in0=lse_t[:], in1=lse_s[:])
                v.drain()
                ins = None
                for T in range(1, NT + 1):
                    i = T - 1
                    ins = nc.vector.tensor_scalar(
                        u64[:, i * C:(i + 1) * C], d0[:], 1.0 / T, zd[:, i:i + 1],
                        AL.mult, AL.add)
                ins.then_inc(vsem, 1)     # 6: u64
                nc.vector.tensor_tensor(out=kz[:], in0=zd[:], in1=wrow64[:],
                                        op=AL.mult)
                ins = None
                for T in range(1, NT + 1):
                    i = T - 1
                    slc = slice(i * C, (i + 1) * C)
                    ins = nc.vector.tensor_scalar_mul(cube_sb[:, slc], es[:, slc],
                                                      rs_s[:, i:i + 1])
                ins.then_inc(vsem, 1)     # 7: cube_sb
                # CE one-hot products fill the wait for the u-fold
                v.wait_ge(d_tg, 16)
                nc.vector.tensor_copy(out=tgf[:], in_=tg[:])
                v.drain()
                nc.vector.tensor_tensor(out=oh[:],
                                        in0=_ap3(tgf[:], bcast_inner=C)[:, 0, :],
                                        in1=iota100[:], op=AL.is_equal)
                v.drain()
                nc.vector.tensor_tensor(out=ohs[:], in0=oh[:], in1=sl_[:], op=AL.mult)
                # bucket chain straight from the u-fold PSUM (sole reader here)
                v.wait_ge(tsem, 1)        # u-fold done
                nc.vector.tensor_scalar(cf[:], psumF[:], INVW, K / 2.0 - 0.5,
                                        AL.mult, AL.add).then_inc(vsem, 1)  # 8
                v.drain()
                nc.vector.tensor_copy(out=ci[:], in_=cf[:])
                v.drain()
                nc.vector.tensor_scalar(hi_i[:], ci[:], 3, None,
                                        AL.arith_shift_right).then_inc(vsem, 1)  # 9
                nc.vector.tensor_scalar(lo_i[:], ci[:], 7, None, AL.bitwise_and)
                v.drain()
                nc.vector.tensor_copy(out=lo_b[:], in_=lo_i[:])
                v.drain()
                v.wait_ge(psem, 4)        # ireps
                for gi in range(NG):
                    cs = slice(GO[gi], GO[gi] + GS[gi])
                    nc.vector.tensor_tensor(
                        out=_gslice(lm2[:], K2, gi),
                        in0=_ap3(lo_b[:, cs], bcast_mid=K2),
                        in1=_mkap(irep8[:], [list(irep8[:].ap[0]),
                                             [64, K2], [1, GS[gi]]]),
                        op=AL.is_equal).then_inc(vsem, 1)   # 9,11,... (lm2)
                    if gi == 0:
                        v.wait_ge(psem, 7)    # hi_b (Pool, also implies ireps)
                    v.drain()
                    nc.vector.tensor_tensor(
                        out=_gslice(eg2[:], K1, gi),
                        in0=_ap3(hi_b[:, cs], bcast_mid=K1),
                        in1=_mkap(irep32[:], [list(irep32[:].ap[0]),
                                              [64, K1], [1, GS[gi]]]),
                        op=AL.is_equal).then_inc(vsem, 1)   # 10,12,... (eg2)
                # CE + KD finals (reduces went to ACT accum)
                v.wait_ge(asem, 10)       # cep/kdzd/ts accums
                nc.vector.tensor_sub(out=ceb[:], in0=lse_s[:, 0:1], in1=cep[:])
                v.drain()
                nc.vector.tensor_add(out=kdv[:], in0=k1act[:],
                                     in1=kdzd[:]).then_inc(vsem, 1)  # 20 (ceb+kdv)
                v.wait_ge(tsem, 6)        # histogram done
                nc.vector.tensor_reduce(out=r2[:, 0:1], in_=psumW[:, 0:K2],
                                        axis=AX.X, op=AL.add)
                nc.vector.tensor_reduce(out=r2[:, 1:2], in_=psumW[:, K2:2 * K2],
                                        axis=AX.X, op=AL.add)
                nc.vector.tensor_copy(out=w2sb[:], in_=psumW[:]).then_inc(vsem, 1)  # 21
                v.wait_ge(tsem, 7)        # wrev/t1 mms done
                nc.vector.tensor_copy(out=wrevsb[:], in_=psmall[:, SC_WR:SC_WR + 2 * K2])
                nc.vector.tensor_copy(out=t1sb[:], in_=psmall[:, SC_T1:SC_T1 + 2]).then_inc(vsem, 1)  # 22
                v.wait_ge(tsem, 8)        # U/S1 mms done
                nc.vector.tensor_tensor(out=p8[:, 0:K2], in0=m1lo[:],
                                        in1=psmall[0:K2, SC_UT:SC_UT + K2], op=AL.mult)
                nc.vector.tensor_tensor(out=p8[:, K2:2 * K2], in0=m1lo[:],
                                        in1=psmall[0:K2, SC_US:SC_US + K2], op=AL.mult)
                v.drain()
                nc.vector.tensor_reduce(out=s2v[:, 0:1], in_=p8[:, 0:K2],
                                        axis=AX.X, op=AL.add)
                nc.vector.tensor_reduce(out=s2v[:, 1:2], in_=p8[:, K2:2 * K2],
                                        axis=AX.X, op=AL.add).then_inc(vsem, 1)  # 23
                v.wait_ge(tsem, 9)        # all scalar mms done
                nc.vector.tensor_copy(out=sbs[:, 0:11], in_=psmall[0:1, 0:11])
                tt_, ss_, ts_ = sbs[:, 0:1], sbs[:, 1:2], sbs[:, 2:3]
                kd_, ce_ = sbs[:, 3:4], sbs[:, 4:5]
                sg_, sh_ = sbs[:, 5:6], sbs[:, 6:7]
                s1t, s1s, s2t, s2s = (sbs[:, i:i + 1] for i in range(7, 11))
                v.drain()
                nc.vector.tensor_add(out=fs[:, 0:1], in0=s1t, in1=s2t)
                nc.vector.tensor_add(out=fs[:, 1:2], in0=s1s, in1=s2s)
                nc.vector.tensor_mul(out=fs[:, 2:3], in0=tt_, in1=tt_)
                nc.vector.tensor_mul(out=fs[:, 3:4], in0=ss_, in1=ss_)
                nc.vector.tensor_mul(out=fs[:, 4:5], in0=ts_, in1=ts_)
                nc.vector.tensor_add(out=fs[:, 5:6], in0=sg_, in1=sh_)
                v.drain()
                nc.vector.tensor_sub(out=fs[:, 6:7], in0=fs[:, 0:1], in1=fs[:, 1:2])
                nc.vector.tensor_add(out=fs[:, 7:8], in0=fs[:, 2:3], in1=fs[:, 3:4])
                nc.vector.tensor_add(out=fs[:, 8:9], in0=fs[:, 5:6], in1=kd_)
                v.drain()
                nc.vector.scalar_tensor_tensor(out=fs[:, 9:10], in0=fs[:, 4:5],
                                               scalar=-2.0, in1=fs[:, 7:8],
                                               op0=AL.mult, op1=AL.add)
                nc.vector.scalar_tensor_tensor(out=fs[:, 10:11], in0=ce_,
                                               scalar=NT * (1.0 - ALPHA) / B,
                                               in1=fs[:, 8:9],
                                               op0=AL.mult, op1=AL.add)
                v.drain()
                nc.vector.scalar_tensor_tensor(out=fs[:, 11:12], in0=fs[:, 6:7],
                                               scalar=2.0, in1=fs[:, 9:10],
                                               op0=AL.mult, op1=AL.add)
                v.drain()
                nc.vector.scalar_tensor_tensor(out=fs[:, 0:1], in0=fs[:, 11:12],
                                               scalar=0.00025, in1=fs[:, 10:11],
                                               op0=AL.mult, op1=AL.add).then_inc(vsem, 1)  # 24

            # ---------------- PE ----------------
            # tsem: 1=u-fold 2=foldS 3=foldT 4=transposes 5=grams 6=hist
            #       7=wrev+t1 8=U+S1 9=S2+scalars
            @block.tensor
            def _(t):
                t.wait_ge(psem, 2)        # identities
                t.wait_ge(vsem, 6)        # u64
                nc.tensor.matmul(psumF[:], lhsT=e1b[:], rhs=u64[:, 0:HC],
                                 start=True, stop=False)
                nc.tensor.matmul(psumF[:], lhsT=e2b[:], rhs=u64[:, HC:FLAT],
                                 start=False, stop=True).then_inc(tsem, 1)
                t.wait_ge(vsem, 7)        # cube_sb
                nc.tensor.matmul(psumFs[:], lhsT=e1b[:], rhs=cube_sb[:, 0:HC],
                                 start=True, stop=False, skip_group_check=True)
                nc.tensor.matmul(psumFs[:], lhsT=e2b[:], rhs=cube_sb[:, HC:FLAT],
                                 start=False, stop=True,
                                 skip_group_check=True).then_inc(tsem, 1)  # 2
                t.wait_ge(psem, 6)        # cube_tb
                t.wait_ge(vsem, 8)        # cf has finished reading psumF
                nc.tensor.matmul(psumF[:], lhsT=e1b[:], rhs=cube_tb[:, 0:HC],
                                 start=True, stop=False, skip_group_check=True)
                nc.tensor.matmul(psumF[:], lhsT=e2b[:], rhs=cube_tb[:, HC:FLAT],
                                 start=False, stop=True,
                                 skip_group_check=True).then_inc(tsem, 1)  # 3
                ins = None
                for k in range(NT):
                    slc = slice(k * C, (k + 1) * C)
                    nc.tensor.transpose(out=ptrT[:, k, :], in_=cube_tb[:, slc],
                                        identity=id64b[:])
                    ins = nc.tensor.transpose(out=ptrS[:, k, :], in_=cube_sb[:, slc],
                                              identity=id64b[:])
                ins.then_inc(tsem, 1)     # 4
                t.wait_ge(asem, 8)        # trT/trS/trSn
                t.wait_ge(psem, 5)        # cube_snb
                ins = None
                for k in range(NT):
                    slc = slice(k * C, (k + 1) * C)
                    nc.tensor.matmul(psum_hd[:, slc], lhsT=cube_tb[:, slc],
                                     rhs=cube_tb[:, slc], start=True, stop=False,
                                     skip_group_check=True)
                    nc.tensor.matmul(psum_hd[:, slc], lhsT=cube_snb[:, slc],
                                     rhs=cube_sb[:, slc], start=False, stop=True,
                                     skip_group_check=True)
                    nc.tensor.matmul(psum_gd[:, k * 64:(k + 1) * 64],
                                     lhsT=trT[:, k, :], rhs=trT[:, k, :],
                                     start=True, stop=False, skip_group_check=True)
                    ins = nc.tensor.matmul(psum_gd[:, k * 64:(k + 1) * 64],
                                           lhsT=trSn[:, k, :], rhs=trS[:, k, :],
                                           start=False, stop=True,
                                           skip_group_check=True)
                ins.then_inc(tsem, 1)     # 5
                ins = None
                for gi in range(NG):
                    t.wait_ge(vsem, 11 + 2 * gi)
                    t.wait_ge(psem, 10 + gi)
                    for i in range(GS[gi]):
                        ch = GO[gi] + i
                        ins = nc.tensor.matmul(psumW[:],
                                               lhsT=_chunkap(eg2[:], K1, ch),
                                               rhs=_chunkap(tsef2[:], 2 * K2, ch),
                                               start=(ch == 0), stop=(ch == NCHUNK - 1),
                                               skip_group_check=True)
                ins.then_inc(tsem, 1)     # 6: histogram done
                t.wait_ge(vsem, 21)       # w2sb + r2
                nc.tensor.matmul(psmall[:, SC_WR:SC_WR + 2 * K2], lhsT=prev32[:],
                                 rhs=w2sb[:], start=True, stop=True,
                                 skip_group_check=True)
                nc.tensor.matmul(psmall[:, SC_T1:SC_T1 + 2], lhsT=m1[:], rhs=r2[:],
                                 start=True, stop=True,
                                 skip_group_check=True).then_inc(tsem, 1)  # 7
                t.wait_ge(vsem, 22)       # wrevsb + t1sb
                nc.tensor.matmul(psmall[0:K2, SC_UT:SC_UT + K2],
                                 lhsT=w2sb[:, 0:K2], rhs=wrevsb[:, 0:K2],
                                 start=True, stop=True, skip_group_check=True)
                nc.tensor.matmul(psmall[0:K2, SC_US:SC_US + K2],
                                 lhsT=w2sb[:, K2:2 * K2], rhs=wrevsb[:, K2:2 * K2],
                                 start=True, stop=True, skip_group_check=True)
                nc.tensor.matmul(psmall[0:1, SC_S1T:SC_S1T + 1], lhsT=t1sb[:, 0:1],
                                 rhs=r2[:, 0:1], start=True, stop=True,
                                 skip_group_check=True)
                nc.tensor.matmul(psmall[0:1, SC_S1S:SC_S1S + 1], lhsT=t1sb[:, 1:2],
                                 rhs=r2[:, 1:2], start=True, stop=True,
                                 skip_group_check=True).then_inc(tsem, 1)  # 8
                t.wait_ge(asem, 7)        # tt128/ss128
                nc.tensor.matmul(psmall[0:1, SC_TT:SC_TT + 1], lhsT=ones[:, 0:1],
                                 rhs=tt128[:], start=True, stop=True,
                                 skip_group_check=True)
                nc.tensor.matmul(psmall[0:1, SC_SS:SC_SS + 1], lhsT=ones[:, 0:1],
                                 rhs=ss128[:], start=True, stop=True,
                                 skip_group_check=True)
                t.wait_ge(asem, 10)       # ts128
                nc.tensor.matmul(psmall[0:1, SC_TS:SC_TS + 1], lhsT=ones[:, 0:1],
                                 rhs=ts128[:], start=True, stop=True,
                                 skip_group_check=True)
                nc.tensor.matmul(psmall[0:1, SC_KD:SC_KD + 1], lhsT=ones[0:64, 0:1],
                                 rhs=kdv[:], start=True, stop=True,
                                 skip_group_check=True)
                t.wait_ge(vsem, 20)       # ceb+kdv
                nc.tensor.matmul(psmall[0:1, SC_CE:SC_CE + 1], lhsT=ones[0:64, 0:1],
                                 rhs=ceb[:], start=True, stop=True,
                                 skip_group_check=True)
                t.wait_ge(asem, 11)       # accg/acch
                nc.tensor.matmul(psmall[0:1, SC_SG:SC_SG + 1], lhsT=ones[0:64, 0:1],
                                 rhs=accg[:], start=True, stop=True,
                                 skip_group_check=True)
                nc.tensor.matmul(psmall[0:1, SC_SH:SC_SH + 1], lhsT=ones[0:100, 0:1],
                                 rhs=acch[:], start=True, stop=True,
                                 skip_group_check=True)
                t.wait_ge(vsem, 23)       # s2v
                nc.tensor.matmul(psmall[0:1, SC_S2T:SC_S2T + 1], lhsT=ones[0:K2, 0:1],
                                 rhs=s2v[:, 0:1], start=True, stop=True,
                                 skip_group_check=True)
                nc.tensor.matmul(psmall[0:1, SC_S2S:SC_S2S + 1], lhsT=ones[0:K2, 0:1],
                                 rhs=s2v[:, 1:2], start=True, stop=True,
                                 skip_group_check=True).then_inc(tsem, 1)  # 9

    return nc


_cache = {}


def _get_nc():
    if "nc" not in _cache:
        _cache["nc"] = build()
    return _cache["nc"]


def kernel(logits_student, logits_teacher, target):
    from concourse.bass_utils import run_bass_kernel_spmd

    nc = _get_nc()
    in_map = {
        "logits_student": np.ascontiguousarray(logits_student, dtype=np.float32),
        "logits_teacher": np.ascontiguousarray(logits_teacher, dtype=np.float32),
        "target": np.ascontiguousarray(np.asarray(target).reshape(B, 1).astype(np.int32)),
    }
    core_ids = list(range(8))
    res = run_bass_kernel_spmd(nc, [in_map] * 8, core_ids)
    out = res.results[0]["out"]
    return np.float32(out.reshape(())).reshape(())


# revision 46
# speedup vs baseline: 2.4840x; 1.0042x over previous
# Trainium2 Bass kernel for nn_CKDLoss: KD loss + virtual-outer-product L1/L2
# + Gram-matrix sub-losses, computed entirely on device.
#
# Sharding: total work after algorithmic reduction is tiny and latency-bound;
# cross-core collectives cost more than the whole computation. Every core runs
# the identical full computation on replicated inputs; host takes core 0.
#
# L1 math: with u = log s - log t (normalized softmax cubes flattened to N),
#   sum_{a,b} |t_a t_b - s_a s_b| = 2*(S_tt - S_ss)   (T = S = 320 cancel),
#   S_tt = sum_{pairs: u_a+u_b<0} t_a t_b.
# Key identity: u = (sl - tl)/T + (lse_t - lse_s) per element — no exp/log of
# cube values on the u path; five tensor_scalar ops on the raw logits plus
# the row-lse bias build u64 exactly.
# Bucketize c = floor((u+UMAX)*K/(2 UMAX)) in [0,K), c = 8*hi + lo.
# The positive-pair test c_a+c_b <= K-2 splits exactly into
#   (hi_a+hi_b <= K1-2)  OR  (hi_a+hi_b = K1-1 AND lo_a+lo_b <= K2-2).
# Build W[hi, lo] = sum_n t_n 1[hi_n=hi] 1[lo_n=lo] (PSUM-accumulated one-hot
# matmuls, bf16). Reversed rows Wrev = P_antidiag @ W via one permutation
# matmul. Then with r[hi] = sum_lo W[hi, lo]:
#   S1 = r^T M1 r                 (M1[a,b] = 1[a+b<=K1-2])
#   U  = W^T Wrev  (8x8),  S2 = sum m1lo * U   (m1lo[a,b] = 1[a+b<=K2-2])
#   S_tt = S1 + S2.
# bf16 in the heavy path is safe: measured end-to-end shift vs f32 is ~1e-5
# of the loss (the bucketization itself is 2.3e-3).
#
# Layout: [64, 512] flat tensors (500 data + 12 zero pad) fold to [128, 256]
# via two permutation matmuls (split at flat col 256). f32r at 256 output
# columns runs 1 cycle/row, so the u-fold costs ~0.5us. Pad elements carry
# zero weight everywhere, so they never contribute. One-hots are built in
# [128, K, chunk] layout (bucket index as MIDDLE dim) so all build operands
# have packed 2-byte last dims -> DVE 2x mode. Pool (gpsimd) cannot run
# is_equal/shift through the walrus codegen, so DVE builds the one-hots and
# Pool applies the t/s weights. A PSUM bank is read by at most one engine
# per phase (HW forbids concurrent multi-engine reads of a bank).

import numpy as np
from contextlib import ExitStack

B, C, NT = 64, 100, 5            # batch, classes, temps 1..5
N = B * C * NT                   # 32000 flattened cube elements
K1, K2 = 32, 8                   # two-level bucket split, K = 256
K = K1 * K2
UMAX = 16.0
INVW = K / (2.0 * UMAX)          # 8.0
ALPHA = 0.7
FLAT = 512                       # padded flat width (500 data + 12 pad)
HC = FLAT // 2                   # 256 folded columns
NCHUNK = HC                      # 256 PE chunks of 128 elements
NG = 5
GS = [72, 72, 64, 40, 8]        # descending group sizes (small tail group)
GO = [0, 72, 144, 208, 248]


def _mkap(tensor_ap, dims, extra_off=0):
    import concourse.bass as bass
    return bass.AP(tensor=tensor_ap.tensor, offset=tensor_ap.offset + extra_off,
                   ap=[list(d) for d in dims])


def _ap3(ap, bcast_inner=None, bcast_mid=None):
    """Append/insert stride-0 dims on an AP: [P,F] -> [P,F,bi] or [P,bm,F]."""
    dims = [list(d) for d in ap.ap]
    if bcast_inner is not None:
        dims = dims + [[0, bcast_inner]]
    if bcast_mid is not None:
        dims = [dims[0], [0, bcast_mid]] + dims[1:]
    return _mkap(ap, dims)


def _gslice(t_ap, ncols, gi, colhalf=0):
    """[128, ncols, NCHUNK] tensor -> [128, ncols, GS[gi]] AP for group gi."""
    dims = [list(t_ap.ap[0]), [NCHUNK, ncols], [1, GS[gi]]]
    return _mkap(t_ap, dims, extra_off=colhalf * K2 * NCHUNK + GO[gi])


def _chunkap(t_ap, ncols, ch):
    """[128, ncols, NCHUNK] tensor -> [128, ncols] AP for chunk ch."""
    dims = [list(t_ap.ap[0]), [NCHUNK, ncols]]
    return _mkap(t_ap, dims, extra_off=ch)


def build():
    import concourse.bass as bass
    from concourse import mybir

    dt = mybir.dt
    AL = mybir.AluOpType
    AF = mybir.ActivationFunctionType
    AX = mybir.AxisListType
    bf = dt.bfloat16
    fr = dt.float32r

    nc = bass.Bass()
    ls_d = nc.declare_dram_parameter("logits_student", [B, C], dt.float32, isOutput=False)
    lt_d = nc.declare_dram_parameter("logits_teacher", [B, C], dt.float32, isOutput=False)
    tg_d = nc.declare_dram_parameter("target", [B, 1], dt.int32, isOutput=False)
    out_d = nc.declare_dram_parameter("out", [1, 1], dt.float32, isOutput=True)

    ctx = ExitStack()
    _n = [0]

    def sb(shape, d=dt.float32):
        _n[0] += 1
        return ctx.enter_context(nc.sbuf_tensor(f"sb{_n[0]}", shape, d))

    def ps(shape, d=dt.float32):
        _n[0] += 1
        return ctx.enter_context(nc.psum_tensor(f"ps{_n[0]}", shape, d))

    with ctx:
        # ---- constants ----
        ones = sb([128, 1])
        ones_b = sb([128, 1], bf)
        iota100 = sb([64, C])
        onesq = sb([32, 32])
        m1 = sb([32, 32])                 # 1[a+b<=K1-2]
        m1lo = sb([K2, K2])               # 1[a+b<=K2-2]
        prev32 = sb([32, 32])             # antidiagonal permutation 1[a+b=31]
        id64b = sb([64, 64], bf)
        e1b = sb([64, 128], bf)           # fold identity half 1: 1[f==p]
        e2b = sb([64, 128], bf)           # fold identity half 2: 1[f==p+64]
        wrow64 = sb([64, NT])             # KD weights -a*T^2/(B*C)
        irep32 = sb([128, K1, 72], bf)    # irep32[p, j, m] = j
        irep8 = sb([128, K2, 72], bf)
        # ---- inputs ----
        sl_ = sb([64, C])
        tl_ = sb([64, C])
        tg = sb([64, 1], dt.int32)
        # ---- softmax / u stage ----
        sls = sb([64, NT * C])            # student logits / T
        tls = sb([64, NT * C])            # teacher logits / T
        d0 = sb([64, C])                  # sl - tl
        u64 = sb([64, FLAT], bf)          # (sl-tl)/T + zd, padded
        es = sb([64, NT * C], bf)         # exp(student)
        et = sb([64, NT * C], bf)
        se_s = sb([64, NT], bf)
        se_t = sb([64, NT], bf)
        rs_s = sb([64, NT])
        rs_t = sb([64, NT])
        rs_sn = sb([64, NT])
        lse_s = sb([64, NT])
        lse_t = sb([64, NT])
        zd = sb([64, NT])
        cube_tb = sb([64, FLAT], bf)      # normalized teacher cube (padded)
        cube_sb = sb([64, FLAT], bf)
        cube_snb = sb([64, FLAT], bf)
        # ---- folded [128, 256] ----
        t128b = sb([128, HC], bf)
        s128b = sb([128, HC], bf)
        cf = sb([128, HC])
        ci = sb([128, HC], dt.int32)
        hi_i = sb([128, HC], dt.int32)
        lo_i = sb([128, HC], dt.int32)
        hi_b = sb([128, HC], bf)
        lo_b = sb([128, HC], bf)
        # ---- histogram build ----
        eg2 = sb([128, K1, NCHUNK], bf)
        lm2 = sb([128, K2, NCHUNK], bf)
        tsef2 = sb([128, 2 * K2, NCHUNK], bf)
        # ---- grams ----
        trT = sb([100, NT, 64], bf)
        trS = sb([100, NT, 64], bf)
        trSn = sb([100, NT, 64], bf)
        sqg = sb([64, NT * 64], bf)
        sqh = sb([100, NT * C], bf)
        accg = sb([64, 1])
        acch = sb([100, 1])
        # ---- L2 / KD / CE ----
        sq_t = sb([128, HC], bf)
        sq_s = sb([128, HC], bf)
        tt128 = sb([128, 1])
        ss128 = sb([128, 1])
        tsprod = sb([128, HC], bf)
        ts128 = sb([128, 1])
        d0b = sb([64, C], bf)
        cw = sb([64, NT * C], bf)         # cube_tb * (w_T/T)
        cwp = sb([64, NT * C], bf)        # cw * d0
        kscr = sb([64, NT * C], bf)       # ACT accum scratch
        cscr = sb([64, C], bf)            # CE accum scratch
        zscr = sb([64, NT], bf)
        tscr = sb([128, HC], bf)
        k1act = sb([64, 1])
        kz = sb([64, NT])
        kdzd = sb([64, 1])
        kdv = sb([64, 1])
        tgf = sb([64, 1])
        oh = sb([64, C])
        ohs = sb([64, C])
        cep = sb([64, 1])
        ceb = sb([64, 1])
        # ---- contraction ----
        w2sb = sb([32, 2 * K2])
        wrevsb = sb([32, 2 * K2])
        r2 = sb([32, 2])
        t1sb = sb([32, 2])
        p8 = sb([K2, 2 * K2])
        s2v = sb([K2, 2])
        # ---- final ----
        sbs = sb([1, 16])
        fs = sb([1, 12])
        warm = sb([1, 1])
        # ---- PSUM: 8 tensors = 8 banks ----
        psumF = ps([128, HC])             # u-fold, then teacher fold
        psumFs = ps([128, HC])            # student fold
        ptrT = ps([100, NT, 64], bf)
        ptrS = ps([100, NT, 64], bf)
        psum_gd = ps([64, NT * 64])
        psum_hd = ps([100, NT * C])
        psumW = ps([K1, 2 * K2])
        psmall = ps([32, 64])

        SC_TT, SC_SS, SC_TS, SC_KD, SC_CE, SC_SG, SC_SH = 0, 1, 2, 3, 4, 5, 6
        SC_S1T, SC_S1S, SC_S2T, SC_S2S = 7, 8, 9, 10
        SC_UT, SC_US = 16, 24
        SC_T1 = 32
        SC_WR = 40

        wv = [-ALPHA * T * T / (B * C) for T in range(1, NT + 1)]

        with (
            nc.semaphore("d_in") as d_in,
            nc.semaphore("d_tl") as d_tl,
            nc.semaphore("d_tg") as d_tg,
            nc.semaphore("d_out") as d_out,
            nc.semaphore("vsem") as vsem,
            nc.semaphore("psem") as psem,
            nc.semaphore("asem") as asem,
            nc.semaphore("tsem") as tsem,
            nc.Block() as block,
        ):
            # ---------------- SP: DMA only ----------------
            @block.sync
            def _(s):
                s.dma_start(out=sl_[:], in_=ls_d[:, :]).then_inc(d_in, 16)
                s.dma_start(out=tl_[:], in_=lt_d[:, :]).then_inc(d_tl, 16)
                s.dma_start(out=tg[:], in_=tg_d[:, :]).then_inc(d_tg, 16)
                s.wait_ge(vsem, 24)       # final scalar ready
                s.dma_start(out=out_d[:, :], in_=fs[:, 0:1]).then_inc(d_out, 16)
                s.wait_ge(d_out, 16)

            # ---------------- Pool ----------------
            # psem: 1=ones 2=quick constants 3=tls 4=ireps 5=cube_snb
            #       6=cube_tb 7=tsprod 8=hi_b 9..13=tsef2 groups
            @block.gpsimd
            def _(g):
                g.memset(ones[:], 1.0).then_inc(psem, 1)   # 1: ACT warmup gate
                g.memset(ones_b[:], 1.0)
                g.memset(onesq[:], 1.0)
                g.memset(id64b[:], 0.0)
                g.memset(prev32[:], 0.0)
                g.memset(e1b[:], 0.0)
                g.memset(e2b[:], 0.0)
                g.iota(iota100[:], [[1, C]], channel_multiplier=0,
                       allow_small_or_imprecise_dtypes=True)
                for T in range(1, NT + 1):
                    g.memset(wrow64[:, T - 1:T], wv[T - 1])
                # zero pads (data cols are written later; ranges are disjoint)
                g.memset(u64[:, NT * C:FLAT], 0.0)
                g.memset(cube_tb[:, NT * C:FLAT], 0.0)
                g.memset(cube_sb[:, NT * C:FLAT], 0.0)
                g.memset(cube_snb[:, NT * C:FLAT], 0.0)
                g.drain()
                g.affine_select(m1[:], onesq[:], [[-1, 32]], AL.is_ge, 0.0,
                                base=K1 - 2, channel_multiplier=-1)
                g.affine_select(m1lo[:], onesq[0:K2, 0:K2], [[-1, K2]], AL.is_ge,
                                0.0, base=K2 - 2, channel_multiplier=-1)
                g.affine_select(id64b[:], id64b[:], [[-1, 64]], AL.not_equal,
                                1.0, base=0, channel_multiplier=1)
                g.affine_select(prev32[:], prev32[:], [[-1, 32]], AL.not_equal,
                                1.0, base=K1 - 1, channel_multiplier=-1)
                g.affine_select(e1b[:], e1b[:], [[-1, 128]], AL.not_equal,
                                1.0, base=0, channel_multiplier=1)
                g.affine_select(e2b[:], e2b[:], [[-1, 128]], AL.not_equal,
                                1.0, base=64, channel_multiplier=1).then_inc(psem, 1)   # 2
                # teacher prescale (ahead of the slow irep iotas)
                g.wait_ge(d_tl, 16)
                ins = None
                for T in range(1, NT + 1):
                    i = T - 1
                    ins = nc.gpsimd.tensor_scalar_mul(
                        tls[:, i * C:(i + 1) * C], tl_[:], 1.0 / T)
                ins.then_inc(psem, 1)     # 3: tls
                g.iota(irep32[:], [[1, K1], [0, 72]], channel_multiplier=0,
                       allow_small_or_imprecise_dtypes=True)
                g.iota(irep8[:], [[1, K2], [0, 72]], channel_multiplier=0,
                       allow_small_or_imprecise_dtypes=True).then_inc(psem, 1)  # 4
                g.wait_ge(vsem, 4)        # rs_sn
                ins = None
                for T in range(1, NT + 1):
                    i = T - 1
                    slc = slice(i * C, (i + 1) * C)
                    ins = nc.gpsimd.tensor_scalar_mul(cube_snb[:, slc], es[:, slc],
                                                      rs_sn[:, i:i + 1])
                ins.then_inc(psem, 1)     # 5: cube_snb
                g.wait_ge(vsem, 5)        # rs_t
                ins = None
                for T in range(1, NT + 1):
                    i = T - 1
                    slc = slice(i * C, (i + 1) * C)
                    ins = nc.gpsimd.tensor_scalar_mul(cube_tb[:, slc], et[:, slc],
                                                      rs_t[:, i:i + 1])
                ins.then_inc(psem, 1)     # 6: cube_tb
                g.wait_ge(vsem, 9)        # hi_i (DVE shift)
                nc.gpsimd.tensor_copy(out=hi_b[:], in_=hi_i[:]).then_inc(psem, 1)  # 7
                g.wait_ge(asem, 6)        # t128b + s128b (ACT copies)
                nc.gpsimd.tensor_tensor(out=tsprod[:], in0=t128b[:], in1=s128b[:],
                                        op=AL.mult).then_inc(psem, 1)  # 8: tsprod
                # KD product pieces (consumed by one ACT accum op)
                nc.gpsimd.tensor_copy(out=d0b[:], in_=d0[:])
                ins = None
                for T in range(1, NT + 1):
                    i = T - 1
                    ins = nc.gpsimd.tensor_scalar_mul(
                        cw[:, i * C:(i + 1) * C], cube_tb[:, i * C:(i + 1) * C],
                        wv[i] / T)
                g.drain()
                nc.gpsimd.tensor_tensor(out=cwp[:], in0=cw[:],
                                        in1=_ap3(d0b[:], bcast_mid=NT),
                                        op=AL.mult).then_inc(psem, 1)  # 9: cwp
                for gi in range(NG):
                    cs = slice(GO[gi], GO[gi] + GS[gi])
                    g.wait_ge(vsem, 10 + 2 * gi)   # lm2 group built
                    nc.gpsimd.tensor_tensor(
                        out=_gslice(tsef2[:], K2, gi),
                        in0=_gslice(lm2[:], K2, gi),
                        in1=_ap3(t128b[:, cs], bcast_mid=K2), op=AL.mult)
                    g.drain()
                    nc.gpsimd.tensor_tensor(
                        out=_gslice(tsef2[:], K2, gi, colhalf=1),
                        in0=_gslice(lm2[:], K2, gi),
                        in1=_ap3(s128b[:, cs], bcast_mid=K2),
                        op=AL.mult).then_inc(psem, 1)   # 10..14


            # ---------------- ACT ----------------
            # asem: 1=exp_s 2=exp_t 3=lse_s 4=lse_t 5=s128b 6=t128b
            #       7=L2 squares 8=tr copies 9=gram squares
            @block.scalar
            def _(a):
                a.wait_ge(psem, 1)
                nc.scalar.activation(out=warm[:], in_=ones[0:1, :], func=AF.Exp)
                a.wait_ge(vsem, 1)        # sls
                nc.scalar.activation(out=es[:], in_=sls[:], func=AF.Exp).then_inc(asem, 1)
                a.wait_ge(psem, 3)        # tls
                nc.scalar.activation(out=et[:], in_=tls[:], func=AF.Exp).then_inc(asem, 1)
                a.wait_ge(vsem, 2)        # se_s
                nc.scalar.activation(out=lse_s[:], in_=se_s[:],
                                     func=AF.Ln).then_inc(asem, 1)
                a.wait_ge(vsem, 3)        # se_t (before recips)
                nc.scalar.activation(out=lse_t[:], in_=se_t[:],
                                     func=AF.Ln).then_inc(asem, 1)
                a.wait_ge(tsem, 2)        # fold S in PSUM
                nc.scalar.activation(out=s128b[:], in_=psumFs[:],
                                     func=AF.Identity).then_inc(asem, 1)  # 5
                a.wait_ge(tsem, 3)        # fold T in PSUM
                nc.scalar.activation(out=t128b[:], in_=psumF[:],
                                     func=AF.Identity).then_inc(asem, 1)  # 6
                nc.scalar.activation(out=sq_t[:], in_=psumF[:], func=AF.Square,
                                     accum_out=tt128[:])
                nc.scalar.activation(out=sq_s[:], in_=psumFs[:], func=AF.Square,
                                     accum_out=ss128[:]).then_inc(asem, 1)  # 7
                a.wait_ge(tsem, 4)        # transposes done
                nc.scalar.activation(out=trT[:], in_=ptrT[:], func=AF.Identity)
                nc.scalar.activation(out=trS[:], in_=ptrS[:], func=AF.Identity)
                nc.scalar.activation(out=trSn[:], in_=ptrS[:], func=AF.Identity,
                                     scale=-1.0).then_inc(asem, 1)   # 8
                a.wait_ge(psem, 9)        # cwp
                nc.scalar.activation(out=kscr[:], in_=cwp[:], func=AF.Identity,
                                     accum_out=k1act[:]).then_inc(asem, 1)  # 9
                a.wait_ge(vsem, 8)        # ohs + kz written (cf implies both)
                a.wait_ge(psem, 8)        # tsprod
                nc.scalar.activation(out=cscr[:], in_=ohs[:], func=AF.Identity,
                                     accum_out=cep[:])
                nc.scalar.activation(out=zscr[:], in_=kz[:], func=AF.Identity,
                                     accum_out=kdzd[:])
                nc.scalar.activation(out=tscr[:], in_=tsprod[:], func=AF.Identity,
                                     accum_out=ts128[:]).then_inc(asem, 1)  # 10
                a.wait_ge(tsem, 5)        # gram mms done
                nc.scalar.activation(out=sqg[:], in_=psum_gd[:], func=AF.Square,
                                     accum_out=accg[:])
                nc.scalar.activation(out=sqh[:], in_=psum_hd[:], func=AF.Square,
                                     accum_out=acch[:]).then_inc(asem, 1)  # 11

            # ---------------- DVE ----------------
            # vsem: 1=sls 2=se_s 3=rs_sn 4=se_t+recip_t 5=u64 6=cube_sb 7=ceb
            #       8=cf 9=hi_i  10,12,14,16,18=lm2  11,13,15,17,19=eg2
            #       20=kdv+ts128 21=w2sb+r2 22=wrevsb+t1sb 23=s2v 24=final
            @block.vector
            def _(v):
                v.wait_ge(d_in, 16)
                ins = None
                for T in range(1, NT + 1):
                    i = T - 1
                    ins = nc.vector.tensor_scalar_mul(
                        sls[:, i * C:(i + 1) * C], sl_[:], 1.0 / T)
                ins.then_inc(vsem, 1)
                v.wait_ge(d_tl, 16)
                nc.vector.tensor_sub(out=d0[:], in0=sl_[:], in1=tl_[:])
                v.wait_ge(asem, 1)        # exp_s
                v.drain()
                with nc.allow_low_precision(reason="se sums tolerate bf16"):
                    nc.vector.tensor_reduce(out=se_s[:], in_=es[:].rearrange(
                        "p (t c) -> p t c", t=NT), axis=AX.X, op=AL.add).then_inc(vsem, 1)
                v.wait_ge(asem, 2)        # exp_t
                v.drain()
                with nc.allow_low_precision(reason="se sums tolerate bf16"):
                    nc.vector.tensor_reduce(out=se_t[:], in_=et[:].rearrange(
                        "p (t c) -> p t c", t=NT), axis=AX.X, op=AL.add).then_inc(vsem, 1)  # 3
                v.drain()
                nc.vector.reciprocal(out=rs_s[:], in_=se_s[:])
                v.drain()
                nc.vector.tensor_scalar_mul(rs_sn[:], rs_s[:], -1.0).then_inc(vsem, 1)  # 4
                v.drain()
                nc.vector.reciprocal(out=rs_t[:], in_=se_t[:]).then_inc(vsem, 1)  # 5
                v.wait_ge(asem, 4)        # lse_t (and lse_s)
                nc.vector.tensor_sub(out=zd[:], in0=lse_t[:], in1=lse_s[:])
                v.drain()
                ins = None
                for T in range(1, NT + 1):
                    i = T - 1
                    ins = nc.vector.tensor_scalar(
                        u64[:, i * C:(i + 1) * C], d0[:], 1.0 / T, zd[:, i:i + 1],
                        AL.mult, AL.add)
                ins.then_inc(vsem, 1)     # 6: u64
                nc.vector.tensor_tensor(out=kz[:], in0=zd[:], in1=wrow64[:],
                                        op=AL.mult)
                ins = None
                for T in range(1, NT + 1):
                    i = T - 1
                    slc = slice(i * C, (i + 1) * C)
                    ins = nc.vector.tensor_scalar_mul(cube_sb[:, slc], es[:, slc],
                                                      rs_s[:, i:i + 1])
                ins.then_inc(vsem, 1)     # 7: cube_sb
                # CE one-hot products fill the wait for the u-fold
                v.wait_ge(d_tg, 16)
                nc.vector.tensor_copy(out=tgf[:], in_=tg[:])
                v.drain()
                nc.vector.tensor_tensor(out=oh[:],
                                        in0=_ap3(tgf[:], bcast_inner=C)[:, 0, :],
                                        in1=iota100[:], op=AL.is_equal)
                v.drain()
                nc.vector.tensor_tensor(out=ohs[:], in0=oh[:], in1=sl_[:], op=AL.mult)
                # bucket chain straight from the u-fold PSUM (sole reader here)
                v.wait_ge(tsem, 1)        # u-fold done
                nc.vector.tensor_scalar(cf[:], psumF[:], INVW, K / 2.0 - 0.5,
                                        AL.mult, AL.add).then_inc(vsem, 1)  # 8
                v.drain()
                nc.vector.tensor_copy(out=ci[:], in_=cf[:])
                v.drain()
                nc.vector.tensor_scalar(hi_i[:], ci[:], 3, None,
                                        AL.arith_shift_right).then_inc(vsem, 1)  # 9
                nc.vector.tensor_scalar(lo_i[:], ci[:], 7, None, AL.bitwise_and)
                v.drain()
                nc.vector.tensor_copy(out=lo_b[:], in_=lo_i[:])
                v.drain()
                v.wait_ge(psem, 4)        # ireps
                for gi in range(NG):
                    cs = slice(GO[gi], GO[gi] + GS[gi])
                    nc.vector.tensor_tensor(
                        out=_gslice(lm2[:], K2, gi),
                        in0=_ap3(lo_b[:, cs], bcast_mid=K2),
                        in1=_mkap(irep8[:], [list(irep8[:].ap[0]),
                                             [72, K2], [1, GS[gi]]]),
                        op=AL.is_equal).then_inc(vsem, 1)   # 9,11,... (lm2)
                    if gi == 0:
                        v.wait_ge(psem, 7)    # hi_b (Pool, also implies ireps)
                    v.drain()
                    nc.vector.tensor_tensor(
                        out=_gslice(eg2[:], K1, gi),
                        in0=_ap3(hi_b[:, cs], bcast_mid=K1),
                        in1=_mkap(irep32[:], [list(irep32[:].ap[0]),
                                              [72, K1], [1, GS[gi]]]),
                        op=AL.is_equal).then_inc(vsem, 1)   # 10,12,... (eg2)
                # CE + KD finals (reduces went to ACT accum)
                v.wait_ge(asem, 10)       # cep/kdzd/ts accums
                nc.vector.tensor_sub(out=ceb[:], in0=lse_s[:, 0:1], in1=cep[:])
                v.drain()
                nc.vector.tensor_add(out=kdv[:], in0=k1act[:],
                                     in1=kdzd[:]).then_inc(vsem, 1)  # 20 (ceb+kdv)
                v.wait_ge(tsem, 6)        # histogram done
                nc.vector.tensor_reduce(out=r2[:, 0:1], in_=psumW[:, 0:K2],
                                        axis=AX.X, op=AL.add)
                nc.vector.tensor_reduce(out=r2[:, 1:2], in_=psumW[:, K2:2 * K2],
                                        axis=AX.X, op=AL.add)
                nc.vector.tensor_copy(out=w2sb[:], in_=psumW[:]).then_inc(vsem, 1)  # 21
                v.wait_ge(tsem, 7)        # wrev/t1 mms done
                nc.vector.tensor_copy(out=wrevsb[:], in_=psmall[:, SC_WR:SC_WR + 2 * K2])
                nc.vector.tensor_copy(out=t1sb[:], in_=psmall[:, SC_T1:SC_T1 + 2]).then_inc(vsem, 1)  # 22
                v.wait_ge(tsem, 8)        # U/S1 mms done
                nc.vector.tensor_tensor(out=p8[:, 0:K2], in0=m1lo[:],
                                        in1=psmall[0:K2, SC_UT:SC_UT + K2], op=AL.mult)
                nc.vector.tensor_tensor(out=p8[:, K2:2 * K2], in0=m1lo[:],
                                        in1=psmall[0:K2, SC_US:SC_US + K2], op=AL.mult)
                v.drain()
                nc.vector.tensor_reduce(out=s2v[:, 0:1], in_=p8[:, 0:K2],
                                        axis=AX.X, op=AL.add)
                nc.vector.tensor_reduce(out=s2v[:, 1:2], in_=p8[:, K2:2 * K2],
                                        axis=AX.X, op=AL.add).then_inc(vsem, 1)  # 23
                v.wait_ge(tsem, 10)       # all scalar mms done
                nc.vector.tensor_copy(out=sbs[:, 0:11], in_=psmall[0:1, 0:11])
                tt_, ss_, ts_ = sbs[:, 0:1], sbs[:, 1:2], sbs[:, 2:3]
                kd_, ce_ = sbs[:, 3:4], sbs[:, 4:5]
                sg_, sh_ = sbs[:, 5:6], sbs[:, 6:7]
                s1t, s1s, s2t, s2s = (sbs[:, i:i + 1] for i in range(7, 11))
                v.drain()
                nc.vector.tensor_add(out=fs[:, 0:1], in0=s1t, in1=s2t)
                nc.vector.tensor_add(out=fs[:, 1:2], in0=s1s, in1=s2s)
                nc.vector.tensor_mul(out=fs[:, 2:3], in0=tt_, in1=tt_)
                nc.vector.tensor_mul(out=fs[:, 3:4], in0=ss_, in1=ss_)
                nc.vector.tensor_mul(out=fs[:, 4:5], in0=ts_, in1=ts_)
                nc.vector.tensor_add(out=fs[:, 5:6], in0=sg_, in1=sh_)
                v.drain()
                nc.vector.tensor_sub(out=fs[:, 6:7], in0=fs[:, 0:1], in1=fs[:, 1:2])
                nc.vector.tensor_add(out=fs[:, 7:8], in0=fs[:, 2:3], in1=fs[:, 3:4])
                nc.vector.tensor_add(out=fs[:, 8:9], in0=fs[:, 5:6], in1=kd_)
                v.drain()
                nc.vector.scalar_tensor_tensor(out=fs[:, 9:10], in0=fs[:, 4:5],
                                               scalar=-2.0, in1=fs[:, 7:8],
                                               op0=AL.mult, op1=AL.add)
                nc.vector.scalar_tensor_tensor(out=fs[:, 10:11], in0=ce_,
                                               scalar=NT * (1.0 - ALPHA) / B,
                                               in1=fs[:, 8:9],
                                               op0=AL.mult, op1=AL.add)
                v.drain()
                nc.vector.scalar_tensor_tensor(out=fs[:, 11:12], in0=fs[:, 6:7],
                                               scalar=2.0, in1=fs[:, 9:10],
                                               op0=AL.mult, op1=AL.add)
                v.drain()
                nc.vector.scalar_tensor_tensor(out=fs[:, 0:1], in0=fs[:, 11:12],
                                               scalar=0.00025, in1=fs[:, 10:11],
                                               op0=AL.mult, op1=AL.add).then_inc(vsem, 1)  # 24

            # ---------------- PE ----------------
            # tsem: 1=u-fold 2=foldS 3=foldT 4=transposes 5=grams 6=hist
            #       7=wrev+t1 8=U+S1 9=S2+scalars
            @block.tensor
            def _(t):
                t.wait_ge(psem, 2)        # identities
                t.wait_ge(vsem, 6)        # u64
                nc.tensor.matmul(psumF[:], lhsT=e1b[:], rhs=u64[:, 0:HC],
                                 start=True, stop=False)
                nc.tensor.matmul(psumF[:], lhsT=e2b[:], rhs=u64[:, HC:FLAT],
                                 start=False, stop=True).then_inc(tsem, 1)
                t.wait_ge(vsem, 7)        # cube_sb
                nc.tensor.matmul(psumFs[:], lhsT=e1b[:], rhs=cube_sb[:, 0:HC],
                                 start=True, stop=False, skip_group_check=True)
                nc.tensor.matmul(psumFs[:], lhsT=e2b[:], rhs=cube_sb[:, HC:FLAT],
                                 start=False, stop=True,
                                 skip_group_check=True).then_inc(tsem, 1)  # 2
                t.wait_ge(psem, 6)        # cube_tb
                t.wait_ge(vsem, 8)        # cf has finished reading psumF
                nc.tensor.matmul(psumF[:], lhsT=e1b[:], rhs=cube_tb[:, 0:HC],
                                 start=True, stop=False, skip_group_check=True)
                nc.tensor.matmul(psumF[:], lhsT=e2b[:], rhs=cube_tb[:, HC:FLAT],
                                 start=False, stop=True,
                                 skip_group_check=True).then_inc(tsem, 1)  # 3
                ins = None
                for k in range(NT):
                    slc = slice(k * C, (k + 1) * C)
                    nc.tensor.transpose(out=ptrT[:, k, :], in_=cube_tb[:, slc],
                                        identity=id64b[:])
                    ins = nc.tensor.transpose(out=ptrS[:, k, :], in_=cube_sb[:, slc],
                                              identity=id64b[:])
                ins.then_inc(tsem, 1)     # 4
                t.wait_ge(asem, 8)        # trT/trS/trSn
                t.wait_ge(psem, 5)        # cube_snb
                ins = None
                for k in range(NT):
                    slc = slice(k * C, (k + 1) * C)
                    nc.tensor.matmul(psum_hd[:, slc], lhsT=cube_tb[:, slc],
                                     rhs=cube_tb[:, slc], start=True, stop=False,
                                     skip_group_check=True)
                    nc.tensor.matmul(psum_hd[:, slc], lhsT=cube_snb[:, slc],
                                     rhs=cube_sb[:, slc], start=False, stop=True,
                                     skip_group_check=True)
                    nc.tensor.matmul(psum_gd[:, k * 64:(k + 1) * 64],
                                     lhsT=trT[:, k, :], rhs=trT[:, k, :],
                                     start=True, stop=False, skip_group_check=True)
                    ins = nc.tensor.matmul(psum_gd[:, k * 64:(k + 1) * 64],
                                           lhsT=trSn[:, k, :], rhs=trS[:, k, :],
                                           start=False, stop=True,
                                           skip_group_check=True)
                ins.then_inc(tsem, 1)     # 5
                ins = None
                for gi in range(NG):
                    t.wait_ge(vsem, 11 + 2 * gi)
                    t.wait_ge(psem, 10 + gi)
                    for i in range(GS[gi]):
                        ch = GO[gi] + i
                        ins = nc.tensor.matmul(psumW[:],
                                               lhsT=_chunkap(eg2[:], K1, ch),
                                               rhs=_chunkap(tsef2[:], 2 * K2, ch),
                                               start=(ch == 0), stop=(ch == NCHUNK - 1),
                                               skip_group_check=True)
                ins.then_inc(tsem, 1)     # 6: histogram done
                t.wait_ge(vsem, 21)       # w2sb + r2
                nc.tensor.matmul(psmall[:, SC_WR:SC_WR + 2 * K2], lhsT=prev32[:],
                                 rhs=w2sb[:], start=True, stop=True,
                                 skip_group_check=True)
                nc.tensor.matmul(psmall[:, SC_T1:SC_T1 + 2], lhsT=m1[:], rhs=r2[:],
                                 start=True, stop=True,
                                 skip_group_check=True).then_inc(tsem, 1)  # 7
                t.wait_ge(vsem, 22)       # wrevsb + t1sb
                nc.tensor.matmul(psmall[0:K2, SC_UT:SC_UT + K2],
                                 lhsT=w2sb[:, 0:K2], rhs=wrevsb[:, 0:K2],
                                 start=True, stop=True, skip_group_check=True)
                nc.tensor.matmul(psmall[0:K2, SC_US:SC_US + K2],
                                 lhsT=w2sb[:, K2:2 * K2], rhs=wrevsb[:, K2:2 * K2],
                                 start=True, stop=True, skip_group_check=True)
                nc.tensor.matmul(psmall[0:1, SC_S1T:SC_S1T + 1], lhsT=t1sb[:, 0:1],
                                 rhs=r2[:, 0:1], start=True, stop=True,
                                 skip_group_check=True)
                nc.tensor.matmul(psmall[0:1, SC_S1S:SC_S1S + 1], lhsT=t1sb[:, 1:2],
                                 rhs=r2[:, 1:2], start=True, stop=True,
                                 skip_group_check=True).then_inc(tsem, 1)  # 8
                t.wait_ge(asem, 7)        # tt128/ss128
                nc.tensor.matmul(psmall[0:1, SC_TT:SC_TT + 1], lhsT=ones[:, 0:1],
                                 rhs=tt128[:], start=True, stop=True,
                                 skip_group_check=True)
                nc.tensor.matmul(psmall[0:1, SC_SS:SC_SS + 1], lhsT=ones[:, 0:1],
                                 rhs=ss128[:], start=True, stop=True,
                                 skip_group_check=True)
                t.wait_ge(asem, 10)       # ts128
                nc.tensor.matmul(psmall[0:1, SC_TS:SC_TS + 1], lhsT=ones[:, 0:1],
                                 rhs=ts128[:], start=True, stop=True,
                                 skip_group_check=True)
                nc.tensor.matmul(psmall[0:1, SC_KD:SC_KD + 1], lhsT=ones[0:64, 0:1],
                                 rhs=kdv[:], start=True, stop=True,
                                 skip_group_check=True)
                t.wait_ge(vsem, 20)       # ceb+kdv
                nc.tensor.matmul(psmall[0:1, SC_CE:SC_CE + 1], lhsT=ones[0:64, 0:1],
                                 rhs=ceb[:], start=True, stop=True,
                                 skip_group_check=True)
                t.wait_ge(asem, 11)       # accg/acch
                nc.tensor.matmul(psmall[0:1, SC_SG:SC_SG + 1], lhsT=ones[0:64, 0:1],
                                 rhs=accg[:], start=True, stop=True,
                                 skip_group_check=True)
                nc.tensor.matmul(psmall[0:1, SC_SH:SC_SH + 1], lhsT=ones[0:100, 0:1],
                                 rhs=acch[:], start=True, stop=True,
                                 skip_group_check=True).then_inc(tsem, 1)  # 9: scalars
                t.wait_ge(vsem, 23)       # s2v
                nc.tensor.matmul(psmall[0:1, SC_S2T:SC_S2T + 1], lhsT=ones[0:K2, 0:1],
                                 rhs=s2v[:, 0:1], start=True, stop=True,
                                 skip_group_check=True)
                nc.tensor.matmul(psmall[0:1, SC_S2S:SC_S2S + 1], lhsT=ones[0:K2, 0:1],
                                 rhs=s2v[:, 1:2], start=True, stop=True,
                                 skip_group_check=True).then_inc(tsem, 1)  # 10: S2

    return nc


_cache = {}


def _get_nc():
    if "nc" not in _cache:
        _cache["nc"] = build()
    return _cache["nc"]


def kernel(logits_student, logits_teacher, target):
    from concourse.bass_utils import run_bass_kernel_spmd

    nc = _get_nc()
    in_map = {
        "logits_student": np.ascontiguousarray(logits_student, dtype=np.float32),
        "logits_teacher": np.ascontiguousarray(logits_teacher, dtype=np.float32),
        "target": np.ascontiguousarray(np.asarray(target).reshape(B, 1).astype(np.int32)),
    }
    core_ids = list(range(8))
    res = run_bass_kernel_spmd(nc, [in_map] * 8, core_ids)
    out = res.results[0]["out"]
    return np.float32(out.reshape(())).reshape(())


# revision 54
# speedup vs baseline: 2.5298x; 1.0184x over previous
# Trainium2 Bass kernel for nn_CKDLoss: KD loss + virtual-outer-product L1/L2
# + Gram-matrix sub-losses, computed entirely on device.
#
# Sharding: total work after algorithmic reduction is tiny and latency-bound;
# cross-core collectives cost more than the whole computation. Every core runs
# the identical full computation on replicated inputs; host takes core 0.
#
# L1 math: with u = log s - log t (normalized softmax cubes flattened to N),
#   sum_{a,b} |t_a t_b - s_a s_b| = 2*(S_tt - S_ss)   (T = S = 320 cancel),
#   S_tt = sum_{pairs: u_a+u_b<0} t_a t_b.
# Key identity: u = (sl - tl)/T + (lse_t - lse_s) per element — no exp/log of
# cube values on the u path; five tensor_scalar ops on the raw logits plus
# the row-lse bias build u64 exactly.
# Bucketize c = floor((u+UMAX)*K/(2 UMAX)) in [0,K), c = 8*hi + lo.
# The positive-pair test c_a+c_b <= K-2 splits exactly into
#   (hi_a+hi_b <= K1-2)  OR  (hi_a+hi_b = K1-1 AND lo_a+lo_b <= K2-2).
# Build W[hi, lo] = sum_n t_n 1[hi_n=hi] 1[lo_n=lo] (PSUM-accumulated one-hot
# matmuls, bf16). Reversed rows Wrev = P_antidiag @ W via one permutation
# matmul. Then with r[hi] = sum_lo W[hi, lo]:
#   S1 = r^T M1 r                 (M1[a,b] = 1[a+b<=K1-2])
#   U  = W^T Wrev  (8x8),  S2 = sum m1lo * U   (m1lo[a,b] = 1[a+b<=K2-2])
#   S_tt = S1 + S2.
# bf16 in the heavy path is safe: measured end-to-end shift vs f32 is ~1e-5
# of the loss (the bucketization itself is 2.3e-3).
#
# Layout: [64, 512] flat tensors (500 data + 12 zero pad) fold to [128, 256]
# via two permutation matmuls (split at flat col 256). f32r at 256 output
# columns runs 1 cycle/row, so the u-fold costs ~0.5us. Pad elements carry
# zero weight everywhere, so they never contribute. One-hots are built in
# [128, K, chunk] layout (bucket index as MIDDLE dim) so all build operands
# have packed 2-byte last dims -> DVE 2x mode. Pool (gpsimd) cannot run
# is_equal/shift through the walrus codegen, so DVE builds the one-hots and
# Pool applies the t/s weights. A PSUM bank is read by at most one engine
# per phase (HW forbids concurrent multi-engine reads of a bank).

import numpy as np
from contextlib import ExitStack

B, C, NT = 64, 100, 5            # batch, classes, temps 1..5
N = B * C * NT                   # 32000 flattened cube elements
K1, K2 = 32, 8                   # two-level bucket split, K = 256
K = K1 * K2
UMAX = 16.0
INVW = K / (2.0 * UMAX)          # 8.0
ALPHA = 0.7
FLAT = 512                       # padded flat width (500 data + 12 pad)
HC = FLAT // 2                   # 256 folded columns
NCHUNK = HC                      # 256 PE chunks of 128 elements
NG = 5
GS = [72, 72, 64, 40, 8]        # descending group sizes (small tail group)
GO = [0, 72, 144, 208, 248]


def _mkap(tensor_ap, dims, extra_off=0):
    import concourse.bass as bass
    return bass.AP(tensor=tensor_ap.tensor, offset=tensor_ap.offset + extra_off,
                   ap=[list(d) for d in dims])


def _ap3(ap, bcast_inner=None, bcast_mid=None):
    """Append/insert stride-0 dims on an AP: [P,F] -> [P,F,bi] or [P,bm,F]."""
    dims = [list(d) for d in ap.ap]
    if bcast_inner is not None:
        dims = dims + [[0, bcast_inner]]
    if bcast_mid is not None:
        dims = [dims[0], [0, bcast_mid]] + dims[1:]
    return _mkap(ap, dims)


def _gslice(t_ap, ncols, gi, colhalf=0):
    """[128, ncols, NCHUNK] tensor -> [128, ncols, GS[gi]] AP for group gi."""
    dims = [list(t_ap.ap[0]), [NCHUNK, ncols], [1, GS[gi]]]
    return _mkap(t_ap, dims, extra_off=colhalf * K2 * NCHUNK + GO[gi])


def _chunkap(t_ap, ncols, ch):
    """[128, ncols, NCHUNK] tensor -> [128, ncols] AP for chunk ch."""
    dims = [list(t_ap.ap[0]), [NCHUNK, ncols]]
    return _mkap(t_ap, dims, extra_off=ch)


def build():
    import concourse.bass as bass
    from concourse import mybir

    dt = mybir.dt
    AL = mybir.AluOpType
    AF = mybir.ActivationFunctionType
    AX = mybir.AxisListType
    bf = dt.bfloat16
    fr = dt.float32r

    nc = bass.Bass()
    ls_d = nc.declare_dram_parameter("logits_student", [B, C], dt.float32, isOutput=False)
    lt_d = nc.declare_dram_parameter("logits_teacher", [B, C], dt.float32, isOutput=False)
    tg_d = nc.declare_dram_parameter("target", [B, 1], dt.int32, isOutput=False)
    out_d = nc.declare_dram_parameter("out", [1, 1], dt.float32, isOutput=True)

    ctx = ExitStack()
    _n = [0]

    def sb(shape, d=dt.float32):
        _n[0] += 1
        return ctx.enter_context(nc.sbuf_tensor(f"sb{_n[0]}", shape, d))

    def ps(shape, d=dt.float32):
        _n[0] += 1
        return ctx.enter_context(nc.psum_tensor(f"ps{_n[0]}", shape, d))

    with ctx:
        # ---- constants ----
        ones = sb([128, 1])
        ones_b = sb([128, 1], bf)
        iota100 = sb([64, C])
        onesq = sb([32, 32])
        m1 = sb([32, 32])                 # 1[a+b<=K1-2]
        m1lo = sb([K2, K2])               # 1[a+b<=K2-2]
        prev32 = sb([32, 32])             # antidiagonal permutation 1[a+b=31]
        id64b = sb([64, 64], bf)
        e1b = sb([64, 128], bf)           # fold identity half 1: 1[f==p]
        e2b = sb([64, 128], bf)           # fold identity half 2: 1[f==p+64]
        wrow64 = sb([64, NT])             # KD weights -a*T^2/(B*C)
        irep32 = sb([128, K1, 72], bf)    # irep32[p, j, m] = j
        irep8 = sb([128, K2, 72], bf)
        # ---- inputs ----
        sl_ = sb([64, C])
        tl_ = sb([64, C])
        tg = sb([64, 1], dt.int32)
        # ---- softmax / u stage ----
        sls = sb([64, NT * C])            # student logits / T
        tls = sb([64, NT * C])            # teacher logits / T
        d0 = sb([64, C])                  # sl - tl
        u64 = sb([64, FLAT], bf)          # (sl-tl)/T + zd, padded
        es = sb([64, NT * C], bf)         # exp(student)
        et = sb([64, NT * C], bf)
        se_s = sb([64, NT], bf)
        se_t = sb([64, NT], bf)
        rs_s = sb([64, NT])
        rs_t = sb([64, NT])
        rs_sn = sb([64, NT])
        lse_s = sb([64, NT])
        lse_t = sb([64, NT])
        zd = sb([64, NT])
        zdb = sb([64, NT], bf)
        cube_tb = sb([64, FLAT], bf)      # normalized teacher cube (padded)
        cube_sb = sb([64, FLAT], bf)
        cube_snb = sb([64, FLAT], bf)
        # ---- folded [128, 256] ----
        t128b = sb([128, HC], bf)
        s128b = sb([128, HC], bf)
        cf = sb([128, HC])
        ci = sb([128, HC], dt.int32)
        hi_i = sb([128, HC], dt.int32)
        lo_i = sb([128, HC], dt.int32)
        hi_b = sb([128, HC], bf)
        lo_b = sb([128, HC], bf)
        # ---- histogram build ----
        eg2 = sb([128, K1, NCHUNK], bf)
        lm2 = sb([128, K2, NCHUNK], bf)
        tsef2 = sb([128, 2 * K2, NCHUNK], bf)
        # ---- grams ----
        trT = sb([100, NT, 64], bf)
        trS = sb([100, NT, 64], bf)
        trSn = sb([100, NT, 64], bf)
        sqg = sb([64, NT * 64], bf)
        sqh = sb([100, NT * C], bf)
        accg = sb([64, 1])
        acch = sb([100, 1])
        # ---- L2 / KD / CE ----
        sq_t = sb([128, HC], bf)
        sq_s = sb([128, HC], bf)
        tt128 = sb([128, 1])
        ss128 = sb([128, 1])
        tsprod = sb([128, HC], bf)
        ts128 = sb([128, 1])
        d0b = sb([64, C], bf)
        cw = sb([64, NT * C], bf)         # cube_tb * (w_T/T)
        cwp = sb([64, NT * C], bf)        # cw * d0
        kscr = sb([64, NT * C], bf)       # ACT accum scratch
        cscr = sb([64, C], bf)            # CE accum scratch
        zscr = sb([64, NT], bf)
        tscr = sb([128, HC], bf)
        k1act = sb([64, 1])
        kz = sb([64, NT])
        kdzd = sb([64, 1])
        kdv = sb([64, 1])
        tgf = sb([64, 1])
        oh = sb([64, C])
        ohs = sb([64, C])
        cep = sb([64, 1])
        ceb = sb([64, 1])
        # ---- contraction ----
        w2sb = sb([32, 2 * K2])
        wrevsb = sb([32, 2 * K2])
        r2 = sb([32, 2])
        t1sb = sb([32, 2])
        p8 = sb([K2, 2 * K2])
        s2v = sb([K2, 2])
        # ---- final ----
        sbs = sb([1, 16])
        fs = sb([1, 12])
        warm = sb([1, 1])
        # ---- PSUM: 8 tensors = 8 banks ----
        psumF = ps([128, HC])             # u-fold, then teacher fold
        psumFs = ps([128, HC])            # student fold
        ptrT = ps([100, NT, 64], bf)
        ptrS = ps([100, NT, 64], bf)
        psum_gd = ps([64, NT * 64])
        psum_hd = ps([100, NT * C])
        psumW = ps([K1, 2 * K2])
        psmall = ps([32, 64])

        SC_TT, SC_SS, SC_TS, SC_KD, SC_CE, SC_SG, SC_SH = 0, 1, 2, 3, 4, 5, 6
        SC_S1T, SC_S1S, SC_S2T, SC_S2S = 7, 8, 9, 10
        SC_UT, SC_US = 16, 24
        SC_T1 = 32
        SC_WR = 40

        wv = [-ALPHA * T * T / (B * C) for T in range(1, NT + 1)]

        with (
            nc.semaphore("d_in") as d_in,
            nc.semaphore("d_tl") as d_tl,
            nc.semaphore("d_tg") as d_tg,
            nc.semaphore("d_out") as d_out,
            nc.semaphore("vsem") as vsem,
            nc.semaphore("psem") as psem,
            nc.semaphore("asem") as asem,
            nc.semaphore("tsem") as tsem,
            nc.Block() as block,
        ):
            # ---------------- SP: DMA only ----------------
            @block.sync
            def _(s):
                s.dma_start(out=sl_[:], in_=ls_d[:, :]).then_inc(d_in, 16)
                s.dma_start(out=tl_[:], in_=lt_d[:, :]).then_inc(d_tl, 16)
                s.dma_start(out=tg[:], in_=tg_d[:, :]).then_inc(d_tg, 16)
                s.wait_ge(vsem, 25)       # final scalar ready
                s.dma_start(out=out_d[:, :], in_=fs[:, 0:1]).then_inc(d_out, 16)
                s.wait_ge(d_out, 16)

            # ---------------- Pool ----------------
            # psem: 1=ones 2=quick constants 3=tls 4=ireps 5=cube_snb
            #       6=cube_tb 7=tsprod 8=hi_b 9..13=tsef2 groups
            @block.gpsimd
            def _(g):
                g.memset(ones[:], 1.0).then_inc(psem, 1)   # 1: ACT warmup gate
                g.memset(ones_b[:], 1.0)
                g.memset(onesq[:], 1.0)
                g.memset(id64b[:], 0.0)
                g.memset(prev32[:], 0.0)
                g.memset(e1b[:], 0.0)
                g.memset(e2b[:], 0.0)
                g.iota(iota100[:], [[1, C]], channel_multiplier=0,
                       allow_small_or_imprecise_dtypes=True)
                for T in range(1, NT + 1):
                    g.memset(wrow64[:, T - 1:T], wv[T - 1])
                # zero pads (data cols are written later; ranges are disjoint)
                g.memset(u64[:, NT * C:FLAT], 0.0)
                g.memset(cube_tb[:, NT * C:FLAT], 0.0)
                g.memset(cube_sb[:, NT * C:FLAT], 0.0)
                g.memset(cube_snb[:, NT * C:FLAT], 0.0)
                g.drain()
                g.affine_select(m1[:], onesq[:], [[-1, 32]], AL.is_ge, 0.0,
                                base=K1 - 2, channel_multiplier=-1)
                g.affine_select(m1lo[:], onesq[0:K2, 0:K2], [[-1, K2]], AL.is_ge,
                                0.0, base=K2 - 2, channel_multiplier=-1)
                g.affine_select(id64b[:], id64b[:], [[-1, 64]], AL.not_equal,
                                1.0, base=0, channel_multiplier=1)
                g.affine_select(prev32[:], prev32[:], [[-1, 32]], AL.not_equal,
                                1.0, base=K1 - 1, channel_multiplier=-1)
                g.affine_select(e1b[:], e1b[:], [[-1, 128]], AL.not_equal,
                                1.0, base=0, channel_multiplier=1)
                g.affine_select(e2b[:], e2b[:], [[-1, 128]], AL.not_equal,
                                1.0, base=64, channel_multiplier=1).then_inc(psem, 1)   # 2
                # teacher prescale (ahead of the slow irep iotas)
                g.wait_ge(d_tl, 16)
                ins = None
                for T in range(1, NT + 1):
                    i = T - 1
                    ins = nc.gpsimd.tensor_scalar_mul(
                        tls[:, i * C:(i + 1) * C], tl_[:], 1.0 / T)
                ins.then_inc(psem, 1)     # 3: tls
                g.iota(irep32[:], [[1, K1], [0, 72]], channel_multiplier=0,
                       allow_small_or_imprecise_dtypes=True)
                g.iota(irep8[:], [[1, K2], [0, 72]], channel_multiplier=0,
                       allow_small_or_imprecise_dtypes=True).then_inc(psem, 1)  # 4
                g.wait_ge(vsem, 5)        # rs_sn
                ins = None
                for T in range(1, NT + 1):
                    i = T - 1
                    slc = slice(i * C, (i + 1) * C)
                    ins = nc.gpsimd.tensor_scalar_mul(cube_snb[:, slc], es[:, slc],
                                                      rs_sn[:, i:i + 1])
                ins.then_inc(psem, 1)     # 5: cube_snb
                g.wait_ge(vsem, 6)        # rs_t
                ins = None
                for T in range(1, NT + 1):
                    i = T - 1
                    slc = slice(i * C, (i + 1) * C)
                    ins = nc.gpsimd.tensor_scalar_mul(cube_tb[:, slc], et[:, slc],
                                                      rs_t[:, i:i + 1])
                ins.then_inc(psem, 1)     # 6: cube_tb
                g.wait_ge(vsem, 10)       # hi_i (DVE shift)
                nc.gpsimd.tensor_copy(out=hi_b[:], in_=hi_i[:]).then_inc(psem, 1)  # 7
                g.wait_ge(asem, 6)        # t128b + s128b (ACT copies)
                nc.gpsimd.tensor_tensor(out=tsprod[:], in0=t128b[:], in1=s128b[:],
                                        op=AL.mult).then_inc(psem, 1)  # 8: tsprod
                # KD product pieces (consumed by one ACT accum op)
                nc.gpsimd.tensor_copy(out=d0b[:], in_=d0[:])
                ins = None
                for T in range(1, NT + 1):
                    i = T - 1
                    ins = nc.gpsimd.tensor_scalar_mul(
                        cw[:, i * C:(i + 1) * C], cube_tb[:, i * C:(i + 1) * C],
                        wv[i] / T)
                g.drain()
                nc.gpsimd.tensor_tensor(out=cwp[:], in0=cw[:],
                                        in1=_ap3(d0b[:], bcast_mid=NT),
                                        op=AL.mult).then_inc(psem, 1)  # 9: cwp
                for gi in range(NG):
                    cs = slice(GO[gi], GO[gi] + GS[gi])
                    g.wait_ge(vsem, 11 + 2 * gi)   # lm2 group built
                    nc.gpsimd.tensor_tensor(
                        out=_gslice(tsef2[:], K2, gi),
                        in0=_gslice(lm2[:], K2, gi),
                        in1=_ap3(t128b[:, cs], bcast_mid=K2), op=AL.mult)
                    g.drain()
                    nc.gpsimd.tensor_tensor(
                        out=_gslice(tsef2[:], K2, gi, colhalf=1),
                        in0=_gslice(lm2[:], K2, gi),
                        in1=_ap3(s128b[:, cs], bcast_mid=K2),
                        op=AL.mult).then_inc(psem, 1)   # 10..14


            # ---------------- ACT ----------------
            # asem: 1=exp_s 2=exp_t 3=lse_s 4=lse_t 5=s128b 6=t128b
            #       7=L2 squares 8=tr copies 9=gram squares
            @block.scalar
            def _(a):
                a.wait_ge(psem, 1)
                nc.scalar.activation(out=warm[:], in_=ones[0:1, :], func=AF.Exp)
                a.wait_ge(vsem, 1)        # sls
                nc.scalar.activation(out=es[:], in_=sls[:], func=AF.Exp).then_inc(asem, 1)
                a.wait_ge(psem, 3)        # tls
                nc.scalar.activation(out=et[:], in_=tls[:], func=AF.Exp).then_inc(asem, 1)
                a.wait_ge(vsem, 3)        # se_s
                nc.scalar.activation(out=lse_s[:], in_=se_s[:],
                                     func=AF.Ln).then_inc(asem, 1)
                a.wait_ge(vsem, 4)        # se_t (before recips)
                nc.scalar.activation(out=lse_t[:], in_=se_t[:],
                                     func=AF.Ln).then_inc(asem, 1)
                a.wait_ge(tsem, 2)        # fold S in PSUM
                nc.scalar.activation(out=s128b[:], in_=psumFs[:],
                                     func=AF.Identity).then_inc(asem, 1)  # 5
                a.wait_ge(tsem, 3)        # fold T in PSUM
                nc.scalar.activation(out=t128b[:], in_=psumF[:],
                                     func=AF.Identity).then_inc(asem, 1)  # 6
                nc.scalar.activation(out=sq_t[:], in_=psumF[:], func=AF.Square,
                                     accum_out=tt128[:])
                nc.scalar.activation(out=sq_s[:], in_=psumFs[:], func=AF.Square,
                                     accum_out=ss128[:]).then_inc(asem, 1)  # 7
                a.wait_ge(tsem, 4)        # transposes done
                nc.scalar.activation(out=trT[:], in_=ptrT[:], func=AF.Identity)
                nc.scalar.activation(out=trS[:], in_=ptrS[:], func=AF.Identity)
                nc.scalar.activation(out=trSn[:], in_=ptrS[:], func=AF.Identity,
                                     scale=-1.0).then_inc(asem, 1)   # 8
                a.wait_ge(psem, 9)        # cwp
                nc.scalar.activation(out=kscr[:], in_=cwp[:], func=AF.Identity,
                                     accum_out=k1act[:]).then_inc(asem, 1)  # 9
                a.wait_ge(vsem, 9)        # ohs + kz written (cf implies both)
                a.wait_ge(psem, 8)        # tsprod
                nc.scalar.activation(out=cscr[:], in_=ohs[:], func=AF.Identity,
                                     accum_out=cep[:])
                nc.scalar.activation(out=zscr[:], in_=kz[:], func=AF.Identity,
                                     accum_out=kdzd[:])
                nc.scalar.activation(out=tscr[:], in_=tsprod[:], func=AF.Identity,
                                     accum_out=ts128[:]).then_inc(asem, 1)  # 10
                a.wait_ge(tsem, 5)        # gram mms done
                nc.scalar.activation(out=sqg[:], in_=psum_gd[:], func=AF.Square,
                                     accum_out=accg[:])
                nc.scalar.activation(out=sqh[:], in_=psum_hd[:], func=AF.Square,
                                     accum_out=acch[:]).then_inc(asem, 1)  # 11

            # ---------------- DVE ----------------
            # vsem: 1=sls 2=se_s 3=rs_sn 4=se_t+recip_t 5=u64 6=cube_sb 7=ceb
            #       8=cf 9=hi_i  10,12,14,16,18=lm2  11,13,15,17,19=eg2
            #       20=kdv+ts128 21=w2sb+r2 22=wrevsb+t1sb 23=s2v 24=final
            @block.vector
            def _(v):
                v.wait_ge(d_in, 16)
                ins = None
                for T in range(1, NT + 1):
                    i = T - 1
                    ins = nc.vector.tensor_scalar_mul(
                        sls[:, i * C:(i + 1) * C], sl_[:], 1.0 / T)
                ins.then_inc(vsem, 1)
                v.wait_ge(d_tl, 16)
                nc.vector.tensor_sub(out=d0[:], in0=sl_[:], in1=tl_[:])
                v.drain()
                ins = None
                for T in range(1, NT + 1):
                    i = T - 1
                    ins = nc.vector.tensor_scalar_mul(
                        u64[:, i * C:(i + 1) * C], d0[:], 1.0 / T)
                ins.then_inc(vsem, 1)     # 2: ud (zd added during the fold)
                v.wait_ge(asem, 1)        # exp_s
                v.drain()
                with nc.allow_low_precision(reason="se sums tolerate bf16"):
                    nc.vector.tensor_reduce(out=se_s[:], in_=es[:].rearrange(
                        "p (t c) -> p t c", t=NT), axis=AX.X, op=AL.add).then_inc(vsem, 1)  # 3
                v.wait_ge(asem, 2)        # exp_t
                v.drain()
                with nc.allow_low_precision(reason="se sums tolerate bf16"):
                    nc.vector.tensor_reduce(out=se_t[:], in_=et[:].rearrange(
                        "p (t c) -> p t c", t=NT), axis=AX.X, op=AL.add).then_inc(vsem, 1)  # 4
                v.drain()
                nc.vector.reciprocal(out=rs_s[:], in_=se_s[:])
                v.drain()
                nc.vector.tensor_scalar_mul(rs_sn[:], rs_s[:], -1.0).then_inc(vsem, 1)  # 5
                v.drain()
                nc.vector.reciprocal(out=rs_t[:], in_=se_t[:]).then_inc(vsem, 1)  # 6
                v.wait_ge(asem, 4)        # lse_t (and lse_s)
                nc.vector.tensor_sub(out=zd[:], in0=lse_t[:], in1=lse_s[:])
                v.drain()
                nc.vector.tensor_copy(out=zdb[:], in_=zd[:]).then_inc(vsem, 1)  # 7
                v.drain()
                nc.vector.tensor_tensor(out=kz[:], in0=zd[:], in1=wrow64[:],
                                        op=AL.mult)
                ins = None
                for T in range(1, NT + 1):
                    i = T - 1
                    slc = slice(i * C, (i + 1) * C)
                    ins = nc.vector.tensor_scalar_mul(cube_sb[:, slc], es[:, slc],
                                                      rs_s[:, i:i + 1])
                ins.then_inc(vsem, 1)     # 8: cube_sb
                # CE one-hot products fill the wait for the u-fold
                v.wait_ge(d_tg, 16)
                nc.vector.tensor_copy(out=tgf[:], in_=tg[:])
                v.drain()
                nc.vector.tensor_tensor(out=oh[:],
                                        in0=_ap3(tgf[:], bcast_inner=C)[:, 0, :],
                                        in1=iota100[:], op=AL.is_equal)
                v.drain()
                nc.vector.tensor_tensor(out=ohs[:], in0=oh[:], in1=sl_[:], op=AL.mult)
                # bucket chain straight from the u-fold PSUM (sole reader here)
                v.wait_ge(tsem, 1)        # u-fold done
                nc.vector.tensor_scalar(cf[:], psumF[:], INVW, K / 2.0 - 0.5,
                                        AL.mult, AL.add).then_inc(vsem, 1)  # 8
                v.drain()
                nc.vector.tensor_copy(out=ci[:], in_=cf[:])
                v.drain()
                nc.vector.tensor_scalar(hi_i[:], ci[:], 3, None,
                                        AL.arith_shift_right).then_inc(vsem, 1)  # 9
                nc.vector.tensor_scalar(lo_i[:], ci[:], 7, None, AL.bitwise_and)
                v.drain()
                nc.vector.tensor_copy(out=lo_b[:], in_=lo_i[:])
                v.drain()
                v.wait_ge(psem, 4)        # ireps
                for gi in range(NG):
                    cs = slice(GO[gi], GO[gi] + GS[gi])
                    nc.vector.tensor_tensor(
                        out=_gslice(lm2[:], K2, gi),
                        in0=_ap3(lo_b[:, cs], bcast_mid=K2),
                        in1=_mkap(irep8[:], [list(irep8[:].ap[0]),
                                             [72, K2], [1, GS[gi]]]),
                        op=AL.is_equal).then_inc(vsem, 1)   # 9,11,... (lm2)
                    if gi == 0:
                        v.wait_ge(psem, 7)    # hi_b (Pool, also implies ireps)
                    v.drain()
                    nc.vector.tensor_tensor(
                        out=_gslice(eg2[:], K1, gi),
                        in0=_ap3(hi_b[:, cs], bcast_mid=K1),
                        in1=_mkap(irep32[:], [list(irep32[:].ap[0]),
                                              [72, K1], [1, GS[gi]]]),
                        op=AL.is_equal).then_inc(vsem, 1)   # 10,12,... (eg2)
                # CE + KD finals (reduces went to ACT accum)
                v.wait_ge(asem, 10)       # cep/kdzd/ts accums
                nc.vector.tensor_sub(out=ceb[:], in0=lse_s[:, 0:1], in1=cep[:])
                v.drain()
                nc.vector.tensor_add(out=kdv[:], in0=k1act[:],
                                     in1=kdzd[:]).then_inc(vsem, 1)  # 21 (ceb+kdv)
                v.wait_ge(tsem, 6)        # histogram done
                nc.vector.tensor_reduce(out=r2[:, 0:1], in_=psumW[:, 0:K2],
                                        axis=AX.X, op=AL.add)
                nc.vector.tensor_reduce(out=r2[:, 1:2], in_=psumW[:, K2:2 * K2],
                                        axis=AX.X, op=AL.add)
                nc.vector.tensor_copy(out=w2sb[:], in_=psumW[:]).then_inc(vsem, 1)  # 22
                v.wait_ge(tsem, 7)        # wrev/t1 mms done
                nc.vector.tensor_copy(out=wrevsb[:], in_=psmall[:, SC_WR:SC_WR + 2 * K2])
                nc.vector.tensor_copy(out=t1sb[:], in_=psmall[:, SC_T1:SC_T1 + 2]).then_inc(vsem, 1)  # 23
                v.wait_ge(tsem, 8)        # U/S1 mms done
                nc.vector.tensor_tensor(out=p8[:, 0:K2], in0=m1lo[:],
                                        in1=psmall[0:K2, SC_UT:SC_UT + K2], op=AL.mult)
                nc.vector.tensor_tensor(out=p8[:, K2:2 * K2], in0=m1lo[:],
                                        in1=psmall[0:K2, SC_US:SC_US + K2], op=AL.mult)
                v.drain()
                nc.vector.tensor_reduce(out=s2v[:, 0:1], in_=p8[:, 0:K2],
                                        axis=AX.X, op=AL.add)
                nc.vector.tensor_reduce(out=s2v[:, 1:2], in_=p8[:, K2:2 * K2],
                                        axis=AX.X, op=AL.add).then_inc(vsem, 1)  # 24
                v.wait_ge(tsem, 10)       # all scalar mms done
                nc.vector.tensor_copy(out=sbs[:, 0:11], in_=psmall[0:1, 0:11])
                tt_, ss_, ts_ = sbs[:, 0:1], sbs[:, 1:2], sbs[:, 2:3]
                kd_, ce_ = sbs[:, 3:4], sbs[:, 4:5]
                sg_, sh_ = sbs[:, 5:6], sbs[:, 6:7]
                s1t, s1s, s2t, s2s = (sbs[:, i:i + 1] for i in range(7, 11))
                v.drain()
                nc.vector.tensor_add(out=fs[:, 0:1], in0=s1t, in1=s2t)
                nc.vector.tensor_add(out=fs[:, 1:2], in0=s1s, in1=s2s)
                nc.vector.tensor_mul(out=fs[:, 2:3], in0=tt_, in1=tt_)
                nc.vector.tensor_mul(out=fs[:, 3:4], in0=ss_, in1=ss_)
                nc.vector.tensor_mul(out=fs[:, 4:5], in0=ts_, in1=ts_)
                nc.vector.tensor_add(out=fs[:, 5:6], in0=sg_, in1=sh_)
                v.drain()
                nc.vector.tensor_sub(out=fs[:, 6:7], in0=fs[:, 0:1], in1=fs[:, 1:2])
                nc.vector.tensor_add(out=fs[:, 7:8], in0=fs[:, 2:3], in1=fs[:, 3:4])
                nc.vector.tensor_add(out=fs[:, 8:9], in0=fs[:, 5:6], in1=kd_)
                v.drain()
                nc.vector.scalar_tensor_tensor(out=fs[:, 9:10], in0=fs[:, 4:5],
                                               scalar=-2.0, in1=fs[:, 7:8],
                                               op0=AL.mult, op1=AL.add)
                nc.vector.scalar_tensor_tensor(out=fs[:, 10:11], in0=ce_,
                                               scalar=NT * (1.0 - ALPHA) / B,
                                               in1=fs[:, 8:9],
                                               op0=AL.mult, op1=AL.add)
                v.drain()
                nc.vector.scalar_tensor_tensor(out=fs[:, 11:12], in0=fs[:, 6:7],
                                               scalar=2.0, in1=fs[:, 9:10],
                                               op0=AL.mult, op1=AL.add)
                v.drain()
                nc.vector.scalar_tensor_tensor(out=fs[:, 0:1], in0=fs[:, 11:12],
                                               scalar=0.00025, in1=fs[:, 10:11],
                                               op0=AL.mult, op1=AL.add).then_inc(vsem, 1)  # 25

            # ---------------- PE ----------------
            # tsem: 1=u-fold 2=foldS 3=foldT 4=transposes 5=grams 6=hist
            #       7=wrev+t1 8=U+S1 9=S2+scalars
            @block.tensor
            def _(t):
                t.wait_ge(psem, 2)        # identities
                t.wait_ge(vsem, 2)        # ud (= (sl-tl)/T, no zd yet)
                nc.tensor.matmul(psumF[:], lhsT=e1b[:], rhs=u64[:, 0:HC],
                                 start=True, stop=False, skip_group_check=True)
                nc.tensor.matmul(psumF[:], lhsT=e2b[:], rhs=u64[:, HC:FLAT],
                                 start=False, stop=True, skip_group_check=True)
                # accumulate the per-(row, temp-block) lse bias into the fold:
                # rows 0:64 cover temps (1,1,2,2,3-head); rows 64:128 cover
                # (3-tail,4,4,5,5,pad). Broadcast APs over zd columns.
                t.wait_ge(vsem, 7)        # zd
                nc.tensor.matmul(psumF[:, 0:200], lhsT=e1b[:],
                                 rhs=_mkap(zdb[:], [list(zdb[:].ap[0]), [1, 2], [0, C]]),
                                 start=False, stop=False, skip_group_check=True)
                nc.tensor.matmul(psumF[:, 200:HC], lhsT=e1b[:],
                                 rhs=_mkap(zdb[:], [list(zdb[:].ap[0]), [0, HC - 200]],
                                           extra_off=2),
                                 start=False, stop=False, skip_group_check=True)
                nc.tensor.matmul(psumF[:, 0:44], lhsT=e2b[:],
                                 rhs=_mkap(zdb[:], [list(zdb[:].ap[0]), [0, 44]],
                                           extra_off=2),
                                 start=False, stop=False, skip_group_check=True)
                nc.tensor.matmul(psumF[:, 44:244], lhsT=e2b[:],
                                 rhs=_mkap(zdb[:], [list(zdb[:].ap[0]), [1, 2], [0, C]],
                                           extra_off=3),
                                 start=False, stop=True,
                                 skip_group_check=True).then_inc(tsem, 1)
                t.wait_ge(vsem, 8)        # cube_sb
                nc.tensor.matmul(psumFs[:], lhsT=e1b[:], rhs=cube_sb[:, 0:HC],
                                 start=True, stop=False, skip_group_check=True)
                nc.tensor.matmul(psumFs[:], lhsT=e2b[:], rhs=cube_sb[:, HC:FLAT],
                                 start=False, stop=True,
                                 skip_group_check=True).then_inc(tsem, 1)  # 2
                t.wait_ge(psem, 6)        # cube_tb
                t.wait_ge(vsem, 9)        # cf has finished reading psumF
                nc.tensor.matmul(psumF[:], lhsT=e1b[:], rhs=cube_tb[:, 0:HC],
                                 start=True, stop=False, skip_group_check=True)
                nc.tensor.matmul(psumF[:], lhsT=e2b[:], rhs=cube_tb[:, HC:FLAT],
                                 start=False, stop=True,
                                 skip_group_check=True).then_inc(tsem, 1)  # 3
                ins = None
                for k in range(NT):
                    slc = slice(k * C, (k + 1) * C)
                    nc.tensor.transpose(out=ptrT[:, k, :], in_=cube_tb[:, slc],
                                        identity=id64b[:])
                    ins = nc.tensor.transpose(out=ptrS[:, k, :], in_=cube_sb[:, slc],
                                              identity=id64b[:])
                ins.then_inc(tsem, 1)     # 4
                t.wait_ge(asem, 8)        # trT/trS/trSn
                t.wait_ge(psem, 5)        # cube_snb
                ins = None
                for k in range(NT):
                    slc = slice(k * C, (k + 1) * C)
                    nc.tensor.matmul(psum_hd[:, slc], lhsT=cube_tb[:, slc],
                                     rhs=cube_tb[:, slc], start=True, stop=False,
                                     skip_group_check=True)
                    nc.tensor.matmul(psum_hd[:, slc], lhsT=cube_snb[:, slc],
                                     rhs=cube_sb[:, slc], start=False, stop=True,
                                     skip_group_check=True)
                    nc.tensor.matmul(psum_gd[:, k * 64:(k + 1) * 64],
                                     lhsT=trT[:, k, :], rhs=trT[:, k, :],
                                     start=True, stop=False, skip_group_check=True)
                    ins = nc.tensor.matmul(psum_gd[:, k * 64:(k + 1) * 64],
                                           lhsT=trSn[:, k, :], rhs=trS[:, k, :],
                                           start=False, stop=True,
                                           skip_group_check=True)
                ins.then_inc(tsem, 1)     # 5
                ins = None
                for gi in range(NG):
                    t.wait_ge(vsem, 12 + 2 * gi)
                    t.wait_ge(psem, 10 + gi)
                    for i in range(GS[gi]):
                        ch = GO[gi] + i
                        ins = nc.tensor.matmul(psumW[:],
                                               lhsT=_chunkap(eg2[:], K1, ch),
                                               rhs=_chunkap(tsef2[:], 2 * K2, ch),
                                               start=(ch == 0), stop=(ch == NCHUNK - 1),
                                               skip_group_check=True)
                ins.then_inc(tsem, 1)     # 6: histogram done
                t.wait_ge(vsem, 22)       # w2sb + r2
                nc.tensor.matmul(psmall[:, SC_WR:SC_WR + 2 * K2], lhsT=prev32[:],
                                 rhs=w2sb[:], start=True, stop=True,
                                 skip_group_check=True)
                nc.tensor.matmul(psmall[:, SC_T1:SC_T1 + 2], lhsT=m1[:], rhs=r2[:],
                                 start=True, stop=True,
                                 skip_group_check=True).then_inc(tsem, 1)  # 7
                t.wait_ge(vsem, 23)       # wrevsb + t1sb
                nc.tensor.matmul(psmall[0:K2, SC_UT:SC_UT + K2],
                                 lhsT=w2sb[:, 0:K2], rhs=wrevsb[:, 0:K2],
                                 start=True, stop=True, skip_group_check=True)
                nc.tensor.matmul(psmall[0:K2, SC_US:SC_US + K2],
                                 lhsT=w2sb[:, K2:2 * K2], rhs=wrevsb[:, K2:2 * K2],
                                 start=True, stop=True, skip_group_check=True)
                nc.tensor.matmul(psmall[0:1, SC_S1T:SC_S1T + 1], lhsT=t1sb[:, 0:1],
                                 rhs=r2[:, 0:1], start=True, stop=True,
                                 skip_group_check=True)
                nc.tensor.matmul(psmall[0:1, SC_S1S:SC_S1S + 1], lhsT=t1sb[:, 1:2],
                                 rhs=r2[:, 1:2], start=True, stop=True,
                                 skip_group_check=True).then_inc(tsem, 1)  # 8
                t.wait_ge(asem, 7)        # tt128/ss128
                nc.tensor.matmul(psmall[0:1, SC_TT:SC_TT + 1], lhsT=ones[:, 0:1],
                                 rhs=tt128[:], start=True, stop=True,
                                 skip_group_check=True)
                nc.tensor.matmul(psmall[0:1, SC_SS:SC_SS + 1], lhsT=ones[:, 0:1],
                                 rhs=ss128[:], start=True, stop=True,
                                 skip_group_check=True)
                t.wait_ge(asem, 10)       # ts128
                nc.tensor.matmul(psmall[0:1, SC_TS:SC_TS + 1], lhsT=ones[:, 0:1],
                                 rhs=ts128[:], start=True, stop=True,
                                 skip_group_check=True)
                nc.tensor.matmul(psmall[0:1, SC_KD:SC_KD + 1], lhsT=ones[0:64, 0:1],
                                 rhs=kdv[:], start=True, stop=True,
                                 skip_group_check=True)
                t.wait_ge(vsem, 21)       # ceb+kdv
                nc.tensor.matmul(psmall[0:1, SC_CE:SC_CE + 1], lhsT=ones[0:64, 0:1],
                                 rhs=ceb[:], start=True, stop=True,
                                 skip_group_check=True)
                t.wait_ge(asem, 11)       # accg/acch
                nc.tensor.matmul(psmall[0:1, SC_SG:SC_SG + 1], lhsT=ones[0:64, 0:1],
                                 rhs=accg[:], start=True, stop=True,
                                 skip_group_check=True)
                nc.tensor.matmul(psmall[0:1, SC_SH:SC_SH + 1], lhsT=ones[0:100, 0:1],
                                 rhs=acch[:], start=True, stop=True,
                                 skip_group_check=True).then_inc(tsem, 1)  # 9: scalars
                t.wait_ge(vsem, 24)       # s2v
                nc.tensor.matmul(psmall[0:1, SC_S2T:SC_S2T + 1], lhsT=ones[0:K2, 0:1],
                                 rhs=s2v[:, 0:1], start=True, stop=True,
                                 skip_group_check=True)
                nc.tensor.matmul(psmall[0:1, SC_S2S:SC_S2S + 1], lhsT=ones[0:K2, 0:1],
                                 rhs=s2v[:, 1:2], start=True, stop=True,
                                 skip_group_check=True).then_inc(tsem, 1)  # 10: S2

    return nc


_cache = {}


def _get_nc():
    if "nc" not in _cache:
        _cache["nc"] = build()
    return _cache["nc"]


def kernel(logits_student, logits_teacher, target):
    from concourse.bass_utils import run_bass_kernel_spmd

    nc = _get_nc()
    in_map = {
        "logits_student": np.ascontiguousarray(logits_student, dtype=np.float32),
        "logits_teacher": np.ascontiguousarray(logits_teacher, dtype=np.float32),
        "target": np.ascontiguousarray(np.asarray(target).reshape(B, 1).astype(np.int32)),
    }
    core_ids = list(range(8))
    res = run_bass_kernel_spmd(nc, [in_map] * 8, core_ids)
    out = res.results[0]["out"]
    return np.float32(out.reshape(())).reshape(())


# revision 55
# speedup vs baseline: 2.5670x; 1.0147x over previous
# Trainium2 Bass kernel for nn_CKDLoss: KD loss + virtual-outer-product L1/L2
# + Gram-matrix sub-losses, computed entirely on device.
#
# Sharding: total work after algorithmic reduction is tiny and latency-bound;
# cross-core collectives cost more than the whole computation. Every core runs
# the identical full computation on replicated inputs; host takes core 0.
#
# L1 math: with u = log s - log t (normalized softmax cubes flattened to N),
#   sum_{a,b} |t_a t_b - s_a s_b| = 2*(S_tt - S_ss)   (T = S = 320 cancel),
#   S_tt = sum_{pairs: u_a+u_b<0} t_a t_b.
# Key identity: u = (sl - tl)/T + (lse_t - lse_s) per element — no exp/log of
# cube values on the u path; five tensor_scalar ops on the raw logits plus
# the row-lse bias build u64 exactly.
# Bucketize c = floor((u+UMAX)*K/(2 UMAX)) in [0,K), c = 8*hi + lo.
# The positive-pair test c_a+c_b <= K-2 splits exactly into
#   (hi_a+hi_b <= K1-2)  OR  (hi_a+hi_b = K1-1 AND lo_a+lo_b <= K2-2).
# Build W[hi, lo] = sum_n t_n 1[hi_n=hi] 1[lo_n=lo] (PSUM-accumulated one-hot
# matmuls, bf16). Reversed rows Wrev = P_antidiag @ W via one permutation
# matmul. Then with r[hi] = sum_lo W[hi, lo]:
#   S1 = r^T M1 r                 (M1[a,b] = 1[a+b<=K1-2])
#   U  = W^T Wrev  (8x8),  S2 = sum m1lo * U   (m1lo[a,b] = 1[a+b<=K2-2])
#   S_tt = S1 + S2.
# bf16 in the heavy path is safe: measured end-to-end shift vs f32 is ~1e-5
# of the loss (the bucketization itself is 2.3e-3).
#
# Layout: [64, 512] flat tensors (500 data + 12 zero pad) fold to [128, 256]
# via two permutation matmuls (split at flat col 256). f32r at 256 output
# columns runs 1 cycle/row, so the u-fold costs ~0.5us. Pad elements carry
# zero weight everywhere, so they never contribute. One-hots are built in
# [128, K, chunk] layout (bucket index as MIDDLE dim) so all build operands
# have packed 2-byte last dims -> DVE 2x mode. Pool (gpsimd) cannot run
# is_equal/shift through the walrus codegen, so DVE builds the one-hots and
# Pool applies the t/s weights. A PSUM bank is read by at most one engine
# per phase (HW forbids concurrent multi-engine reads of a bank).

import numpy as np
from contextlib import ExitStack

B, C, NT = 64, 100, 5            # batch, classes, temps 1..5
N = B * C * NT                   # 32000 flattened cube elements
K1, K2 = 32, 8                   # two-level bucket split, K = 256
K = K1 * K2
UMAX = 16.0
INVW = K / (2.0 * UMAX)          # 8.0
ALPHA = 0.7
FLAT = 512                       # padded flat width (500 data + 12 pad)
HC = FLAT // 2                   # 256 folded columns
NCHUNK = HC                      # 256 PE chunks of 128 elements
NG = 5
GS = [72, 72, 64, 40, 8]        # descending group sizes (small tail group)
GO = [0, 72, 144, 208, 248]


def _mkap(tensor_ap, dims, extra_off=0):
    import concourse.bass as bass
    return bass.AP(tensor=tensor_ap.tensor, offset=tensor_ap.offset + extra_off,
                   ap=[list(d) for d in dims])


def _ap3(ap, bcast_inner=None, bcast_mid=None):
    """Append/insert stride-0 dims on an AP: [P,F] -> [P,F,bi] or [P,bm,F]."""
    dims = [list(d) for d in ap.ap]
    if bcast_inner is not None:
        dims = dims + [[0, bcast_inner]]
    if bcast_mid is not None:
        dims = [dims[0], [0, bcast_mid]] + dims[1:]
    return _mkap(ap, dims)


def _gslice(t_ap, ncols, gi, colhalf=0):
    """[128, ncols, NCHUNK] tensor -> [128, ncols, GS[gi]] AP for group gi."""
    dims = [list(t_ap.ap[0]), [NCHUNK, ncols], [1, GS[gi]]]
    return _mkap(t_ap, dims, extra_off=colhalf * K2 * NCHUNK + GO[gi])


def _chunkap(t_ap, ncols, ch):
    """[128, ncols, NCHUNK] tensor -> [128, ncols] AP for chunk ch."""
    dims = [list(t_ap.ap[0]), [NCHUNK, ncols]]
    return _mkap(t_ap, dims, extra_off=ch)


def build():
    import concourse.bass as bass
    from concourse import mybir

    dt = mybir.dt
    AL = mybir.AluOpType
    AF = mybir.ActivationFunctionType
    AX = mybir.AxisListType
    bf = dt.bfloat16
    fr = dt.float32r

    nc = bass.Bass()
    ls_d = nc.declare_dram_parameter("logits_student", [B, C], dt.float32, isOutput=False)
    lt_d = nc.declare_dram_parameter("logits_teacher", [B, C], dt.float32, isOutput=False)
    tg_d = nc.declare_dram_parameter("target", [B, 1], dt.int32, isOutput=False)
    out_d = nc.declare_dram_parameter("out", [1, 1], dt.float32, isOutput=True)

    ctx = ExitStack()
    _n = [0]

    def sb(shape, d=dt.float32):
        _n[0] += 1
        return ctx.enter_context(nc.sbuf_tensor(f"sb{_n[0]}", shape, d))

    def ps(shape, d=dt.float32):
        _n[0] += 1
        return ctx.enter_context(nc.psum_tensor(f"ps{_n[0]}", shape, d))

    with ctx:
        # ---- constants ----
        ones = sb([128, 1])
        ones_b = sb([128, 1], bf)
        iota100 = sb([64, C])
        onesq = sb([32, 32])
        m1 = sb([32, 32])                 # 1[a+b<=K1-2]
        m1lo = sb([K2, K2])               # 1[a+b<=K2-2]
        prev32 = sb([32, 32])             # antidiagonal permutation 1[a+b=31]
        id64b = sb([64, 64], bf)
        e1b = sb([64, 128], bf)           # fold identity half 1: 1[f==p]
        e2b = sb([64, 128], bf)           # fold identity half 2: 1[f==p+64]
        wrow64 = sb([64, NT])             # KD weights -a*T^2/(B*C)
        irep32 = sb([128, K1, 72], bf)    # irep32[p, j, m] = j
        irep8 = sb([128, K2, 72], bf)
        # ---- inputs ----
        sl_ = sb([64, C])
        tl_ = sb([64, C])
        tg = sb([64, 1], dt.int32)
        # ---- softmax / u stage ----
        sls = sb([64, NT * C])            # student logits / T
        tls = sb([64, NT * C])            # teacher logits / T
        d0 = sb([64, C])                  # sl - tl
        u64 = sb([64, FLAT], bf)          # (sl-tl)/T + zd, padded
        es = sb([64, NT * C], bf)         # exp(student)
        et = sb([64, NT * C], bf)
        se_s = sb([64, NT], bf)
        se_t = sb([64, NT], bf)
        rs_s = sb([64, NT])
        rs_t = sb([64, NT])
        rs_sn = sb([64, NT])
        lse_s = sb([64, NT])
        lse_t = sb([64, NT])
        zd = sb([64, NT])
        zdb = sb([64, NT], bf)
        cube_tb = sb([64, FLAT], bf)      # normalized teacher cube (padded)
        cube_sb = sb([64, FLAT], bf)
        cube_snb = sb([64, FLAT], bf)
        # ---- folded [128, 256] ----
        t128b = sb([128, HC], bf)
        s128b = sb([128, HC], bf)
        cf = sb([128, HC])
        ci = sb([128, HC], dt.int32)
        hi_i = sb([128, HC], dt.int32)
        lo_i = sb([128, HC], dt.int32)
        hi_b = sb([128, HC], bf)
        lo_b = sb([128, HC], bf)
        # ---- histogram build ----
        eg2 = sb([128, K1, NCHUNK], bf)
        lm2 = sb([128, K2, NCHUNK], bf)
        tsef2 = sb([128, 2 * K2, NCHUNK], bf)
        # ---- grams ----
        trT = sb([100, NT, 64], bf)
        trS = sb([100, NT, 64], bf)
        trSn = sb([100, NT, 64], bf)
        sqg = sb([64, NT * 64], bf)
        sqh = sb([100, NT * C], bf)
        accg = sb([64, 1])
        acch = sb([100, 1])
        # ---- L2 / KD / CE ----
        sq_t = sb([128, HC], bf)
        sq_s = sb([128, HC], bf)
        tt128 = sb([128, 1])
        ss128 = sb([128, 1])
        tsprod = sb([128, HC], bf)
        ts128 = sb([128, 1])
        d0b = sb([64, C], bf)
        cw = sb([64, NT * C], bf)         # cube_tb * (w_T/T)
        cwp = sb([64, NT * C], bf)        # cw * d0
        kscr = sb([64, NT * C], bf)       # ACT accum scratch
        cscr = sb([64, C], bf)            # CE accum scratch
        zscr = sb([64, NT], bf)
        tscr = sb([128, HC], bf)
        k1act = sb([64, 1])
        kz = sb([64, NT])
        kdzd = sb([64, 1])
        kdv = sb([64, 1])
        tgf = sb([64, 1])
        oh = sb([64, C])
        ohs = sb([64, C])
        cep = sb([64, 1])
        ceb = sb([64, 1])
        # ---- contraction ----
        w2sb = sb([32, 2 * K2])
        wrevsb = sb([32, 2 * K2])
        r2 = sb([32, 2])
        t1sb = sb([32, 2])
        p8 = sb([K2, 2 * K2])
        s2v = sb([K2, 2])
        # ---- final ----
        sbs = sb([1, 16])
        fs = sb([1, 12])
        warm = sb([1, 1])
        # ---- PSUM: 8 tensors = 8 banks ----
        psumF = ps([128, HC])             # u-fold, then teacher fold
        psumFs = ps([128, HC])            # student fold
        ptrT = ps([100, NT, 64], bf)
        ptrS = ps([100, NT, 64], bf)
        psum_gd = ps([64, NT * 64])
        psum_hd = ps([100, NT * C])
        psumW = ps([K1, 2 * K2])
        psmall = ps([32, 64])

        SC_TT, SC_SS, SC_TS, SC_KD, SC_CE, SC_SG, SC_SH = 0, 1, 2, 3, 4, 5, 6
        SC_S1T, SC_S1S, SC_S2T, SC_S2S = 7, 8, 9, 10
        SC_UT, SC_US = 16, 24
        SC_T1 = 32
        SC_WR = 40

        wv = [-ALPHA * T * T / (B * C) for T in range(1, NT + 1)]

        with (
            nc.semaphore("d_in") as d_in,
            nc.semaphore("d_tl") as d_tl,
            nc.semaphore("d_tg") as d_tg,
            nc.semaphore("d_out") as d_out,
            nc.semaphore("vsem") as vsem,
            nc.semaphore("psem") as psem,
            nc.semaphore("asem") as asem,
            nc.semaphore("tsem") as tsem,
            nc.Block() as block,
        ):
            # ---------------- SP: DMA only ----------------
            @block.sync
            def _(s):
                s.dma_start(out=sl_[:], in_=ls_d[:, :]).then_inc(d_in, 16)
                s.dma_start(out=tl_[:], in_=lt_d[:, :]).then_inc(d_tl, 16)
                s.dma_start(out=tg[:], in_=tg_d[:, :]).then_inc(d_tg, 16)
                s.wait_ge(vsem, 25)       # final scalar ready
                s.dma_start(out=out_d[:, :], in_=fs[:, 0:1]).then_inc(d_out, 16)
                s.wait_ge(d_out, 16)

            # ---------------- Pool ----------------
            # psem: 1=ones 2=quick constants 3=tls 4=ireps 5=cube_snb
            #       6=cube_tb 7=tsprod 8=hi_b 9..13=tsef2 groups
            @block.gpsimd
            def _(g):
                g.memset(ones[:], 1.0).then_inc(psem, 1)   # 1: ACT warmup gate
                g.memset(ones_b[:], 1.0)
                g.memset(onesq[:], 1.0)
                g.memset(id64b[:], 0.0)
                g.memset(prev32[:], 0.0)
                g.memset(e1b[:], 0.0)
                g.memset(e2b[:], 0.0)
                g.iota(iota100[:], [[1, C]], channel_multiplier=0,
                       allow_small_or_imprecise_dtypes=True)
                for T in range(1, NT + 1):
                    g.memset(wrow64[:, T - 1:T], wv[T - 1])
                # zero pads (data cols are written later; ranges are disjoint)
                g.memset(u64[:, NT * C:FLAT], 0.0)
                g.memset(cube_tb[:, NT * C:FLAT], 0.0)
                g.memset(cube_sb[:, NT * C:FLAT], 0.0)
                g.memset(cube_snb[:, NT * C:FLAT], 0.0)
                g.drain()
                g.affine_select(m1[:], onesq[:], [[-1, 32]], AL.is_ge, 0.0,
                                base=K1 - 2, channel_multiplier=-1)
                g.affine_select(m1lo[:], onesq[0:K2, 0:K2], [[-1, K2]], AL.is_ge,
                                0.0, base=K2 - 2, channel_multiplier=-1)
                g.affine_select(id64b[:], id64b[:], [[-1, 64]], AL.not_equal,
                                1.0, base=0, channel_multiplier=1)
                g.affine_select(prev32[:], prev32[:], [[-1, 32]], AL.not_equal,
                                1.0, base=K1 - 1, channel_multiplier=-1)
                g.affine_select(e1b[:], e1b[:], [[-1, 128]], AL.not_equal,
                                1.0, base=0, channel_multiplier=1)
                g.affine_select(e2b[:], e2b[:], [[-1, 128]], AL.not_equal,
                                1.0, base=64, channel_multiplier=1).then_inc(psem, 1)   # 2
                # teacher prescale (ahead of the slow irep iotas)
                g.wait_ge(d_tl, 16)
                ins = None
                for T in range(1, NT + 1):
                    i = T - 1
                    ins = nc.gpsimd.tensor_scalar_mul(
                        tls[:, i * C:(i + 1) * C], tl_[:], 1.0 / T)
                ins.then_inc(psem, 1)     # 3: tls
                g.iota(irep32[:], [[1, K1], [0, 72]], channel_multiplier=0,
                       allow_small_or_imprecise_dtypes=True)
                g.iota(irep8[:], [[1, K2], [0, 72]], channel_multiplier=0,
                       allow_small_or_imprecise_dtypes=True).then_inc(psem, 1)  # 4
                g.wait_ge(vsem, 5)        # rs_sn
                ins = None
                for T in range(1, NT + 1):
                    i = T - 1
                    slc = slice(i * C, (i + 1) * C)
                    ins = nc.gpsimd.tensor_scalar_mul(cube_snb[:, slc], es[:, slc],
                                                      rs_sn[:, i:i + 1])
                ins.then_inc(psem, 1)     # 5: cube_snb
                g.wait_ge(vsem, 6)        # rs_t
                ins = None
                for T in range(1, NT + 1):
                    i = T - 1
                    slc = slice(i * C, (i + 1) * C)
                    ins = nc.gpsimd.tensor_scalar_mul(cube_tb[:, slc], et[:, slc],
                                                      rs_t[:, i:i + 1])
                ins.then_inc(psem, 1)     # 6: cube_tb
                g.wait_ge(vsem, 10)       # hi_i (DVE shift)
                nc.gpsimd.tensor_copy(out=hi_b[:], in_=hi_i[:]).then_inc(psem, 1)  # 7
                g.wait_ge(asem, 6)        # t128b + s128b (ACT copies)
                nc.gpsimd.tensor_tensor(out=tsprod[:], in0=t128b[:], in1=s128b[:],
                                        op=AL.mult).then_inc(psem, 1)  # 8: tsprod
                # KD product pieces (consumed by one ACT accum op)
                nc.gpsimd.tensor_copy(out=d0b[:], in_=d0[:])
                ins = None
                for T in range(1, NT + 1):
                    i = T - 1
                    ins = nc.gpsimd.tensor_scalar_mul(
                        cw[:, i * C:(i + 1) * C], cube_tb[:, i * C:(i + 1) * C],
                        wv[i] / T)
                g.drain()
                nc.gpsimd.tensor_tensor(out=cwp[:], in0=cw[:],
                                        in1=_ap3(d0b[:], bcast_mid=NT),
                                        op=AL.mult).then_inc(psem, 1)  # 9: cwp
                for gi in range(NG):
                    cs = slice(GO[gi], GO[gi] + GS[gi])
                    g.wait_ge(vsem, 11 + 2 * gi)   # lm2 group built
                    nc.gpsimd.tensor_tensor(
                        out=_gslice(tsef2[:], K2, gi),
                        in0=_gslice(lm2[:], K2, gi),
                        in1=_ap3(t128b[:, cs], bcast_mid=K2), op=AL.mult)
                    g.drain()
                    nc.gpsimd.tensor_tensor(
                        out=_gslice(tsef2[:], K2, gi, colhalf=1),
                        in0=_gslice(lm2[:], K2, gi),
                        in1=_ap3(s128b[:, cs], bcast_mid=K2),
                        op=AL.mult).then_inc(psem, 1)   # 10..14


            # ---------------- ACT ----------------
            # asem: 1=exp_s 2=exp_t 3=lse_s 4=lse_t 5=s128b 6=t128b
            #       7=L2 squares 8=tr copies 9=gram squares
            @block.scalar
            def _(a):
                a.wait_ge(psem, 1)
                nc.scalar.activation(out=warm[:], in_=ones[0:1, :], func=AF.Exp)
                a.wait_ge(vsem, 1)        # sls
                nc.scalar.activation(out=es[:], in_=sls[:], func=AF.Exp).then_inc(asem, 1)
                a.wait_ge(psem, 3)        # tls
                nc.scalar.activation(out=et[:], in_=tls[:], func=AF.Exp).then_inc(asem, 1)
                a.wait_ge(vsem, 3)        # se_s
                nc.scalar.activation(out=lse_s[:], in_=se_s[:],
                                     func=AF.Ln).then_inc(asem, 1)
                a.wait_ge(vsem, 4)        # se_t (before recips)
                nc.scalar.activation(out=lse_t[:], in_=se_t[:],
                                     func=AF.Ln).then_inc(asem, 1)
                a.wait_ge(tsem, 2)        # fold S in PSUM
                nc.scalar.activation(out=s128b[:], in_=psumFs[:],
                                     func=AF.Identity).then_inc(asem, 1)  # 5
                a.wait_ge(tsem, 3)        # fold T in PSUM
                nc.scalar.activation(out=t128b[:], in_=psumF[:],
                                     func=AF.Identity).then_inc(asem, 1)  # 6
                nc.scalar.activation(out=sq_t[:], in_=psumF[:], func=AF.Square,
                                     accum_out=tt128[:])
                nc.scalar.activation(out=sq_s[:], in_=psumFs[:], func=AF.Square,
                                     accum_out=ss128[:]).then_inc(asem, 1)  # 7
                a.wait_ge(tsem, 4)        # transposes done
                nc.scalar.activation(out=trT[:], in_=ptrT[:], func=AF.Identity)
                nc.scalar.activation(out=trS[:], in_=ptrS[:], func=AF.Identity)
                nc.scalar.activation(out=trSn[:], in_=ptrS[:], func=AF.Identity,
                                     scale=-1.0).then_inc(asem, 1)   # 8
                a.wait_ge(psem, 9)        # cwp
                nc.scalar.activation(out=kscr[:], in_=cwp[:], func=AF.Identity,
                                     accum_out=k1act[:]).then_inc(asem, 1)  # 9
                a.wait_ge(vsem, 9)        # ohs + kz written (cf implies both)
                a.wait_ge(psem, 8)        # tsprod
                nc.scalar.activation(out=cscr[:], in_=ohs[:], func=AF.Identity,
                                     accum_out=cep[:])
                nc.scalar.activation(out=zscr[:], in_=kz[:], func=AF.Identity,
                                     accum_out=kdzd[:])
                nc.scalar.activation(out=tscr[:], in_=tsprod[:], func=AF.Identity,
                                     accum_out=ts128[:]).then_inc(asem, 1)  # 10
                a.wait_ge(tsem, 5)        # gram mms done
                nc.scalar.activation(out=sqg[:], in_=psum_gd[:], func=AF.Square,
                                     accum_out=accg[:])
                nc.scalar.activation(out=sqh[:], in_=psum_hd[:], func=AF.Square,
                                     accum_out=acch[:]).then_inc(asem, 1)  # 11

            # ---------------- DVE ----------------
            # vsem: 1=sls 2=se_s 3=rs_sn 4=se_t+recip_t 5=u64 6=cube_sb 7=ceb
            #       8=cf 9=hi_i  10,12,14,16,18=lm2  11,13,15,17,19=eg2
            #       20=kdv+ts128 21=w2sb+r2 22=wrevsb+t1sb 23=s2v 24=final
            @block.vector
            def _(v):
                v.wait_ge(d_in, 16)
                ins = None
                for T in range(1, NT + 1):
                    i = T - 1
                    ins = nc.vector.tensor_scalar_mul(
                        sls[:, i * C:(i + 1) * C], sl_[:], 1.0 / T)
                ins.then_inc(vsem, 1)
                v.wait_ge(d_tl, 16)
                nc.vector.tensor_sub(out=d0[:], in0=sl_[:], in1=tl_[:])
                v.drain()
                ins = None
                for T in range(1, NT + 1):
                    i = T - 1
                    ins = nc.vector.tensor_scalar_mul(
                        u64[:, i * C:(i + 1) * C], d0[:], 1.0 / T)
                ins.then_inc(vsem, 1)     # 2: ud (zd added during the fold)
                v.wait_ge(asem, 1)        # exp_s
                v.drain()
                with nc.allow_low_precision(reason="se sums tolerate bf16"):
                    nc.vector.tensor_reduce(out=se_s[:], in_=es[:].rearrange(
                        "p (t c) -> p t c", t=NT), axis=AX.X, op=AL.add).then_inc(vsem, 1)  # 3
                v.wait_ge(asem, 2)        # exp_t
                v.drain()
                with nc.allow_low_precision(reason="se sums tolerate bf16"):
                    nc.vector.tensor_reduce(out=se_t[:], in_=et[:].rearrange(
                        "p (t c) -> p t c", t=NT), axis=AX.X, op=AL.add).then_inc(vsem, 1)  # 4
                v.drain()
                nc.vector.reciprocal(out=rs_s[:], in_=se_s[:])
                v.drain()
                nc.vector.tensor_scalar_mul(rs_sn[:], rs_s[:], -1.0).then_inc(vsem, 1)  # 5
                v.drain()
                nc.vector.reciprocal(out=rs_t[:], in_=se_t[:]).then_inc(vsem, 1)  # 6
                v.wait_ge(asem, 4)        # lse_t (and lse_s)
                nc.vector.tensor_sub(out=zd[:], in0=lse_t[:], in1=lse_s[:])
                v.drain()
                nc.vector.tensor_copy(out=zdb[:], in_=zd[:]).then_inc(vsem, 1)  # 7
                v.drain()
                nc.vector.tensor_tensor(out=kz[:], in0=zd[:], in1=wrow64[:],
                                        op=AL.mult)
                ins = None
                for T in range(1, NT + 1):
                    i = T - 1
                    slc = slice(i * C, (i + 1) * C)
                    ins = nc.vector.tensor_scalar_mul(cube_sb[:, slc], es[:, slc],
                                                      rs_s[:, i:i + 1])
                ins.then_inc(vsem, 1)     # 8: cube_sb
                # CE one-hot products fill the wait for the u-fold
                v.wait_ge(d_tg, 16)
                nc.vector.tensor_copy(out=tgf[:], in_=tg[:])
                v.drain()
                nc.vector.tensor_tensor(out=oh[:],
                                        in0=_ap3(tgf[:], bcast_inner=C)[:, 0, :],
                                        in1=iota100[:], op=AL.is_equal)
                v.drain()
                nc.vector.tensor_tensor(out=ohs[:], in0=oh[:], in1=sl_[:], op=AL.mult)
                # bucket chain straight from the u-fold PSUM (sole reader here)
                v.wait_ge(tsem, 1)        # u-fold done
                # bucket index in one op: the int32 output rounds the result
                nc.vector.tensor_scalar(ci[:], psumF[:], INVW, K / 2.0 - 0.5,
                                        AL.mult, AL.add).then_inc(vsem, 1)  # 9
                v.drain()
                nc.vector.tensor_scalar(hi_i[:], ci[:], 3, None,
                                        AL.arith_shift_right).then_inc(vsem, 1)  # 10
                nc.vector.tensor_scalar(lo_i[:], ci[:], 7, None, AL.bitwise_and)
                v.drain()
                nc.vector.tensor_copy(out=lo_b[:], in_=lo_i[:])
                v.drain()
                v.wait_ge(psem, 4)        # ireps
                for gi in range(NG):
                    cs = slice(GO[gi], GO[gi] + GS[gi])
                    nc.vector.tensor_tensor(
                        out=_gslice(lm2[:], K2, gi),
                        in0=_ap3(lo_b[:, cs], bcast_mid=K2),
                        in1=_mkap(irep8[:], [list(irep8[:].ap[0]),
                                             [72, K2], [1, GS[gi]]]),
                        op=AL.is_equal).then_inc(vsem, 1)   # 9,11,... (lm2)
                    if gi == 0:
                        v.wait_ge(psem, 7)    # hi_b (Pool, also implies ireps)
                    v.drain()
                    nc.vector.tensor_tensor(
                        out=_gslice(eg2[:], K1, gi),
                        in0=_ap3(hi_b[:, cs], bcast_mid=K1),
                        in1=_mkap(irep32[:], [list(irep32[:].ap[0]),
                                              [72, K1], [1, GS[gi]]]),
                        op=AL.is_equal).then_inc(vsem, 1)   # 10,12,... (eg2)
                # CE + KD finals (reduces went to ACT accum)
                v.wait_ge(asem, 10)       # cep/kdzd/ts accums
                nc.vector.tensor_sub(out=ceb[:], in0=lse_s[:, 0:1], in1=cep[:])
                v.drain()
                nc.vector.tensor_add(out=kdv[:], in0=k1act[:],
                                     in1=kdzd[:]).then_inc(vsem, 1)  # 21 (ceb+kdv)
                v.wait_ge(tsem, 6)        # histogram done
                nc.vector.tensor_reduce(out=r2[:, 0:1], in_=psumW[:, 0:K2],
                                        axis=AX.X, op=AL.add)
                nc.vector.tensor_reduce(out=r2[:, 1:2], in_=psumW[:, K2:2 * K2],
                                        axis=AX.X, op=AL.add)
                nc.vector.tensor_copy(out=w2sb[:], in_=psumW[:]).then_inc(vsem, 1)  # 22
                v.wait_ge(tsem, 7)        # wrev/t1 mms done
                nc.vector.tensor_copy(out=wrevsb[:], in_=psmall[:, SC_WR:SC_WR + 2 * K2])
                nc.vector.tensor_copy(out=t1sb[:], in_=psmall[:, SC_T1:SC_T1 + 2]).then_inc(vsem, 1)  # 23
                v.wait_ge(tsem, 8)        # U/S1 mms done
                nc.vector.tensor_tensor(out=p8[:, 0:K2], in0=m1lo[:],
                                        in1=psmall[0:K2, SC_UT:SC_UT + K2], op=AL.mult)
                nc.vector.tensor_tensor(out=p8[:, K2:2 * K2], in0=m1lo[:],
                                        in1=psmall[0:K2, SC_US:SC_US + K2], op=AL.mult)
                v.drain()
                nc.vector.tensor_reduce(out=s2v[:, 0:1], in_=p8[:, 0:K2],
                                        axis=AX.X, op=AL.add)
                nc.vector.tensor_reduce(out=s2v[:, 1:2], in_=p8[:, K2:2 * K2],
                                        axis=AX.X, op=AL.add).then_inc(vsem, 1)  # 24
                v.wait_ge(tsem, 10)       # all scalar mms done
                nc.vector.tensor_copy(out=sbs[:, 0:11], in_=psmall[0:1, 0:11])
                tt_, ss_, ts_ = sbs[:, 0:1], sbs[:, 1:2], sbs[:, 2:3]
                kd_, ce_ = sbs[:, 3:4], sbs[:, 4:5]
                sg_, sh_ = sbs[:, 5:6], sbs[:, 6:7]
                s1t, s1s, s2t, s2s = (sbs[:, i:i + 1] for i in range(7, 11))
                v.drain()
                nc.vector.tensor_add(out=fs[:, 0:1], in0=s1t, in1=s2t)
                nc.vector.tensor_add(out=fs[:, 1:2], in0=s1s, in1=s2s)
                nc.vector.tensor_mul(out=fs[:, 2:3], in0=tt_, in1=tt_)
                nc.vector.tensor_mul(out=fs[:, 3:4], in0=ss_, in1=ss_)
                nc.vector.tensor_mul(out=fs[:, 4:5], in0=ts_, in1=ts_)
                nc.vector.tensor_add(out=fs[:, 5:6], in0=sg_, in1=sh_)
                v.drain()
                nc.vector.tensor_sub(out=fs[:, 6:7], in0=fs[:, 0:1], in1=fs[:, 1:2])
                nc.vector.tensor_add(out=fs[:, 7:8], in0=fs[:, 2:3], in1=fs[:, 3:4])
                nc.vector.tensor_add(out=fs[:, 8:9], in0=fs[:, 5:6], in1=kd_)
                v.drain()
                nc.vector.scalar_tensor_tensor(out=fs[:, 9:10], in0=fs[:, 4:5],
                                               scalar=-2.0, in1=fs[:, 7:8],
                                               op0=AL.mult, op1=AL.add)
                nc.vector.scalar_tensor_tensor(out=fs[:, 10:11], in0=ce_,
                                               scalar=NT * (1.0 - ALPHA) / B,
                                               in1=fs[:, 8:9],
                                               op0=AL.mult, op1=AL.add)
                v.drain()
                nc.vector.scalar_tensor_tensor(out=fs[:, 11:12], in0=fs[:, 6:7],
                                               scalar=2.0, in1=fs[:, 9:10],
                                               op0=AL.mult, op1=AL.add)
                v.drain()
                nc.vector.scalar_tensor_tensor(out=fs[:, 0:1], in0=fs[:, 11:12],
                                               scalar=0.00025, in1=fs[:, 10:11],
                                               op0=AL.mult, op1=AL.add).then_inc(vsem, 1)  # 25

            # ---------------- PE ----------------
            # tsem: 1=u-fold 2=foldS 3=foldT 4=transposes 5=grams 6=hist
            #       7=wrev+t1 8=U+S1 9=S2+scalars
            @block.tensor
            def _(t):
                t.wait_ge(psem, 2)        # identities
                t.wait_ge(vsem, 2)        # ud (= (sl-tl)/T, no zd yet)
                nc.tensor.matmul(psumF[:], lhsT=e1b[:], rhs=u64[:, 0:HC],
                                 start=True, stop=False, skip_group_check=True)
                nc.tensor.matmul(psumF[:], lhsT=e2b[:], rhs=u64[:, HC:FLAT],
                                 start=False, stop=True, skip_group_check=True)
                # accumulate the per-(row, temp-block) lse bias into the fold:
                # rows 0:64 cover temps (1,1,2,2,3-head); rows 64:128 cover
                # (3-tail,4,4,5,5,pad). Broadcast APs over zd columns.
                t.wait_ge(vsem, 7)        # zd
                nc.tensor.matmul(psumF[:, 0:200], lhsT=e1b[:],
                                 rhs=_mkap(zdb[:], [list(zdb[:].ap[0]), [1, 2], [0, C]]),
                                 start=False, stop=False, skip_group_check=True)
                nc.tensor.matmul(psumF[:, 200:HC], lhsT=e1b[:],
                                 rhs=_mkap(zdb[:], [list(zdb[:].ap[0]), [0, HC - 200]],
                                           extra_off=2),
                                 start=False, stop=False, skip_group_check=True)
                nc.tensor.matmul(psumF[:, 0:44], lhsT=e2b[:],
                                 rhs=_mkap(zdb[:], [list(zdb[:].ap[0]), [0, 44]],
                                           extra_off=2),
                                 start=False, stop=False, skip_group_check=True)
                nc.tensor.matmul(psumF[:, 44:244], lhsT=e2b[:],
                                 rhs=_mkap(zdb[:], [list(zdb[:].ap[0]), [1, 2], [0, C]],
                                           extra_off=3),
                                 start=False, stop=True,
                                 skip_group_check=True).then_inc(tsem, 1)
                t.wait_ge(vsem, 8)        # cube_sb
                nc.tensor.matmul(psumFs[:], lhsT=e1b[:], rhs=cube_sb[:, 0:HC],
                                 start=True, stop=False, skip_group_check=True)
                nc.tensor.matmul(psumFs[:], lhsT=e2b[:], rhs=cube_sb[:, HC:FLAT],
                                 start=False, stop=True,
                                 skip_group_check=True).then_inc(tsem, 1)  # 2
                t.wait_ge(psem, 6)        # cube_tb
                t.wait_ge(vsem, 9)        # cf has finished reading psumF
                nc.tensor.matmul(psumF[:], lhsT=e1b[:], rhs=cube_tb[:, 0:HC],
                                 start=True, stop=False, skip_group_check=True)
                nc.tensor.matmul(psumF[:], lhsT=e2b[:], rhs=cube_tb[:, HC:FLAT],
                                 start=False, stop=True,
                                 skip_group_check=True).then_inc(tsem, 1)  # 3
                ins = None
                for k in range(NT):
                    slc = slice(k * C, (k + 1) * C)
                    nc.tensor.transpose(out=ptrT[:, k, :], in_=cube_tb[:, slc],
                                        identity=id64b[:])
                    ins = nc.tensor.transpose(out=ptrS[:, k, :], in_=cube_sb[:, slc],
                                              identity=id64b[:])
                ins.then_inc(tsem, 1)     # 4
                t.wait_ge(asem, 8)        # trT/trS/trSn
                t.wait_ge(psem, 5)        # cube_snb
                ins = None
                for k in range(NT):
                    slc = slice(k * C, (k + 1) * C)
                    nc.tensor.matmul(psum_hd[:, slc], lhsT=cube_tb[:, slc],
                                     rhs=cube_tb[:, slc], start=True, stop=False,
                                     skip_group_check=True)
                    nc.tensor.matmul(psum_hd[:, slc], lhsT=cube_snb[:, slc],
                                     rhs=cube_sb[:, slc], start=False, stop=True,
                                     skip_group_check=True)
                    nc.tensor.matmul(psum_gd[:, k * 64:(k + 1) * 64],
                                     lhsT=trT[:, k, :], rhs=trT[:, k, :],
                                     start=True, stop=False, skip_group_check=True)
                    ins = nc.tensor.matmul(psum_gd[:, k * 64:(k + 1) * 64],
                                           lhsT=trSn[:, k, :], rhs=trS[:, k, :],
                                           start=False, stop=True,
                                           skip_group_check=True)
                ins.then_inc(tsem, 1)     # 5
                ins = None
                for gi in range(NG):
                    t.wait_ge(vsem, 12 + 2 * gi)
                    t.wait_ge(psem, 10 + gi)
                    for i in range(GS[gi]):
                        ch = GO[gi] + i
                        ins = nc.tensor.matmul(psumW[:],
                                               lhsT=_chunkap(eg2[:], K1, ch),
                                               rhs=_chunkap(tsef2[:], 2 * K2, ch),
                                               start=(ch == 0), stop=(ch == NCHUNK - 1),
                                               skip_group_check=True)
                ins.then_inc(tsem, 1)     # 6: histogram done
                t.wait_ge(vsem, 22)       # w2sb + r2
                nc.tensor.matmul(psmall[:, SC_WR:SC_WR + 2 * K2], lhsT=prev32[:],
                                 rhs=w2sb[:], start=True, stop=True,
                                 skip_group_check=True)
                nc.tensor.matmul(psmall[:, SC_T1:SC_T1 + 2], lhsT=m1[:], rhs=r2[:],
                                 start=True, stop=True,
                                 skip_group_check=True).then_inc(tsem, 1)  # 7
                t.wait_ge(vsem, 23)       # wrevsb + t1sb
                nc.tensor.matmul(psmall[0:K2, SC_UT:SC_UT + K2],
                                 lhsT=w2sb[:, 0:K2], rhs=wrevsb[:, 0:K2],
                                 start=True, stop=True, skip_group_check=True)
                nc.tensor.matmul(psmall[0:K2, SC_US:SC_US + K2],
                                 lhsT=w2sb[:, K2:2 * K2], rhs=wrevsb[:, K2:2 * K2],
                                 start=True, stop=True, skip_group_check=True)
                nc.tensor.matmul(psmall[0:1, SC_S1T:SC_S1T + 1], lhsT=t1sb[:, 0:1],
                                 rhs=r2[:, 0:1], start=True, stop=True,
                                 skip_group_check=True)
                nc.tensor.matmul(psmall[0:1, SC_S1S:SC_S1S + 1], lhsT=t1sb[:, 1:2],
                                 rhs=r2[:, 1:2], start=True, stop=True,
                                 skip_group_check=True).then_inc(tsem, 1)  # 8
                t.wait_ge(asem, 7)        # tt128/ss128
                nc.tensor.matmul(psmall[0:1, SC_TT:SC_TT + 1], lhsT=ones[:, 0:1],
                                 rhs=tt128[:], start=True, stop=True,
                                 skip_group_check=True)
                nc.tensor.matmul(psmall[0:1, SC_SS:SC_SS + 1], lhsT=ones[:, 0:1],
                                 rhs=ss128[:], start=True, stop=True,
                                 skip_group_check=True)
                t.wait_ge(asem, 10)       # ts128
                nc.tensor.matmul(psmall[0:1, SC_TS:SC_TS + 1], lhsT=ones[:, 0:1],
                                 rhs=ts128[:], start=True, stop=True,
                                 skip_group_check=True)
                nc.tensor.matmul(psmall[0:1, SC_KD:SC_KD + 1], lhsT=ones[0:64, 0:1],
                                 rhs=kdv[:], start=True, stop=True,
                                 skip_group_check=True)
                t.wait_ge(vsem, 21)       # ceb+kdv
                nc.tensor.matmul(psmall[0:1, SC_CE:SC_CE + 1], lhsT=ones[0:64, 0:1],
                                 rhs=ceb[:], start=True, stop=True,
                                 skip_group_check=True)
                t.wait_ge(asem, 11)       # accg/acch
                nc.tensor.matmul(psmall[0:1, SC_SG:SC_SG + 1], lhsT=ones[0:64, 0:1],
                                 rhs=accg[:], start=True, stop=True,
                                 skip_group_check=True)
                nc.tensor.matmul(psmall[0:1, SC_SH:SC_SH + 1], lhsT=ones[0:100, 0:1],
                                 rhs=acch[:], start=True, stop=True,
                                 skip_group_check=True).then_inc(tsem, 1)  # 9: scalars
                t.wait_ge(vsem, 24)       # s2v
                nc.tensor.matmul(psmall[0:1, SC_S2T:SC_S2T + 1], lhsT=ones[0:K2, 0:1],
                                 rhs=s2v[:, 0:1], start=True, stop=True,
                                 skip_group_check=True)
                nc.tensor.matmul(psmall[0:1, SC_S2S:SC_S2S + 1], lhsT=ones[0:K2, 0:1],
                                 rhs=s2v[:, 1:2], start=True, stop=True,
                                 skip_group_check=True).then_inc(tsem, 1)  # 10: S2

    return nc


_cache = {}


def _get_nc():
    if "nc" not in _cache:
        _cache["nc"] = build()
    return _cache["nc"]


def kernel(logits_student, logits_teacher, target):
    from concourse.bass_utils import run_bass_kernel_spmd

    nc = _get_nc()
    in_map = {
        "logits_student": np.ascontiguousarray(logits_student, dtype=np.float32),
        "logits_teacher": np.ascontiguousarray(logits_teacher, dtype=np.float32),
        "target": np.ascontiguousarray(np.asarray(target).reshape(B, 1).astype(np.int32)),
    }
    core_ids = list(range(8))
    res = run_bass_kernel_spmd(nc, [in_map] * 8, core_ids)
    out = res.results[0]["out"]
    return np.float32(out.reshape(())).reshape(())
